# revision 1
# baseline (speedup 1.0000x reference)
"""Trainium2 Bass kernel for nn_EDTransformer (encoder-decoder transformer).

Sharding: 8 cores = 4 batch items x 2 sequence halves.
 - Each core owns (item b, half h): computes Q/scores/AV/Wo/MLP/LN for its
   256 local positions, K/V redundantly for the full 512 positions.
 - One 2-core AllGather of fp16 activations per layer (pairs share an item).
 - Unembedding sharded over vocab (4000 rows/core, 32 M-tiles of 125),
   softmax denominator via one 8-core AllReduce of (4,512) partial sums.
Dtypes: fp16 matmul operands (weights pre-transposed+cast on host),
 fp32 PSUM accumulation, fp32 residual stream + LN stats, fp32 output.
"""
import os
import sys

sys.path.insert(0, '/opt/trn_rl_repo')
import numpy as np

import concourse.bacc as bacc
import concourse.tile as tile
import concourse.mybir as mybir
from concourse.bass_utils import run_bass_kernel_spmd

DT = mybir.dt
F16 = DT.float16
F32 = DT.float32

N_CORES = 8
P = 128
DE = 1024          # model dim        (8 ptiles)
KO = DE // P       # 8
DMLP = 4096        # mlp dim          (32 ptiles)
MO = DMLP // P     # 32
H = 16             # heads
DA = 64            # attn dim per head
L = 512            # sequence length
LL = 256           # local positions per core
NV = 32000
NVC = NV // N_CORES  # 4000 vocab rows per core
UM = 125           # vocab M-tile
UMT = NVC // UM    # 32 M-tiles
LENC = 2
LDEC = 2
EPS = 1e-5

PAIR_GROUPS = [[0, 1], [2, 3], [4, 5], [6, 7]]
ALL_GROUP = [list(range(N_CORES))]

_CACHE = {}


# ----------------------------------------------------------------------------
# device program
# ----------------------------------------------------------------------------

def _attn(nc, tc, pools, Eres32, qin16, kvin16, wq_d, wk_d, wv_d, wo_d,
          mask, name):
    """One multi-head attention block; accumulates Wo output into Eres32.

    qin16  : [128, KO, LL] fp16  local stream (query input)
    kvin16 : [128, KO, L]  fp16  full-sequence stream (key/value input)
    wq_d/wk_d/wv_d/wo_d: dram APs [128, KO, 1024] (pre-transposed; wq scaled)
    mask   : [128, 4, LL] fp16 sbuf tile or None
    """
    sb = pools['att']
    p256 = pools['p256']
    p512 = pools['p512']
    ones = pools['ones']
    KT = L // P  # 4 kz tiles

    # q: [128(2h x 64a), pr, LL]
    q16 = sb.tile([P, KO, LL], F16, tag='q16')
    for pr in range(KO):
        wt = pools['wqp'].tile([P, KO, P], F16, tag='wqt')
        nc.sync.dma_start(wt[:], wq_d[:, :, pr * P:(pr + 1) * P])
        ps = p256.tile([P, LL], F32, tag='p256')
        for k in range(KO):
            nc.tensor.matmul(ps[:], wt[:, k, :],
                             qin16[:, k, :], start=(k == 0), stop=(k == KO - 1))
        nc.vector.tensor_copy(q16[:, pr, :], ps[:])
    # k: [128(2h x 64a), pr, L]
    k16 = sb.tile([P, KO, L], F16, tag='k16')
    for pr in range(KO):
        wt = pools['wkp'].tile([P, KO, P], F16, tag='wkt')
        nc.sync.dma_start(wt[:], wk_d[:, :, pr * P:(pr + 1) * P])
        ps = p512.tile([P, L], F32, tag='p512')
        for k in range(KO):
            nc.tensor.matmul(ps[:], wt[:, k, :],
                             kvin16[:, k, :], start=(k == 0), stop=(k == KO - 1))
        nc.vector.tensor_copy(k16[:, pr, :], ps[:])
    # vT: [128(kz), kt, 1024(h*64+o)]
    vt16 = sb.tile([P, KT, H * DA], F16, tag='vt16')
    for nch in range(2):
        wt = pools['wvp'].tile([P, KO, 512], F16, tag='wvt')
        nc.sync.dma_start(wt[:], wv_d[:, :, nch * 512:(nch + 1) * 512])
        for kt in range(KT):
            ps = p512.tile([P, 512], F32, tag='p512')
            for k in range(KO):
                nc.tensor.matmul(ps[:], kvin16[:, k, kt * P:(kt + 1) * P],
                                 wt[:, k, :],
                                 start=(k == 0), stop=(k == KO - 1))
            nc.vector.tensor_copy(vt16[:, kt, nch * 512:(nch + 1) * 512], ps[:])

    # scores -> exp -> (mask) ; exp16: [128(kz), h, kt, LL]
    exp16 = sb.tile([P, H, KT, LL], F16, tag='exp16')
    for h in range(H):
        pr, hp = h // 2, (h % 2) * DA
        for kt in range(KT):
            ps = p256.tile([P, LL], F32, tag='p256')
            nc.tensor.matmul(ps[:], k16[hp:hp + DA, pr, kt * P:(kt + 1) * P],
                             q16[hp:hp + DA, pr, :], start=True, stop=True)
            nc.scalar.activation(exp16[:, h, kt, :], ps[:],
                                 mybir.ActivationFunctionType.Exp)
    if mask is not None:
        for kt in range(KT):
            nc.vector.tensor_tensor(
                exp16[:, :, kt, :], exp16[:, :, kt, :],
                mask[:, kt, None, :].to_broadcast((P, H, LL)),
                mybir.AluOpType.mult)

    # AV with 2-head column packing + per-head normalize.
    # deno: ones-matmul with M=128 -> every partition row holds the colsums.
    y16 = sb.tile([P, KO, LL], F16, tag='y16')
    for pr in range(KO):
        hA, hB = 2 * pr, 2 * pr + 1
        pd = p512.tile([P, 2 * LL], F32, tag='p512')
        for kt in range(KT):
            nc.tensor.matmul(pd[:], ones[:, :],
                             exp16[:, hA:hB + 1, kt, :],
                             start=(kt == 0), stop=(kt == KT - 1))
        ysc = sb.tile([P, 2, LL], F32, tag='ysc')
        nc.vector.reciprocal(ysc[:], pd[:])
        ps = p256.tile([P, LL], F32, tag='p256')
        for kt in range(KT):
            nc.tensor.matmul(ps[:DA, :], vt16[:, kt, hA * DA:(hA + 1) * DA],
                             exp16[:, hA, kt, :], start=(kt == 0),
                             stop=(kt == KT - 1), tile_position=(0, 0))
            nc.tensor.matmul(ps[DA:, :], vt16[:, kt, hB * DA:(hB + 1) * DA],
                             exp16[:, hB, kt, :], start=(kt == 0),
                             stop=(kt == KT - 1), tile_position=(0, DA))
        nc.vector.tensor_tensor(y16[:DA, pr, :], ps[:DA, :], ysc[:DA, 0, :],
                                mybir.AluOpType.mult)
        nc.vector.tensor_tensor(y16[DA:, pr, :], ps[DA:, :], ysc[DA:, 1, :],
                                mybir.AluOpType.mult)

    # Wo -> accumulate into residual
    for dt in range(KO):
        wt = pools['wop'].tile([P, KO, P], F16, tag='wot')
        nc.sync.dma_start(wt[:], wo_d[:, :, dt * P:(dt + 1) * P])
        ps = p256.tile([P, LL], F32, tag='p256')
        for k in range(KO):
            nc.tensor.matmul(ps[:], wt[:, k, :],
                             y16[:, k, :], start=(k == 0), stop=(k == KO - 1))
        nc.vector.tensor_tensor(Eres32[:, dt, :], Eres32[:, dt, :], ps[:],
                                mybir.AluOpType.add)
    tp = pools.get('tapfn')
    if tp:
        tp(f'{name}_q', q16); tp(f'{name}_k', k16); tp(f'{name}_vt', vt16)
        tp(f'{name}_exp', exp16); tp(f'{name}_y', y16)


def _mlp(nc, tc, pools, Eres32, ein16, w1_dram, w2_dram, name):
    p256 = pools['p256']
    h16 = pools['mlp'].tile([P, MO, LL], F16, tag='h16')
    for mt in range(MO):
        w1t = pools['w1p'].tile([P, KO, P], F16, tag='w1t')
        nc.sync.dma_start(w1t[:], w1_dram[:, :, mt * P:(mt + 1) * P])
        ps = p256.tile([P, LL], F32, tag='p256')
        for k in range(KO):
            nc.tensor.matmul(ps[:], w1t[:, k, :], ein16[:, k, :],
                             start=(k == 0), stop=(k == KO - 1))
        nc.scalar.activation(h16[:, mt, :], ps[:],
                             mybir.ActivationFunctionType.Relu)
    for dt in range(KO):
        w2t = pools['w2p'].tile([P, MO, P], F16, tag='w2t')
        nc.sync.dma_start(w2t[:], w2_dram[:, :, dt * P:(dt + 1) * P])
        ps = p256.tile([P, LL], F32, tag='p256')
        for k in range(MO):
            nc.tensor.matmul(ps[:], w2t[:, k, :], h16[:, k, :],
                             start=(k == 0), stop=(k == MO - 1))
        nc.vector.tensor_tensor(Eres32[:, dt, :], Eres32[:, dt, :], ps[:],
                                mybir.AluOpType.add)


def _ln(nc, tc, pools, Eres32, e16out, name):
    """In-place layernorm over features; writes fp16 copy to e16out."""
    p256 = pools['p256']
    ones = pools['ones']
    stat = pools['stat']

    e16pre = pools['lnp'].tile([P, KO, LL], F16, tag='e16pre')
    nc.vector.tensor_copy(e16pre[:], Eres32[:])
    sq16 = pools['lnp'].tile([P, KO, LL], F16, tag='sq16')
    nc.vector.tensor_tensor(sq16[:], e16pre[:], e16pre[:],
                            mybir.AluOpType.mult)
    # sums with M=128 ones -> replicated rows; stats stay [128, LL]
    pss = p256.tile([P, LL], F32, tag='p256')
    psq = p256.tile([P, LL], F32, tag='p256')
    for k in range(KO):
        nc.tensor.matmul(pss[:], ones[:, :], e16pre[:, k, :],
                         start=(k == 0), stop=(k == KO - 1))
    for k in range(KO):
        nc.tensor.matmul(psq[:], ones[:, :], sq16[:, k, :],
                         start=(k == 0), stop=(k == KO - 1))
    mean = stat.tile([P, LL], F32, tag='mean')
    nc.vector.tensor_scalar_mul(mean[:], pss[:], 1.0 / DE)
    var = stat.tile([P, LL], F32, tag='var')
    nc.vector.tensor_scalar_mul(var[:], psq[:], 1.0 / DE)
    msq = stat.tile([P, LL], F32, tag='msq')
    nc.vector.tensor_tensor(msq[:], mean[:], mean[:], mybir.AluOpType.mult)
    nc.vector.tensor_tensor(var[:], var[:], msq[:], mybir.AluOpType.subtract)
    nc.vector.tensor_scalar_mul(var[:], var[:], float(DE) / (DE - 1))
    std = stat.tile([P, LL], F32, tag='std')
    nc.scalar.activation(std[:], var[:], mybir.ActivationFunctionType.Sqrt,
                         bias=pools['eps128'])
    inv = stat.tile([P, LL], F32, tag='inv')
    nc.vector.reciprocal(inv[:], std[:])
    negms = stat.tile([P, LL], F32, tag='negms')
    nc.vector.tensor_tensor(negms[:], mean[:], inv[:], mybir.AluOpType.mult)
    nc.vector.tensor_scalar_mul(negms[:], negms[:], -1.0)
    nc.vector.tensor_tensor(
        Eres32[:], Eres32[:],
        inv[:, None, :].to_broadcast((P, KO, LL)), mybir.AluOpType.mult)
    nc.vector.tensor_tensor(
        Eres32[:], Eres32[:],
        negms[:, None, :].to_broadcast((P, KO, LL)), mybir.AluOpType.add)
    nc.vector.tensor_copy(e16out[:], Eres32[:])
    tp = pools.get('tapfn')
    if tp:
        tp(f'{name}_out', Eres32)


def _allgather_pair(nc, tc, pools, e16loc, full16, agin, agout, tag):
    """e16loc [128, KO, LL] -> pair AllGather -> full16 [128, KO, L]."""
    nc.gpsimd.dma_start(agin[:], e16loc[:])
    nc.gpsimd.collective_compute(
        "AllGather", mybir.AluOpType.bypass,
        ins=[agin[:]], outs=[agout[:]],
        replica_groups=PAIR_GROUPS)
    nc.gpsimd.dma_start(
        full16[:].rearrange('ki ko (r p) -> ki ko r p', r=2),
        agout[:].rearrange('r ki ko p -> ki ko r p'))


def build_program(taps=()):
    taps = set(taps)
    nc = bacc.Bacc("TRN2", target_bir_lowering=False, debug=False,
                   num_devices=N_CORES)

    # ---- dram inputs ----
    din = {}
    def dram_in(nm, shape, dt=F16):
        din[nm] = nc.dram_tensor(nm, list(shape), dt, kind="ExternalInput")
        return din[nm]

    z0f = dram_in('z0_full16', [P, KO, L])
    x0f = dram_in('x0_full16', [P, KO, L])
    z0l32 = dram_in('z0_loc32', [P, KO, LL], F32)
    x0l32 = dram_in('x0_loc32', [P, KO, LL], F32)
    z0l16 = dram_in('z0_loc16', [P, KO, LL])
    x0l16 = dram_in('x0_loc16', [P, KO, LL])
    mask_self = dram_in('mask_self', [P, 4, LL])
    for pfx, nl in (('enc', LENC), ('dec', LDEC)):
        for w in ('wqT', 'wkT', 'wvT', 'woT'):
            dram_in(f'{pfx}_{w}', [nl, P, KO, DE])
        dram_in(f'{pfx}_w1T', [nl, P, KO, DMLP])
        dram_in(f'{pfx}_w2T', [nl, P, MO, DE])
    wuT = dram_in('wuT', [P, KO, NVC])

    outp = nc.dram_tensor('outp', [UMT, UM, 4, L], F32, kind="ExternalOutput")

    # internal dram for collectives
    agin = nc.dram_tensor('agin', [P, KO, LL], F16)
    agout = nc.dram_tensor('agout', [2, P, KO, LL], F16)
    ag8in = nc.dram_tensor('ag8in', [P, KO, LL], F16)
    ag8out = nc.dram_tensor('ag8out', [N_CORES, P, KO, LL], F16,
                            addr_space='Shared')
    arin = nc.dram_tensor('arin', [1, 4, L], F32)
    arout = nc.dram_tensor('arout', [1, 4, L], F32, addr_space="Shared")
    dspd = nc.dram_tensor('dspd', [1, 4, L], F32)


    import contextlib
    with tile.TileContext(nc) as tc, contextlib.ExitStack() as octx:
        const = octx.enter_context(tc.tile_pool(name='const', bufs=1))
        ones = const.tile([P, P], F16)
        nc.vector.memset(ones[:], 1.0)
        eps1 = const.tile([1, 1], F32)
        nc.vector.memset(eps1[:], EPS)
        eps128 = const.tile([P, 1], F32)
        nc.vector.memset(eps128[:], EPS)
        ones32 = const.tile([1, P], F32)
        nc.vector.memset(ones32[:], 1.0)
        msk = const.tile([P, 4, LL], F16)
        nc.sync.dma_start(msk[:], mask_self[:])

        # ================= layer phase =================
        with contextlib.ExitStack() as ctx:
            stream = ctx.enter_context(tc.tile_pool(name='stream', bufs=1))
            att = ctx.enter_context(tc.tile_pool(name='att', bufs=1))
            mlpp = ctx.enter_context(tc.tile_pool(name='mlpp', bufs=1))
            lnp = ctx.enter_context(tc.tile_pool(name='lnp', bufs=1))
            stat = ctx.enter_context(tc.tile_pool(name='stat', bufs=1))
            wqp = ctx.enter_context(tc.tile_pool(name='wqp', bufs=3))
            wkp = ctx.enter_context(tc.tile_pool(name='wkp', bufs=3))
            wvp = ctx.enter_context(tc.tile_pool(name='wvp', bufs=2))
            wop = ctx.enter_context(tc.tile_pool(name='wop', bufs=3))
            w1p = ctx.enter_context(tc.tile_pool(name='w1p', bufs=4))
            w2p = ctx.enter_context(tc.tile_pool(name='w2p', bufs=2))
            p256 = ctx.enter_context(tc.tile_pool(name='p256', bufs=5,
                                                  space='PSUM'))
            p512 = ctx.enter_context(tc.tile_pool(name='p512', bufs=3,
                                                  space='PSUM'))

            pools = dict(att=att, mlp=mlpp, lnp=lnp, p256=p256, p512=p512,
                         stat=stat, ones=ones, ones32=ones32,
                         eps1=eps1, eps128=eps128[:], wqp=wqp, wkp=wkp,
                         wvp=wvp, wop=wop, w1p=w1p, w2p=w2p)

            def tapfn(nm, t):
                if nm not in taps:
                    return
                d = nc.dram_tensor('tap_' + nm, list(t.shape),
                                   t.dtype, kind="ExternalOutput")
                nc.sync.dma_start(d[:], t[:])
            pools['tapfn'] = tapfn

            # ======== encoder ========
            Eres = stream.tile([P, KO, LL], F32, tag='res')
            nc.sync.dma_start(Eres[:], z0l32[:])
            Zfull = stream.tile([P, KO, L], F16, tag='Zfull')
            nc.sync.dma_start(Zfull[:], z0f[:])
            eloc = stream.tile([P, KO, LL], F16, tag='loc_a')
            nc.sync.dma_start(eloc[:], z0l16[:])

            for l in range(LENC):
                _attn(nc, tc, pools, Eres, eloc, Zfull,
                      din['enc_wqT'][l], din['enc_wkT'][l],
                      din['enc_wvT'][l], din['enc_woT'][l], None, f'e{l}a')
                eloc = stream.tile([P, KO, LL], F16, tag='loc_b')
                _ln(nc, tc, pools, Eres, eloc, f'e{l}ln1')
                _mlp(nc, tc, pools, Eres, eloc, din['enc_w1T'][l],
                     din['enc_w2T'][l], f'e{l}m')
                eloc = stream.tile([P, KO, LL], F16, tag='loc_a')
                _ln(nc, tc, pools, Eres, eloc, f'e{l}ln2')
                Zfull = stream.tile([P, KO, L], F16, tag='Zfull')
                _allgather_pair(nc, tc, pools, eloc, Zfull, agin, agout,
                                f'e{l}')


            # ======== decoder ========
            Eres = stream.tile([P, KO, LL], F32, tag='res')
            nc.sync.dma_start(Eres[:], x0l32[:])
            Xfull = stream.tile([P, KO, L], F16, tag='Xfull')
            nc.sync.dma_start(Xfull[:], x0f[:])
            eloc = stream.tile([P, KO, LL], F16, tag='loc_a')
            nc.sync.dma_start(eloc[:], x0l16[:])

            for l in range(LDEC):
                _attn(nc, tc, pools, Eres, eloc, Xfull,
                      din['dec_wqT'][l], din['dec_wkT'][l],
                      din['dec_wvT'][l], din['dec_woT'][l], msk, f'd{l}s')
                eloc = stream.tile([P, KO, LL], F16, tag='loc_b')
                _ln(nc, tc, pools, Eres, eloc, f'd{l}ln1')
                _attn(nc, tc, pools, Eres, eloc, Zfull,
                      din['dec_wqT'][l], din['dec_wkT'][l],
                      din['dec_wvT'][l], din['dec_woT'][l], None, f'd{l}c')
                eloc = stream.tile([P, KO, LL], F16, tag='loc_c')
                _ln(nc, tc, pools, Eres, eloc, f'd{l}ln2')
                _mlp(nc, tc, pools, Eres, eloc, din['dec_w1T'][l],
                     din['dec_w2T'][l], f'd{l}m')
                eloc = stream.tile([P, KO, LL], F16, tag='loc_a')
                _ln(nc, tc, pools, Eres, eloc, f'd{l}ln3')
                if l < LDEC - 1:
                    Xfull = stream.tile([P, KO, L], F16, tag='Xfull')
                    _allgather_pair(nc, tc, pools, eloc, Xfull, agin, agout,
                                    f'd{l}')


            # send final local activations for the 8-core gather
            nc.gpsimd.dma_start(ag8in[:], eloc[:])

        # ================= unembed phase =================
        nc.gpsimd.collective_compute(
            "AllGather", mybir.AluOpType.bypass,
            ins=[ag8in[:]], outs=[ag8out[:]], replica_groups=ALL_GROUP)

        with contextlib.ExitStack() as ctx:
            usb = ctx.enter_context(tc.tile_pool(name='usb', bufs=1))
            ures_p = ctx.enter_context(tc.tile_pool(name='ures_p', bufs=2))
            wup = ctx.enter_context(tc.tile_pool(name='wup', bufs=3))
            u512 = ctx.enter_context(tc.tile_pool(name='u512', bufs=4,
                                                  space='PSUM'))
            udeno = ctx.enter_context(tc.tile_pool(name='udeno', bufs=1,
                                                   space='PSUM'))

            XF = usb.tile([P, KO, 4 * L], F16, tag='XF')
            nc.sync.dma_start(
                XF[:].rearrange('ki ko (r p) -> ki ko r p', r=N_CORES),
                ag8out[:].rearrange('r ki ko p -> ki ko r p'))

            if 'xf' in taps:
                d = nc.dram_tensor('tap_xf', [P, KO, 4 * L], F16,
                                   kind="ExternalOutput")
                nc.sync.dma_start(d[:], XF[:])
            expu = usb.tile([P, UMT, 4, L], F16, tag='expu')
            pdeno = [udeno.tile([1, L], F32, tag=f'pdeno{j}',
                                name=f'pdeno{j}') for j in range(4)]
            for mt in range(UMT):
                wut = wup.tile([P, KO, UM], F16, tag='wut')
                nc.sync.dma_start(wut[:], wuT[:, :, mt * UM:(mt + 1) * UM])
                pss = [u512.tile([P, 512], F32, tag='u512', name=f'ups{j}')
                       for j in range(4)]
                # k-outer: each lhsT load serves 4 matmuls
                for k in range(KO):
                    for j in range(4):
                        nc.tensor.matmul(pss[j][:UM, :], wut[:, k, :],
                                         XF[:, k, j * L:(j + 1) * L],
                                         start=(k == 0), stop=(k == KO - 1))
                for j in range(4):
                    nc.scalar.activation(expu[:UM, mt, j, :], pss[j][:UM, :],
                                         mybir.ActivationFunctionType.Exp)
                    nc.tensor.matmul(pdeno[j][:], ones[:UM, :1],
                                     expu[:UM, mt, j, :],
                                     start=(mt == 0), stop=(mt == UMT - 1))
            deno = usb.tile([1, 4, L], F32, tag='deno')
            for j in range(4):
                nc.vector.tensor_copy(deno[:, j, :], pdeno[j][:])
            nc.gpsimd.dma_start(arin[:], deno[:])
            nc.gpsimd.collective_compute(
                "AllReduce", mybir.AluOpType.add,
                ins=[arin[:]], outs=[arout[:]], replica_groups=ALL_GROUP)
            if 'deno' in taps:
                d = nc.dram_tensor('tap_deno', [1, 4, L], F32,
                                   kind="ExternalOutput")
                nc.sync.dma_start(d[:], deno[:])
            if 'expu' in taps:
                d = nc.dram_tensor('tap_expu', [P, 4, 4, L], F16,
                                   kind="ExternalOutput")
                nc.sync.dma_start(d[:], expu[:, 0:4, :, :])
            # spread the 2048 reciprocals across partitions (via dram views)
            dsp = usb.tile([P, 16], F32, tag='dsp')  # 128 part x 16
            nc.sync.dma_start(
                dsp[:], arout[:].rearrange('o a (p n) -> (o a p) n', p=32))
            nc.vector.reciprocal(dsp[:], dsp[:])
            nc.sync.dma_start(
                dspd[:].rearrange('o a (p n) -> (o a p) n', p=32), dsp[:])
            nc.sync.dma_start(deno[:], dspd[:])
            binv = usb.tile([P, 4, L], F32, tag='binv')
            for j in range(4):
                pb = u512.tile([P, L], F32, tag='u512')
                nc.tensor.matmul(pb[:], ones32[:1, :], deno[:, j, :],
                                 start=True, stop=True)
                nc.vector.tensor_copy(binv[:, j, :], pb[:])
            dmae = [nc.sync, nc.gpsimd, nc.scalar, nc.gpsimd]
            for mt in range(UMT):
                res = ures_p.tile([P, 4, L], F32, tag='ures')
                nc.vector.tensor_tensor(res[:UM], expu[:UM, mt], binv[:UM],
                                        mybir.AluOpType.mult)
                for j in range(4):
                    dmae[j].dma_start(outp[mt, :, j], res[:UM, j])

    nc.compile()
    return nc


# ----------------------------------------------------------------------------
# host-side prep
# ----------------------------------------------------------------------------

def _to_kimaj(a):
    """[K, M] -> [128, K//128, M] with K = ko*128 + ki."""
    K, M = a.shape
    return np.ascontiguousarray(
        a.reshape(K // P, P, M).transpose(1, 0, 2))


def prep_inputs(inputs):
    f = lambda k: np.asarray(inputs[k], dtype=np.float32)
    We, Wp, Wu = f('We'), f('Wp'), f('Wu')
    x = np.asarray(inputs['x']).astype(np.int64)
    z = np.asarray(inputs['z']).astype(np.int64)

    shared = {}
    for pfx, nl in (('enc', LENC), ('dec', LDEC)):
        Wq, Wk, Wv = f(pfx + '_Wq'), f(pfx + '_Wk'), f(pfx + '_Wv')
        Wo, W1, W2 = f(pfx + '_Wo'), f(pfx + '_W1'), f(pfx + '_W2')
        wq, wk, wv, wo, w1, w2 = [], [], [], [], [], []
        for l in range(nl):
            qa = Wq[l].transpose(2, 0, 1).reshape(DE, H * DA) * (DA ** -0.5)
            ka = Wk[l].transpose(2, 0, 1).reshape(DE, H * DA)
            va = Wv[l].transpose(2, 0, 1).reshape(DE, H * DA)
            wq.append(_to_kimaj(qa)); wk.append(_to_kimaj(ka))
            wv.append(_to_kimaj(va))
            wo.append(_to_kimaj(Wo[l].T))
            w1.append(_to_kimaj(W1[l].T))
            w2.append(_to_kimaj(W2[l].T))
        shared[f'{pfx}_wqT'] = np.stack(wq).astype(np.float16)
        shared[f'{pfx}_wkT'] = np.stack(wk).astype(np.float16)
        shared[f'{pfx}_wvT'] = np.stack(wv).astype(np.float16)
        shared[f'{pfx}_woT'] = np.stack(wo).astype(np.float16)
        shared[f'{pfx}_w1T'] = np.stack(w1).astype(np.float16)
        shared[f'{pfx}_w2T'] = np.stack(w2).astype(np.float16)

    pos = Wp[:L]  # [512, 1024]
    in_maps = []
    for c in range(N_CORES):
        b, h = c // 2, c % 2
        m = dict(shared)
        for nm, tok in (('z0', z[b]), ('x0', x[b])):
            E0 = (We[tok] + pos).T.astype(np.float32)      # [1024, 512]
            E0k = E0.reshape(KO, P, L)                     # [ko, ki, p]
            m[nm + '_full16'] = np.ascontiguousarray(
                E0k.transpose(1, 0, 2)).astype(np.float16)
            loc = E0k[:, :, h * LL:(h + 1) * LL].transpose(1, 0, 2)
            m[nm + '_loc32'] = np.ascontiguousarray(loc)
            m[nm + '_loc16'] = np.ascontiguousarray(loc).astype(np.float16)
        kglob = np.arange(L)[:, None]
        qglob = (h * LL + np.arange(LL))[None, :]
        msk = (kglob <= qglob).astype(np.float16)          # [512, 256]
        m['mask_self'] = np.ascontiguousarray(
            msk.reshape(4, P, LL).transpose(1, 0, 2))
        wus = Wu[c * NVC:(c + 1) * NVC].T                  # [1024, 4000]
        m['wuT'] = _to_kimaj(wus).astype(np.float16)
        in_maps.append(m)
    return in_maps


def assemble(results):
    """results: list of per-core dicts with 'outp' [UMT, UM, 4, L] fp32."""
    out = np.empty((4, NV, L), dtype=np.float32)
    for c, r in enumerate(results):
        o = r['outp']                                     # [32, 125, 4, 512]
        o = o.reshape(NVC, 4, L).transpose(1, 0, 2)       # [4, 4000, 512]
        out[:, c * NVC:(c + 1) * NVC, :] = o
    return out


def run(inputs, trace=False, taps=(), trace_kwargs=None):
    key = ('prog', tuple(sorted(taps)))
    if key not in _CACHE:
        _CACHE[key] = build_program(taps=taps)
    nc = _CACHE[key]
    in_maps = prep_inputs(inputs)
    res = run_bass_kernel_spmd(nc, in_maps, list(range(N_CORES)),
                               trace=trace, **(trace_kwargs or {}))
    return res


def kernel(**inputs):
    res = run(inputs, trace=False)
    return assemble(res.results)



# revision 9
# speedup vs baseline: 1.1585x; 1.1585x over previous
"""Trainium2 Bass kernel for nn_EDTransformer (encoder-decoder transformer).

Sharding: 8 cores = 4 batch items x 2 sequence halves.
 - Each core owns (item b, half h): computes Q/scores/AV/Wo/MLP/LN for its
   256 local positions, K/V redundantly for the full 512 positions.
 - One 2-core AllGather of fp16 activations per layer (pairs share an item).
 - Unembedding: pair-split over vocab (16000 rows/core), transposed layout
   (tokens on partitions): softmax denominator via activation accum_out,
   one tiny pair AllReduce, per-partition tensor_scalar normalize.
Dtypes: fp16 residual stream + matmul operands, fp32 PSUM/stats, fp16 output
 scaled by 2^8 (host divides it back out).
"""
import os
import sys

sys.path.insert(0, '/opt/trn_rl_repo')
import numpy as np

import concourse.bacc as bacc
import concourse.tile as tile
import concourse.mybir as mybir
from concourse.bass_utils import run_bass_kernel_spmd

DT = mybir.dt
F16 = DT.float16
F32 = DT.float32
ALU = mybir.AluOpType
AF = mybir.ActivationFunctionType

N_CORES = 8
P = 128
DE = 1024          # model dim        (8 ptiles)
KO = DE // P       # 8
DMLP = 4096        # mlp dim          (32 ptiles)
MO = DMLP // P     # 32
H = 16             # heads
DA = 64            # attn dim per head
L = 512            # sequence length
LL = 256           # local positions per core
KT = L // P        # 4 key tiles
NV = 32000
NVH = NV // 2      # 16000 vocab rows per core (pair-split)
UV = 500           # vocab chunk width
UVC = NVH // UV    # 32 chunks
QT = L // P        # 4 query tiles of 128
LENC = 2
LDEC = 2
EPS = 1e-5
OSCALE = 256.0     # output scale (keeps fp16 out of subnormals)

PAIR_GROUPS = [[0, 1], [2, 3], [4, 5], [6, 7]]

_CACHE = {}


# ----------------------------------------------------------------------------
# device program
# ----------------------------------------------------------------------------

def _attn(nc, tc, pools, Eres16, kvin16, wq_d, wk_d, wv_d, wo_d, mask, name,
          split=None):
    """One multi-head attention block; accumulates Wo output into Eres16.

    Eres16 : [128, KO, LL] fp16  local stream (query input, residual)
    kvin16 : [128, KO, L]  fp16  full-sequence stream (key/value input)
    wq_d/wk_d/wv_d/wo_d: dram APs [128, KO, 1024] (pre-transposed; wq scaled)
    mask   : [128, 4, LL] fp16 sbuf tile or None
    split  : if 'qkv', only emit q/k/v projections and return state;
             if a dict, resume from that state (scores onward).
    """
    sb = pools['att']
    p256 = pools['p256']
    p512 = pools['p512']
    ones = pools['ones']

    if split is None or split == 'qkv':
        # q: [128(2h x 64a), pr, LL]
        q16 = sb.tile([P, KO, LL], F16, tag=name[0] + 'q16')
        for pr in range(KO):
            wt = pools['wqp'].tile([P, KO, P], F16, tag='wqt')
            nc.sync.dma_start(wt[:], wq_d[:, :, pr * P:(pr + 1) * P])
            ps = p256.tile([P, LL], F32, tag='p256')
            for k in range(KO):
                nc.tensor.matmul(ps[:], wt[:, k, :],
                                 Eres16[:, k, :], start=(k == 0),
                                 stop=(k == KO - 1))
            nc.vector.tensor_copy(q16[:, pr, :], ps[:])
        # k: [128(2h x 64a), pr, L]
        k16 = sb.tile([P, KO, L], F16, tag=name[0] + 'k16')
        for pr in range(KO):
            wt = pools['wkp'].tile([P, KO, P], F16, tag='wkt')
            nc.sync.dma_start(wt[:], wk_d[:, :, pr * P:(pr + 1) * P])
            ps = p512.tile([P, L], F32, tag='p512')
            for k in range(KO):
                nc.tensor.matmul(ps[:], wt[:, k, :],
                                 kvin16[:, k, :], start=(k == 0),
                                 stop=(k == KO - 1))
            nc.vector.tensor_copy(k16[:, pr, :], ps[:])
        # vT: [128(kz), kt, 1024(h*64+o)]
        vt16 = sb.tile([P, KT, H * DA], F16, tag=name[0] + 'vt16')
        for nch in range(2):
            wt = pools['wvp'].tile([P, KO, 512], F16, tag='wvt')
            nc.sync.dma_start(wt[:], wv_d[:, :, nch * 512:(nch + 1) * 512])
            for kt in range(KT):
                ps = p512.tile([P, 512], F32, tag='p512')
                for k in range(KO):
                    nc.tensor.matmul(ps[:], kvin16[:, k, kt * P:(kt + 1) * P],
                                     wt[:, k, :],
                                     start=(k == 0), stop=(k == KO - 1))
                nc.vector.tensor_copy(vt16[:, kt, nch * 512:(nch + 1) * 512],
                                      ps[:])
        if split == 'qkv':
            return dict(q16=q16, k16=k16, vt16=vt16)
    else:
        q16, k16, vt16 = split['q16'], split['k16'], split['vt16']

    # per 2-head group: scores -> exp -> (mask) -> deno/recip -> AV -> y
    y16 = sb.tile([P, KO, LL], F16, tag=name[0] + 'y16')
    for pr in range(KO):
        hA = 2 * pr
        expg = pools['attg'].tile([P, 2, KT, LL], F16, tag='expg')
        for h2 in range(2):
            hp = h2 * DA
            for ktp in range(2):
                ps = p512.tile([P, 2, LL], F32, tag='p512')
                for j in range(2):
                    kt = 2 * ktp + j
                    nc.tensor.matmul(ps[:, j, :],
                                     k16[hp:hp + DA, pr, kt * P:(kt + 1) * P],
                                     q16[hp:hp + DA, pr, :],
                                     start=True, stop=True)
                nc.scalar.activation(expg[:, h2, 2 * ktp:2 * ktp + 2, :],
                                     ps[:], AF.Exp)
        if mask is not None:
            for kt in range(KT):
                nc.vector.tensor_tensor(
                    expg[:, :, kt, :], expg[:, :, kt, :],
                    mask[:, kt, None, :].to_broadcast((P, 2, LL)),
                    ALU.mult)
        # deno (2 heads packed on free dim) then fast reciprocal
        pd = p512.tile([P, 2, LL], F32, tag='p512')
        for kt in range(KT):
            nc.tensor.matmul(pd[:], ones[:, :], expg[:, :, kt, :],
                             start=(kt == 0), stop=(kt == KT - 1))
        ysc = pools['stat'].tile([P, 2, LL], F32, tag='ysc')
        nc.vector.reciprocal_approx_fast(ysc[:], pd[:])
        ps = p256.tile([P, LL], F32, tag='p256')
        for kt in range(KT):
            nc.tensor.matmul(ps[:DA, :], vt16[:, kt, hA * DA:(hA + 1) * DA],
                             expg[:, 0, kt, :], start=(kt == 0),
                             stop=(kt == KT - 1), tile_position=(0, 0))
            nc.tensor.matmul(ps[DA:, :], vt16[:, kt, (hA + 1) * DA:(hA + 2) * DA],
                             expg[:, 1, kt, :], start=(kt == 0),
                             stop=(kt == KT - 1), tile_position=(0, DA))
        nc.vector.tensor_tensor(y16[:DA, pr, :], ps[:DA, :], ysc[:DA, 0, :],
                                ALU.mult)
        nc.vector.tensor_tensor(y16[DA:, pr, :], ps[DA:, :], ysc[DA:, 1, :],
                                ALU.mult)

    # Wo -> accumulate into residual
    for dt in range(KO):
        wt = pools['wop'].tile([P, KO, P], F16, tag='wot')
        nc.sync.dma_start(wt[:], wo_d[:, :, dt * P:(dt + 1) * P])
        ps = p256.tile([P, LL], F32, tag='p256')
        for k in range(KO):
            nc.tensor.matmul(ps[:], wt[:, k, :],
                             y16[:, k, :], start=(k == 0), stop=(k == KO - 1))
        nc.vector.tensor_tensor(Eres16[:, dt, :], Eres16[:, dt, :], ps[:],
                                ALU.add)
    tp = pools.get('tapfn')
    if tp:
        tp(f'{name}_y', y16)
    return None


def _mlp(nc, tc, pools, Eres16, w1_dram, w2_dram, name):
    p256 = pools['p256']
    h16 = pools['mlp'].tile([P, MO, LL], F16, tag='h16')
    for mt in range(MO):
        w1t = pools['w1p'].tile([P, KO, P], F16, tag='w1t')
        nc.sync.dma_start(w1t[:], w1_dram[:, :, mt * P:(mt + 1) * P])
        ps = p256.tile([P, LL], F32, tag='p256')
        for k in range(KO):
            nc.tensor.matmul(ps[:], w1t[:, k, :], Eres16[:, k, :],
                             start=(k == 0), stop=(k == KO - 1))
        nc.scalar.activation(h16[:, mt, :], ps[:], AF.Relu)
    for dt in range(KO):
        w2t = pools['w2p'].tile([P, MO, P], F16, tag='w2t')
        nc.sync.dma_start(w2t[:], w2_dram[:, :, dt * P:(dt + 1) * P])
        ps = p256.tile([P, LL], F32, tag='p256')
        for k in range(MO):
            nc.tensor.matmul(ps[:], w2t[:, k, :], h16[:, k, :],
                             start=(k == 0), stop=(k == MO - 1))
        nc.vector.tensor_tensor(Eres16[:, dt, :], Eres16[:, dt, :], ps[:],
                                ALU.add)


def _ln(nc, tc, pools, Eres16, name):
    """In-place layernorm over features on the fp16 stream."""
    p256 = pools['p256']
    ones = pools['ones']
    stat = pools['stat']

    # x^2 on gpsimd (idle engine), sums via ones-matmuls (replicated rows)
    sq16 = pools['lnp'].tile([P, KO, LL], F16, tag='sq16')
    nc.gpsimd.tensor_tensor(sq16[:], Eres16[:], Eres16[:], ALU.mult)
    pss = p256.tile([P, LL], F32, tag='p256')
    psq = p256.tile([P, LL], F32, tag='p256')
    for k in range(KO):
        nc.tensor.matmul(pss[:], ones[:, :], Eres16[:, k, :],
                         start=(k == 0), stop=(k == KO - 1))
    for k in range(KO):
        nc.tensor.matmul(psq[:], ones[:, :], sq16[:, k, :],
                         start=(k == 0), stop=(k == KO - 1))
    mean = stat.tile([P, LL], F32, tag='mean')
    nc.vector.tensor_scalar_mul(mean[:], pss[:], 1.0 / DE)
    # var*(unbiased) = psq/(DE-1) - mean^2*DE/(DE-1)
    msq = stat.tile([P, LL], F32, tag='msq')
    nc.vector.scalar_tensor_tensor(msq[:], mean[:], float(DE) / (DE - 1),
                                   mean[:], ALU.mult, ALU.mult)
    var = stat.tile([P, LL], F32, tag='var')
    nc.vector.scalar_tensor_tensor(var[:], psq[:], 1.0 / (DE - 1),
                                   msq[:], ALU.mult, ALU.subtract)
    std = stat.tile([P, LL], F32, tag='std')
    nc.scalar.activation(std[:], var[:], AF.Sqrt, bias=pools['eps128'])
    inv = stat.tile([P, LL], F32, tag='inv')
    nc.vector.reciprocal_approx_fast(inv[:], std[:])
    inv16 = stat.tile([P, LL], F16, tag='inv16')
    nc.vector.tensor_copy(inv16[:], inv[:])
    negms = stat.tile([P, LL], F16, tag='negms')
    nc.vector.scalar_tensor_tensor(negms[:], mean[:], -1.0, inv[:],
                                   ALU.mult, ALU.mult)
    nc.vector.tensor_tensor(
        Eres16[:], Eres16[:],
        inv16[:, None, :].to_broadcast((P, KO, LL)), ALU.mult)
    nc.vector.tensor_tensor(
        Eres16[:], Eres16[:],
        negms[:, None, :].to_broadcast((P, KO, LL)), ALU.add)
    tp = pools.get('tapfn')
    if tp:
        tp(f'{name}_out', Eres16)


def _allgather_pair(nc, tc, pools, e16loc, full16, agin, agout):
    """e16loc [128, KO, LL] -> pair AllGather -> full16 [128, KO, L]."""
    nc.gpsimd.dma_start(agin[:], e16loc[:])
    nc.gpsimd.collective_compute(
        "AllGather", ALU.bypass,
        ins=[agin[:]], outs=[agout[:]],
        replica_groups=PAIR_GROUPS)
    nc.gpsimd.dma_start(
        full16[:].rearrange('ki ko (r p) -> ki ko r p', r=2),
        agout[:].rearrange('r ki ko p -> ki ko r p'))


def build_program(taps=()):
    taps = set(taps)
    nc = bacc.Bacc("TRN2", target_bir_lowering=False, debug=False,
                   num_devices=N_CORES)

    # ---- dram inputs ----
    din = {}
    def dram_in(nm, shape, dt=F16):
        din[nm] = nc.dram_tensor(nm, list(shape), dt, kind="ExternalInput")
        return din[nm]

    z0f = dram_in('z0_full16', [P, KO, L])
    x0f = dram_in('x0_full16', [P, KO, L])
    z0l16 = dram_in('z0_loc16', [P, KO, LL])
    x0l16 = dram_in('x0_loc16', [P, KO, LL])
    mask_self = dram_in('mask_self', [P, 4, LL])
    for pfx, nl in (('enc', LENC), ('dec', LDEC)):
        for w in ('wqT', 'wkT', 'wvT', 'woT'):
            dram_in(f'{pfx}_{w}', [nl, P, KO, DE])
        dram_in(f'{pfx}_w1T', [nl, P, KO, DMLP])
        dram_in(f'{pfx}_w2T', [nl, P, MO, DE])
    wuC = dram_in('wuC', [UVC, P, KO, UV])

    outp = nc.dram_tensor('outp', [UVC, QT, P, UV], F16, kind="ExternalOutput")

    # internal dram for collectives
    agin = nc.dram_tensor('agin', [P, KO, LL], F16)
    agout = nc.dram_tensor('agout', [2, P, KO, LL], F16)
    arin = nc.dram_tensor('arin', [P, QT], F32)
    arout = nc.dram_tensor('arout', [P, QT], F32)

    import contextlib
    with tile.TileContext(nc) as tc, contextlib.ExitStack() as octx:
        const = octx.enter_context(tc.tile_pool(name='const', bufs=1))
        ones = const.tile([P, P], F16)
        nc.vector.memset(ones[:], 1.0)
        eps128 = const.tile([P, 1], F32)
        nc.vector.memset(eps128[:], EPS)
        ones32 = const.tile([1, P], F32)
        nc.vector.memset(ones32[:], 1.0)
        msk = const.tile([P, 4, LL], F16)
        nc.sync.dma_start(msk[:], mask_self[:])
        XFu = const.tile([P, KO, L], F16)

        # ================= layer phase =================
        with contextlib.ExitStack() as ctx:
            stream = ctx.enter_context(tc.tile_pool(name='stream', bufs=1))
            att = ctx.enter_context(tc.tile_pool(name='att', bufs=1))
            attg = ctx.enter_context(tc.tile_pool(name='attg', bufs=3))
            mlpp = ctx.enter_context(tc.tile_pool(name='mlpp', bufs=1))
            lnp = ctx.enter_context(tc.tile_pool(name='lnp', bufs=1))
            stat = ctx.enter_context(tc.tile_pool(name='stat', bufs=2))
            wqp = ctx.enter_context(tc.tile_pool(name='wqp', bufs=3))
            wkp = ctx.enter_context(tc.tile_pool(name='wkp', bufs=3))
            wvp = ctx.enter_context(tc.tile_pool(name='wvp', bufs=2))
            wop = ctx.enter_context(tc.tile_pool(name='wop', bufs=3))
            w1p = ctx.enter_context(tc.tile_pool(name='w1p', bufs=4))
            w2p = ctx.enter_context(tc.tile_pool(name='w2p', bufs=2))
            p256 = ctx.enter_context(tc.tile_pool(name='p256', bufs=3,
                                                  space='PSUM'))
            p512 = ctx.enter_context(tc.tile_pool(name='p512', bufs=4,
                                                  space='PSUM'))

            pools = dict(att=att, attg=attg, mlp=mlpp, lnp=lnp, p256=p256,
                         p512=p512, stat=stat, ones=ones, ones32=ones32,
                         eps128=eps128[:], wqp=wqp, wkp=wkp,
                         wvp=wvp, wop=wop, w1p=w1p, w2p=w2p)

            def tapfn(nm, t):
                if nm not in taps:
                    return
                d = nc.dram_tensor('tap_' + nm, list(t.shape),
                                   t.dtype, kind="ExternalOutput")
                nc.sync.dma_start(d[:], t[:])
            pools['tapfn'] = tapfn

            # ======== streams ========
            Zres = stream.tile([P, KO, LL], F16, tag='zres')
            nc.sync.dma_start(Zres[:], z0l16[:])
            Zfull = stream.tile([P, KO, L], F16, tag='Zfull')
            nc.sync.dma_start(Zfull[:], z0f[:])
            Xres = stream.tile([P, KO, LL], F16, tag='xres')
            nc.sync.dma_start(Xres[:], x0l16[:])
            Xfull = stream.tile([P, KO, L], F16, tag='Xfull')
            nc.sync.dma_start(Xfull[:], x0f[:])

            # ======== encoder (with dec-l0 self-attn interleaved) ========
            # enc l0 attention
            _attn(nc, tc, pools, Zres, Zfull,
                  din['enc_wqT'][0], din['enc_wkT'][0],
                  din['enc_wvT'][0], din['enc_woT'][0], None, 'e0a')
            # fill: dec l0 self qkv (independent of encoder)
            d0state = _attn(nc, tc, pools, Xres, Xfull,
                            din['dec_wqT'][0], din['dec_wkT'][0],
                            din['dec_wvT'][0], din['dec_woT'][0], msk, 'd0s',
                            split='qkv')
            _ln(nc, tc, pools, Zres, 'e0ln1')
            _mlp(nc, tc, pools, Zres, din['enc_w1T'][0], din['enc_w2T'][0],
                 'e0m')
            _ln(nc, tc, pools, Zres, 'e0ln2')
            Zfull = stream.tile([P, KO, L], F16, tag='Zfull')
            _allgather_pair(nc, tc, pools, Zres, Zfull, agin, agout)

            # enc l1 attention
            _attn(nc, tc, pools, Zres, Zfull,
                  din['enc_wqT'][1], din['enc_wkT'][1],
                  din['enc_wvT'][1], din['enc_woT'][1], None, 'e1a')
            # fill: dec l0 self scores..Wo + ln1
            _attn(nc, tc, pools, Xres, Xfull,
                  din['dec_wqT'][0], din['dec_wkT'][0],
                  din['dec_wvT'][0], din['dec_woT'][0], msk, 'd0s',
                  split=d0state)
            _ln(nc, tc, pools, Xres, 'd0ln1')
            _ln(nc, tc, pools, Zres, 'e1ln1')
            _mlp(nc, tc, pools, Zres, din['enc_w1T'][1], din['enc_w2T'][1],
                 'e1m')
            _ln(nc, tc, pools, Zres, 'e1ln2')
            Zfull = stream.tile([P, KO, L], F16, tag='Zfull')
            _allgather_pair(nc, tc, pools, Zres, Zfull, agin, agout)
            if 'zfinal' in taps:
                tapfn('zfinal', Zfull)

            # ======== decoder ========
            for l in range(LDEC):
                if l > 0:
                    # self-attn (q can start before the AG completes)
                    _attn(nc, tc, pools, Xres, Xfull,
                          din['dec_wqT'][l], din['dec_wkT'][l],
                          din['dec_wvT'][l], din['dec_woT'][l], msk, f'd{l}s')
                    _ln(nc, tc, pools, Xres, f'd{l}ln1')
                _attn(nc, tc, pools, Xres, Zfull,
                      din['dec_wqT'][l], din['dec_wkT'][l],
                      din['dec_wvT'][l], din['dec_woT'][l], None, f'd{l}c')
                _ln(nc, tc, pools, Xres, f'd{l}ln2')
                _mlp(nc, tc, pools, Xres, din['dec_w1T'][l],
                     din['dec_w2T'][l], f'd{l}m')
                _ln(nc, tc, pools, Xres, f'd{l}ln3')
                if l < LDEC - 1:
                    Xfull = stream.tile([P, KO, L], F16, tag='Xfull')
                    _allgather_pair(nc, tc, pools, Xres, Xfull, agin, agout)
                else:
                    _allgather_pair(nc, tc, pools, Xres, XFu, agin, agout)

        # ================= unembed phase =================
        # Xfull: [128ki, KO, 512q]; per chunk: logits^T [128q, 500v]
        with contextlib.ExitStack() as ctx:
            usb = ctx.enter_context(tc.tile_pool(name='usb', bufs=1))
            uout = ctx.enter_context(tc.tile_pool(name='uout', bufs=6))
            wup = ctx.enter_context(tc.tile_pool(name='wup', bufs=4))
            upsum = ctx.enter_context(tc.tile_pool(name='upsum', bufs=8,
                                                   space='PSUM'))

            expu = usb.tile([P, QT, UVC, UV], F16, tag='expu')
            dparts = usb.tile([P, QT, UVC], F32, tag='dparts')
            for vc in range(UVC):
                wt = wup.tile([P, KO, UV], F16, tag='wut')
                nc.sync.dma_start(wt[:], wuC[vc])
                for qt in range(QT):
                    ps = upsum.tile([P, UV], F32, tag='ups')
                    for k in range(KO):
                        nc.tensor.matmul(ps[:], XFu[:, k, qt * P:(qt + 1) * P],
                                         wt[:, k, :],
                                         start=(k == 0), stop=(k == KO - 1))
                    nc.scalar.activation(expu[:, qt, vc, :], ps[:], AF.Exp,
                                         accum_out=dparts[:, qt, vc:vc + 1])
            # deno = sum over vocab chunks (free dim), pair AllReduce
            dloc = usb.tile([P, QT], F32, tag='dloc')
            nc.vector.tensor_reduce(dloc[:], dparts[:],
                                    mybir.AxisListType.X, ALU.add)
            nc.gpsimd.dma_start(arin[:], dloc[:])
            nc.gpsimd.collective_compute(
                "AllReduce", ALU.add,
                ins=[arin[:]], outs=[arout[:]], replica_groups=PAIR_GROUPS)
            dtot = usb.tile([P, QT], F32, tag='dtot')
            nc.gpsimd.dma_start(dtot[:], arout[:])
            rinv = usb.tile([P, QT], F32, tag='rinv')
            nc.vector.reciprocal_approx_fast(rinv[:], dtot[:])
            r16 = usb.tile([P, QT], F32, tag='r16')
            nc.vector.tensor_scalar_mul(r16[:], rinv[:], OSCALE)
            if 'deno' in taps:
                d = nc.dram_tensor('tap_deno', [P, QT], F32,
                                   kind="ExternalOutput")
                nc.sync.dma_start(d[:], dtot[:])
            # normalize + output, spread across engines/queues
            dmae = [nc.sync, nc.gpsimd, nc.scalar, nc.sync]
            i = 0
            for vc in range(UVC):
                for qt in range(QT):
                    ot = uout.tile([P, UV], F16, tag='ot')
                    if i % 3 == 0:
                        nc.vector.tensor_scalar_mul(
                            ot[:], expu[:, qt, vc, :], r16[:, qt:qt + 1])
                    elif i % 3 == 1:
                        nc.gpsimd.tensor_scalar_mul(
                            ot[:], expu[:, qt, vc, :], r16[:, qt:qt + 1])
                    else:
                        nc.scalar.activation(ot[:], expu[:, qt, vc, :],
                                             AF.Copy, scale=r16[:, qt:qt + 1])
                    dmae[i % 4].dma_start(outp[vc, qt], ot[:])
                    i += 1

    nc.compile()
    return nc


# ----------------------------------------------------------------------------
# host-side prep
# ----------------------------------------------------------------------------

def _to_kimaj(a):
    """[K, M] -> [128, K//128, M] with K = ko*128 + ki."""
    K, M = a.shape
    return np.ascontiguousarray(
        a.reshape(K // P, P, M).transpose(1, 0, 2))


def prep_inputs(inputs):
    f = lambda k: np.asarray(inputs[k], dtype=np.float32)
    We, Wp, Wu = f('We'), f('Wp'), f('Wu')
    x = np.asarray(inputs['x']).astype(np.int64)
    z = np.asarray(inputs['z']).astype(np.int64)

    shared = {}
    for pfx, nl in (('enc', LENC), ('dec', LDEC)):
        Wq, Wk, Wv = f(pfx + '_Wq'), f(pfx + '_Wk'), f(pfx + '_Wv')
        Wo, W1, W2 = f(pfx + '_Wo'), f(pfx + '_W1'), f(pfx + '_W2')
        wq, wk, wv, wo, w1, w2 = [], [], [], [], [], []
        for l in range(nl):
            qa = Wq[l].transpose(2, 0, 1).reshape(DE, H * DA) * (DA ** -0.5)
            ka = Wk[l].transpose(2, 0, 1).reshape(DE, H * DA)
            va = Wv[l].transpose(2, 0, 1).reshape(DE, H * DA)
            wq.append(_to_kimaj(qa)); wk.append(_to_kimaj(ka))
            wv.append(_to_kimaj(va))
            wo.append(_to_kimaj(Wo[l].T))
            w1.append(_to_kimaj(W1[l].T))
            w2.append(_to_kimaj(W2[l].T))
        shared[f'{pfx}_wqT'] = np.stack(wq).astype(np.float16)
        shared[f'{pfx}_wkT'] = np.stack(wk).astype(np.float16)
        shared[f'{pfx}_wvT'] = np.stack(wv).astype(np.float16)
        shared[f'{pfx}_woT'] = np.stack(wo).astype(np.float16)
        shared[f'{pfx}_w1T'] = np.stack(w1).astype(np.float16)
        shared[f'{pfx}_w2T'] = np.stack(w2).astype(np.float16)

    # unembedding, pair-split over vocab, chunked contiguous:
    # wuC[vc, ki, ko, v] = Wu[h*NVH + vc*UV + v, ko*128 + ki]
    wuC_h = []
    for h in range(2):
        wh = Wu[h * NVH:(h + 1) * NVH]                 # [16000, 1024]
        a = wh.reshape(UVC, UV, KO, P).transpose(0, 3, 2, 1)
        wuC_h.append(np.ascontiguousarray(a).astype(np.float16))

    pos = Wp[:L]  # [512, 1024]
    in_maps = []
    for c in range(N_CORES):
        b, h = c // 2, c % 2
        m = dict(shared)
        for nm, tok in (('z0', z[b]), ('x0', x[b])):
            E0 = (We[tok] + pos).T.astype(np.float32)      # [1024, 512]
            E0k = E0.reshape(KO, P, L)                     # [ko, ki, p]
            m[nm + '_full16'] = np.ascontiguousarray(
                E0k.transpose(1, 0, 2)).astype(np.float16)
            loc = E0k[:, :, h * LL:(h + 1) * LL].transpose(1, 0, 2)
            m[nm + '_loc16'] = np.ascontiguousarray(loc).astype(np.float16)
        kglob = np.arange(L)[:, None]
        qglob = (h * LL + np.arange(LL))[None, :]
        msk = (kglob <= qglob).astype(np.float16)          # [512, 256]
        m['mask_self'] = np.ascontiguousarray(
            msk.reshape(4, P, LL).transpose(1, 0, 2))
        m['wuC'] = wuC_h[h]
        in_maps.append(m)
    return in_maps


def assemble(results):
    """results: per-core dicts with 'outp' [UVC, QT, P, UV] fp16 (scaled)."""
    out = np.empty((4, NV, L), dtype=np.float32)
    for c, r in enumerate(results):
        b, h = c // 2, c % 2
        o = np.asarray(r['outp'], dtype=np.float32) / OSCALE
        # outp[vc, qt, qi, v] -> [vc, v, qt, qi] -> [16000, 512]
        o = o.transpose(0, 3, 1, 2).reshape(NVH, L)
        out[b, h * NVH:(h + 1) * NVH, :] = o
    return out


def run(inputs, trace=False, taps=(), trace_kwargs=None):
    key = ('prog', tuple(sorted(taps)))
    if key not in _CACHE:
        _CACHE[key] = build_program(taps=taps)
    nc = _CACHE[key]
    in_maps = prep_inputs(inputs)
    res = run_bass_kernel_spmd(nc, in_maps, list(range(N_CORES)),
                               trace=trace, **(trace_kwargs or {}))
    return res


def kernel(**inputs):
    res = run(inputs, trace=False)
    return assemble(res.results)


# revision 11
# speedup vs baseline: 1.2168x; 1.0503x over previous
"""Trainium2 Bass kernel for nn_EDTransformer (encoder-decoder transformer).

Sharding: 8 cores = 4 batch items x 2 sequence halves.
 - Each core owns (item b, half h): computes Q/scores/AV/Wo/MLP/LN for its
   256 local positions, K/V redundantly for the full 512 positions.
 - One 2-core AllGather of fp16 activations per layer (pairs share an item).
 - Unembedding: pair-split over vocab (16000 rows/core), transposed layout
   (tokens on partitions): softmax denominator via activation accum_out,
   one tiny pair AllReduce, per-partition normalize on scalar/vector.
 - PE executes in scheduler order, so independent matmuls (K/V projections,
   decoder fill work) are emitted inside the gaps left by LN/softmax chains.
Dtypes: fp16 residual stream + matmul operands, fp32 PSUM/stats, fp16 output
 scaled by 2^8 (host divides it back out).
"""
import os
import sys

sys.path.insert(0, '/opt/trn_rl_repo')
import numpy as np

import concourse.bacc as bacc
import concourse.tile as tile
import concourse.mybir as mybir
from concourse.bass_utils import run_bass_kernel_spmd

DT = mybir.dt
F16 = DT.float16
F32 = DT.float32
ALU = mybir.AluOpType
AF = mybir.ActivationFunctionType

N_CORES = 8
P = 128
DE = 1024          # model dim        (8 ptiles)
KO = DE // P       # 8
DMLP = 4096        # mlp dim          (32 ptiles)
MO = DMLP // P     # 32
H = 16             # heads
DA = 64            # attn dim per head
L = 512            # sequence length
LL = 256           # local positions per core
KT = L // P        # 4 key tiles
NV = 32000
NVH = NV // 2      # 16000 vocab rows per core (pair-split)
UV = 500           # vocab chunk width
UVC = NVH // UV    # 32 chunks
QT = L // P        # 4 query tiles of 128
LENC = 2
LDEC = 2
EPS = 1e-5
OSCALE = 256.0     # output scale (keeps fp16 out of subnormals)

PAIR_GROUPS = [[0, 1], [2, 3], [4, 5], [6, 7]]

_CACHE = {}


# ----------------------------------------------------------------------------
# device program
# ----------------------------------------------------------------------------

def _attn_kv(nc, pools, kvin16, wk_d, wv_d, tg):
    """K/V projections from the full-sequence stream. tg = tile tag prefix."""
    sb = pools['att']
    p512 = pools['p512']
    k16 = sb.tile([P, KO, L], F16, tag=tg + 'k16')
    for pr in range(KO):
        wt = pools['wkp'].tile([P, KO, P], F16, tag='wkt')
        nc.sync.dma_start(wt[:], wk_d[:, :, pr * P:(pr + 1) * P])
        ps = p512.tile([P, L], F32, tag='p512')
        for k in range(KO):
            nc.tensor.matmul(ps[:], wt[:, k, :],
                             kvin16[:, k, :], start=(k == 0),
                             stop=(k == KO - 1))
        nc.vector.tensor_copy(k16[:, pr, :], ps[:])
    vt16 = sb.tile([P, KT, H * DA], F16, tag=tg + 'vt16')
    for nch in range(2):
        wt = pools['wvp'].tile([P, KO, 512], F16, tag='wvt')
        nc.sync.dma_start(wt[:], wv_d[:, :, nch * 512:(nch + 1) * 512])
        for kt in range(KT):
            ps = p512.tile([P, 512], F32, tag='p512')
            for k in range(KO):
                nc.tensor.matmul(ps[:], kvin16[:, k, kt * P:(kt + 1) * P],
                                 wt[:, k, :],
                                 start=(k == 0), stop=(k == KO - 1))
            nc.vector.tensor_copy(vt16[:, kt, nch * 512:(nch + 1) * 512],
                                  ps[:])
    return dict(k16=k16, vt16=vt16)


def _attn_q(nc, pools, st, qin16, wq_d, tg):
    """Q projection from the local stream (adds to state st)."""
    sb = pools['att']
    p256 = pools['p256']
    q16 = sb.tile([P, KO, LL], F16, tag=tg + 'q16')
    for pr in range(KO):
        wt = pools['wqp'].tile([P, KO, P], F16, tag='wqt')
        nc.sync.dma_start(wt[:], wq_d[:, :, pr * P:(pr + 1) * P])
        ps = p256.tile([P, LL], F32, tag='p256')
        for k in range(KO):
            nc.tensor.matmul(ps[:], wt[:, k, :],
                             qin16[:, k, :], start=(k == 0),
                             stop=(k == KO - 1))
        nc.vector.tensor_copy(q16[:, pr, :], ps[:])
    st['q16'] = q16


def _attn_core(nc, pools, st, Eres16, wo_d, mask, tg):
    """scores -> exp -> (mask) -> deno/recip -> AV -> y -> Wo += residual."""
    sb = pools['att']
    p256 = pools['p256']
    p512 = pools['p512']
    ones = pools['ones']
    q16, k16, vt16 = st['q16'], st['k16'], st['vt16']

    y16 = sb.tile([P, KO, LL], F16, tag=tg + 'y16')
    for pr in range(KO):
        hA = 2 * pr
        expg = pools['attg'].tile([P, 2, KT, LL], F16, tag='expg')
        for h2 in range(2):
            hp = h2 * DA
            for ktp in range(2):
                ps = p512.tile([P, 2, LL], F32, tag='p512')
                for j in range(2):
                    kt = 2 * ktp + j
                    nc.tensor.matmul(ps[:, j, :],
                                     k16[hp:hp + DA, pr, kt * P:(kt + 1) * P],
                                     q16[hp:hp + DA, pr, :],
                                     start=True, stop=True)
                nc.scalar.activation(expg[:, h2, 2 * ktp:2 * ktp + 2, :],
                                     ps[:], AF.Exp)
        if mask is not None:
            for kt in range(KT):
                nc.vector.tensor_tensor(
                    expg[:, :, kt, :], expg[:, :, kt, :],
                    mask[:, kt, None, :].to_broadcast((P, 2, LL)),
                    ALU.mult)
        # deno (2 heads packed on free dim) then fast reciprocal
        pd = p512.tile([P, 2, LL], F32, tag='p512')
        for kt in range(KT):
            nc.tensor.matmul(pd[:], ones[:, :], expg[:, :, kt, :],
                             start=(kt == 0), stop=(kt == KT - 1))
        ysc = pools['yscp'].tile([P, 2, LL], F32, tag='ysc')
        nc.vector.reciprocal_approx_fast(ysc[:], pd[:])
        ps = p256.tile([P, LL], F32, tag='p256')
        for kt in range(KT):
            nc.tensor.matmul(ps[:DA, :], vt16[:, kt, hA * DA:(hA + 1) * DA],
                             expg[:, 0, kt, :], start=(kt == 0),
                             stop=(kt == KT - 1), tile_position=(0, 0))
            nc.tensor.matmul(ps[DA:, :], vt16[:, kt, (hA + 1) * DA:(hA + 2) * DA],
                             expg[:, 1, kt, :], start=(kt == 0),
                             stop=(kt == KT - 1), tile_position=(0, DA))
        nc.vector.tensor_tensor(y16[:DA, pr, :], ps[:DA, :], ysc[:DA, 0, :],
                                ALU.mult)
        nc.vector.tensor_tensor(y16[DA:, pr, :], ps[DA:, :], ysc[DA:, 1, :],
                                ALU.mult)

    # Wo -> accumulate into residual
    for dt in range(KO):
        wt = pools['wop'].tile([P, KO, P], F16, tag='wot')
        nc.sync.dma_start(wt[:], wo_d[:, :, dt * P:(dt + 1) * P])
        ps = p256.tile([P, LL], F32, tag='p256')
        for k in range(KO):
            nc.tensor.matmul(ps[:], wt[:, k, :],
                             y16[:, k, :], start=(k == 0), stop=(k == KO - 1))
        nc.vector.tensor_tensor(Eres16[:, dt, :], Eres16[:, dt, :], ps[:],
                                ALU.add)


def _mlp(nc, pools, Eres16, w1_dram, w2_dram):
    p256 = pools['p256']
    h16 = pools['mlp'].tile([P, MO, LL], F16, tag='h16')
    for mt in range(MO):
        w1t = pools['w1p'].tile([P, KO, P], F16, tag='w1t')
        nc.sync.dma_start(w1t[:], w1_dram[:, :, mt * P:(mt + 1) * P])
        ps = p256.tile([P, LL], F32, tag='p256')
        for k in range(KO):
            nc.tensor.matmul(ps[:], w1t[:, k, :], Eres16[:, k, :],
                             start=(k == 0), stop=(k == KO - 1))
        nc.scalar.activation(h16[:, mt, :], ps[:], AF.Relu)
    for dt in range(KO):
        w2t = pools['w2p'].tile([P, MO, P], F16, tag='w2t')
        nc.sync.dma_start(w2t[:], w2_dram[:, :, dt * P:(dt + 1) * P])
        ps = p256.tile([P, LL], F32, tag='p256')
        for k in range(MO):
            nc.tensor.matmul(ps[:], w2t[:, k, :], h16[:, k, :],
                             start=(k == 0), stop=(k == MO - 1))
        nc.vector.tensor_tensor(Eres16[:, dt, :], Eres16[:, dt, :], ps[:],
                                ALU.add)


def _ln(nc, pools, Eres16, name):
    """In-place layernorm over features on the fp16 stream."""
    p256 = pools['p256']
    ones = pools['ones']
    stat = pools['stat']

    sq16 = pools['lnp'].tile([P, KO, LL], F16, tag='sq16')
    nc.gpsimd.tensor_tensor(sq16[:], Eres16[:], Eres16[:], ALU.mult)
    pss = p256.tile([P, LL], F32, tag='p256')
    psq = p256.tile([P, LL], F32, tag='p256')
    for k in range(KO):
        nc.tensor.matmul(pss[:], ones[:, :], Eres16[:, k, :],
                         start=(k == 0), stop=(k == KO - 1))
    for k in range(KO):
        nc.tensor.matmul(psq[:], ones[:, :], sq16[:, k, :],
                         start=(k == 0), stop=(k == KO - 1))
    mean = stat.tile([P, LL], F32, tag='mean')
    nc.vector.tensor_scalar_mul(mean[:], pss[:], 1.0 / DE)
    msq = stat.tile([P, LL], F32, tag='msq')
    nc.vector.scalar_tensor_tensor(msq[:], mean[:], float(DE) / (DE - 1),
                                   mean[:], ALU.mult, ALU.mult)
    var = stat.tile([P, LL], F32, tag='var')
    nc.vector.scalar_tensor_tensor(var[:], psq[:], 1.0 / (DE - 1),
                                   msq[:], ALU.mult, ALU.subtract)
    std = stat.tile([P, LL], F32, tag='std')
    nc.scalar.activation(std[:], var[:], AF.Sqrt, bias=pools['eps128'])
    inv = stat.tile([P, LL], F32, tag='inv')
    nc.vector.reciprocal_approx_fast(inv[:], std[:])
    inv16 = stat.tile([P, LL], F16, tag='inv16')
    nc.vector.tensor_copy(inv16[:], inv[:])
    negms = stat.tile([P, LL], F16, tag='negms')
    nc.vector.scalar_tensor_tensor(negms[:], mean[:], -1.0, inv[:],
                                   ALU.mult, ALU.mult)
    nc.vector.tensor_tensor(
        Eres16[:], Eres16[:],
        inv16[:, None, :].to_broadcast((P, KO, LL)), ALU.mult)
    nc.vector.tensor_tensor(
        Eres16[:], Eres16[:],
        negms[:, None, :].to_broadcast((P, KO, LL)), ALU.add)
    tp = pools.get('tapfn')
    if tp:
        tp(f'{name}_out', Eres16)


def _allgather_pair(nc, e16loc, full16, agin, agout):
    """e16loc [128, KO, LL] -> pair AllGather -> full16 [128, KO, L]."""
    nc.gpsimd.dma_start(agin[:], e16loc[:])
    nc.gpsimd.collective_compute(
        "AllGather", ALU.bypass,
        ins=[agin[:]], outs=[agout[:]],
        replica_groups=PAIR_GROUPS)
    nc.gpsimd.dma_start(
        full16[:].rearrange('ki ko (r p) -> ki ko r p', r=2),
        agout[:].rearrange('r ki ko p -> ki ko r p'))


def build_program(taps=()):
    taps = set(taps)
    nc = bacc.Bacc("TRN2", target_bir_lowering=False, debug=False,
                   num_devices=N_CORES)

    # ---- dram inputs ----
    din = {}
    def dram_in(nm, shape, dt=F16):
        din[nm] = nc.dram_tensor(nm, list(shape), dt, kind="ExternalInput")
        return din[nm]

    z0f = dram_in('z0_full16', [P, KO, L])
    x0f = dram_in('x0_full16', [P, KO, L])
    z0l16 = dram_in('z0_loc16', [P, KO, LL])
    x0l16 = dram_in('x0_loc16', [P, KO, LL])
    mask_self = dram_in('mask_self', [P, 4, LL])
    for pfx, nl in (('enc', LENC), ('dec', LDEC)):
        for w in ('wqT', 'wkT', 'wvT', 'woT'):
            dram_in(f'{pfx}_{w}', [nl, P, KO, DE])
        dram_in(f'{pfx}_w1T', [nl, P, KO, DMLP])
        dram_in(f'{pfx}_w2T', [nl, P, MO, DE])
    wuC = dram_in('wuC', [UVC, P, KO, UV])

    outp = nc.dram_tensor('outp', [UVC, QT, P, UV], F16, kind="ExternalOutput")

    # internal dram for collectives (one pair per AllGather: a core that
    # runs ahead would otherwise clobber its partner's previous AG output)
    agbufs = [(nc.dram_tensor(f'agin{i}', [P, KO, LL], F16),
               nc.dram_tensor(f'agout{i}', [2, P, KO, LL], F16))
              for i in range(LENC + LDEC)]
    arin = nc.dram_tensor('arin', [P, QT], F32)
    arout = nc.dram_tensor('arout', [P, QT], F32)
    wuin = nc.dram_tensor('wuin', [1, 64], F16)
    wuout = nc.dram_tensor('wuout', [2, 1, 64], F16)

    import contextlib
    with tile.TileContext(nc) as tc, contextlib.ExitStack() as octx:
        const = octx.enter_context(tc.tile_pool(name='const', bufs=1))
        ones = const.tile([P, P], F16)
        nc.vector.memset(ones[:], 1.0)
        eps128 = const.tile([P, 1], F32)
        nc.vector.memset(eps128[:], EPS)
        msk = const.tile([P, 4, LL], F16)
        nc.sync.dma_start(msk[:], mask_self[:])
        XFu = const.tile([P, KO, L], F16)
        # warm up the collective path with a tiny AllGather (overlaps compute)
        wu16 = const.tile([1, 64], F16)
        nc.vector.memset(wu16[:], 0.0)
        nc.gpsimd.dma_start(wuin[:], wu16[:])
        nc.gpsimd.collective_compute(
            "AllGather", ALU.bypass, ins=[wuin[:]], outs=[wuout[:]],
            replica_groups=PAIR_GROUPS)

        # ================= layer phase =================
        with contextlib.ExitStack() as ctx:
            stream = ctx.enter_context(tc.tile_pool(name='stream', bufs=1))
            att = ctx.enter_context(tc.tile_pool(name='att', bufs=1))
            attg = ctx.enter_context(tc.tile_pool(name='attg', bufs=3))
            mlpp = ctx.enter_context(tc.tile_pool(name='mlpp', bufs=1))
            lnp = ctx.enter_context(tc.tile_pool(name='lnp', bufs=1))
            stat = ctx.enter_context(tc.tile_pool(name='stat', bufs=1))
            yscp = ctx.enter_context(tc.tile_pool(name='yscp', bufs=3))
            wqp = ctx.enter_context(tc.tile_pool(name='wqp', bufs=3))
            wkp = ctx.enter_context(tc.tile_pool(name='wkp', bufs=3))
            wvp = ctx.enter_context(tc.tile_pool(name='wvp', bufs=2))
            wop = ctx.enter_context(tc.tile_pool(name='wop', bufs=3))
            w1p = ctx.enter_context(tc.tile_pool(name='w1p', bufs=8))
            w2p = ctx.enter_context(tc.tile_pool(name='w2p', bufs=3))
            p256 = ctx.enter_context(tc.tile_pool(name='p256', bufs=3,
                                                  space='PSUM'))
            p512 = ctx.enter_context(tc.tile_pool(name='p512', bufs=4,
                                                  space='PSUM'))

            pools = dict(att=att, attg=attg, mlp=mlpp, lnp=lnp, p256=p256,
                         p512=p512, stat=stat, yscp=yscp, ones=ones,
                         eps128=eps128[:], wqp=wqp, wkp=wkp,
                         wvp=wvp, wop=wop, w1p=w1p, w2p=w2p)

            def tapfn(nm, t):
                if nm not in taps:
                    return
                d = nc.dram_tensor('tap_' + nm, list(t.shape),
                                   t.dtype, kind="ExternalOutput")
                nc.sync.dma_start(d[:], t[:])
            pools['tapfn'] = tapfn

            # ======== streams ========
            Zres = stream.tile([P, KO, LL], F16, tag='zres')
            nc.sync.dma_start(Zres[:], z0l16[:])
            Zfull = stream.tile([P, KO, L], F16, tag='Zfull')
            nc.sync.dma_start(Zfull[:], z0f[:])
            Xres = stream.tile([P, KO, LL], F16, tag='xres')
            nc.sync.dma_start(Xres[:], x0l16[:])
            Xfull = stream.tile([P, KO, L], F16, tag='Xfull')
            nc.sync.dma_start(Xfull[:], x0f[:])

            W = lambda p, w, l: din[f'{p}_{w}'][l]

            # ======== encoder, dec-l0 self-attn interleaved as PE filler ====
            e0 = _attn_kv(nc, pools, Zfull, W('enc', 'wkT', 0),
                          W('enc', 'wvT', 0), 'e')
            _attn_q(nc, pools, e0, Zres, W('enc', 'wqT', 0), 'e')
            _attn_core(nc, pools, e0, Zres, W('enc', 'woT', 0), None, 'e')
            # filler: dec l0 self kv+q (independent of encoder)
            d0 = _attn_kv(nc, pools, Xfull, W('dec', 'wkT', 0),
                          W('dec', 'wvT', 0), 'd')
            _attn_q(nc, pools, d0, Xres, W('dec', 'wqT', 0), 'd')
            _ln(nc, pools, Zres, 'e0ln1')
            _mlp(nc, pools, Zres, W('enc', 'w1T', 0), W('enc', 'w2T', 0))
            _ln(nc, pools, Zres, 'e0ln2')
            Zfull = stream.tile([P, KO, L], F16, tag='Zfull')
            _allgather_pair(nc, Zres, Zfull, *agbufs[0])

            e1 = _attn_kv(nc, pools, Zfull, W('enc', 'wkT', 1),
                          W('enc', 'wvT', 1), 'e')
            _attn_q(nc, pools, e1, Zres, W('enc', 'wqT', 1), 'e')
            _attn_core(nc, pools, e1, Zres, W('enc', 'woT', 1), None, 'e')
            # filler: dec l0 self scores..Wo + ln1
            _attn_core(nc, pools, d0, Xres, W('dec', 'woT', 0), msk, 'd')
            _ln(nc, pools, Xres, 'd0ln1')
            _ln(nc, pools, Zres, 'e1ln1')
            _mlp(nc, pools, Zres, W('enc', 'w1T', 1), W('enc', 'w2T', 1))
            _ln(nc, pools, Zres, 'e1ln2')
            Zfull = stream.tile([P, KO, L], F16, tag='Zfull')
            _allgather_pair(nc, Zres, Zfull, *agbufs[1])
            if 'zfinal' in taps:
                tapfn('zfinal', Zfull)

            # ======== decoder ========
            for l in range(LDEC):
                # cross-attn: k/v from Z emitted first (fill the ln gap above)
                cst = _attn_kv(nc, pools, Zfull, W('dec', 'wkT', l),
                               W('dec', 'wvT', l), 'd')
                _attn_q(nc, pools, cst, Xres, W('dec', 'wqT', l), 'd')
                _attn_core(nc, pools, cst, Xres, W('dec', 'woT', l), None, 'd')
                _ln(nc, pools, Xres, f'd{l}ln2')
                _mlp(nc, pools, Xres, W('dec', 'w1T', l), W('dec', 'w2T', l))
                _ln(nc, pools, Xres, f'd{l}ln3')
                if l < LDEC - 1:
                    # next layer self-attn: q depends only on the local
                    # stream -> emit before the AllGather; k/v after.
                    sst = {}
                    _attn_q(nc, pools, sst, Xres, W('dec', 'wqT', l + 1), 'd')
                    Xfull = stream.tile([P, KO, L], F16, tag='Xfull')
                    _allgather_pair(nc, Xres, Xfull, *agbufs[2 + l])
                    kv = _attn_kv(nc, pools, Xfull, W('dec', 'wkT', l + 1),
                                  W('dec', 'wvT', l + 1), 'd')
                    sst.update(kv)
                    _attn_core(nc, pools, sst, Xres, W('dec', 'woT', l + 1),
                               msk, 'd')
                    _ln(nc, pools, Xres, f'd{l + 1}ln1')
                else:
                    _allgather_pair(nc, Xres, XFu, *agbufs[2 + l])

        # ================= unembed phase =================
        # XFu: [128ki, KO, 512q]; per chunk: logits^T [128q, 500v]
        with contextlib.ExitStack() as ctx:
            usb = ctx.enter_context(tc.tile_pool(name='usb', bufs=1))
            uout = ctx.enter_context(tc.tile_pool(name='uout', bufs=6))
            wup = ctx.enter_context(tc.tile_pool(name='wup', bufs=6))
            upsum = ctx.enter_context(tc.tile_pool(name='upsum', bufs=8,
                                                   space='PSUM'))

            expu = usb.tile([P, QT, UVC, UV], F16, tag='expu')
            dparts = usb.tile([P, QT, UVC], F32, tag='dparts')
            for vc in range(UVC):
                wt = wup.tile([P, KO, UV], F16, tag='wut')
                nc.sync.dma_start(wt[:], wuC[vc])
                for qt in range(QT):
                    ps = upsum.tile([P, UV], F32, tag='ups')
                    for k in range(KO):
                        nc.tensor.matmul(ps[:], XFu[:, k, qt * P:(qt + 1) * P],
                                         wt[:, k, :],
                                         start=(k == 0), stop=(k == KO - 1))
                    nc.scalar.activation(expu[:, qt, vc, :], ps[:], AF.Exp,
                                         accum_out=dparts[:, qt, vc:vc + 1])
            # deno = sum over vocab chunks (free dim), pair AllReduce
            dloc = usb.tile([P, QT], F32, tag='dloc')
            nc.vector.tensor_reduce(dloc[:], dparts[:],
                                    mybir.AxisListType.X, ALU.add)
            nc.gpsimd.dma_start(arin[:], dloc[:])
            nc.gpsimd.collective_compute(
                "AllReduce", ALU.add,
                ins=[arin[:]], outs=[arout[:]], replica_groups=PAIR_GROUPS)
            dtot = usb.tile([P, QT], F32, tag='dtot')
            nc.gpsimd.dma_start(dtot[:], arout[:])
            rinv = usb.tile([P, QT], F32, tag='rinv')
            nc.vector.reciprocal_approx_fast(rinv[:], dtot[:])
            rsc = usb.tile([P, QT], F32, tag='rsc')
            nc.vector.tensor_scalar_mul(rsc[:], rinv[:], OSCALE)
            r16 = usb.tile([P, QT], F16, tag='r16')
            nc.vector.tensor_copy(r16[:], rsc[:])
            if 'deno' in taps:
                d = nc.dram_tensor('tap_deno', [P, QT], F32,
                                   kind="ExternalOutput")
                nc.sync.dma_start(d[:], dtot[:])
            # normalize + output: broadcast-TT on DVE/gpsimd, copy-scale on ACT
            dmae = [nc.sync, nc.gpsimd, nc.scalar, nc.sync]
            i = 0
            for vc in range(UVC):
                for qt in range(QT):
                    ot = uout.tile([P, UV], F16, tag='ot')
                    m = i % 4
                    if m in (0, 2):
                        nc.vector.tensor_tensor(
                            ot[:], expu[:, qt, vc, :],
                            r16[:, qt:qt + 1].to_broadcast((P, UV)), ALU.mult)
                    elif m == 1:
                        nc.scalar.activation(ot[:], expu[:, qt, vc, :],
                                             AF.Copy, scale=rsc[:, qt:qt + 1])
                    else:
                        nc.gpsimd.tensor_tensor(
                            ot[:], expu[:, qt, vc, :],
                            r16[:, qt:qt + 1].to_broadcast((P, UV)), ALU.mult)
                    dmae[i % 4].dma_start(outp[vc, qt], ot[:])
                    i += 1

    nc.compile()
    return nc


# ----------------------------------------------------------------------------
# host-side prep
# ----------------------------------------------------------------------------

def _to_kimaj(a):
    """[K, M] -> [128, K//128, M] with K = ko*128 + ki."""
    K, M = a.shape
    return np.ascontiguousarray(
        a.reshape(K // P, P, M).transpose(1, 0, 2))


def prep_inputs(inputs):
    f = lambda k: np.asarray(inputs[k], dtype=np.float32)
    We, Wp, Wu = f('We'), f('Wp'), f('Wu')
    x = np.asarray(inputs['x']).astype(np.int64)
    z = np.asarray(inputs['z']).astype(np.int64)

    shared = {}
    for pfx, nl in (('enc', LENC), ('dec', LDEC)):
        Wq, Wk, Wv = f(pfx + '_Wq'), f(pfx + '_Wk'), f(pfx + '_Wv')
        Wo, W1, W2 = f(pfx + '_Wo'), f(pfx + '_W1'), f(pfx + '_W2')
        wq, wk, wv, wo, w1, w2 = [], [], [], [], [], []
        for l in range(nl):
            qa = Wq[l].transpose(2, 0, 1).reshape(DE, H * DA) * (DA ** -0.5)
            ka = Wk[l].transpose(2, 0, 1).reshape(DE, H * DA)
            va = Wv[l].transpose(2, 0, 1).reshape(DE, H * DA)
            wq.append(_to_kimaj(qa)); wk.append(_to_kimaj(ka))
            wv.append(_to_kimaj(va))
            wo.append(_to_kimaj(Wo[l].T))
            w1.append(_to_kimaj(W1[l].T))
            w2.append(_to_kimaj(W2[l].T))
        shared[f'{pfx}_wqT'] = np.stack(wq).astype(np.float16)
        shared[f'{pfx}_wkT'] = np.stack(wk).astype(np.float16)
        shared[f'{pfx}_wvT'] = np.stack(wv).astype(np.float16)
        shared[f'{pfx}_woT'] = np.stack(wo).astype(np.float16)
        shared[f'{pfx}_w1T'] = np.stack(w1).astype(np.float16)
        shared[f'{pfx}_w2T'] = np.stack(w2).astype(np.float16)

    # unembedding, pair-split over vocab, chunked contiguous:
    # wuC[vc, ki, ko, v] = Wu[h*NVH + vc*UV + v, ko*128 + ki]
    wuC_h = []
    for h in range(2):
        wh = Wu[h * NVH:(h + 1) * NVH]                 # [16000, 1024]
        a = wh.reshape(UVC, UV, KO, P).transpose(0, 3, 2, 1)
        wuC_h.append(np.ascontiguousarray(a).astype(np.float16))

    pos = Wp[:L]  # [512, 1024]
    in_maps = []
    for c in range(N_CORES):
        b, h = c // 2, c % 2
        m = dict(shared)
        for nm, tok in (('z0', z[b]), ('x0', x[b])):
            E0 = (We[tok] + pos).T.astype(np.float32)      # [1024, 512]
            E0k = E0.reshape(KO, P, L)                     # [ko, ki, p]
            m[nm + '_full16'] = np.ascontiguousarray(
                E0k.transpose(1, 0, 2)).astype(np.float16)
            loc = E0k[:, :, h * LL:(h + 1) * LL].transpose(1, 0, 2)
            m[nm + '_loc16'] = np.ascontiguousarray(loc).astype(np.float16)
        kglob = np.arange(L)[:, None]
        qglob = (h * LL + np.arange(LL))[None, :]
        msk = (kglob <= qglob).astype(np.float16)          # [512, 256]
        m['mask_self'] = np.ascontiguousarray(
            msk.reshape(4, P, LL).transpose(1, 0, 2))
        m['wuC'] = wuC_h[h]
        in_maps.append(m)
    return in_maps


def assemble(results):
    """results: per-core dicts with 'outp' [UVC, QT, P, UV] fp16 (scaled)."""
    out = np.empty((4, NV, L), dtype=np.float32)
    for c, r in enumerate(results):
        b, h = c // 2, c % 2
        o = np.asarray(r['outp'], dtype=np.float32) / OSCALE
        # outp[vc, qt, qi, v] -> [vc, v, qt, qi] -> [16000, 512]
        o = o.transpose(0, 3, 1, 2).reshape(NVH, L)
        out[b, h * NVH:(h + 1) * NVH, :] = o
    return out


def run(inputs, trace=False, taps=(), trace_kwargs=None):
    key = ('prog', tuple(sorted(taps)))
    if key not in _CACHE:
        _CACHE[key] = build_program(taps=taps)
    nc = _CACHE[key]
    in_maps = prep_inputs(inputs)
    res = run_bass_kernel_spmd(nc, in_maps, list(range(N_CORES)),
                               trace=trace, **(trace_kwargs or {}))
    return res


def kernel(**inputs):
    res = run(inputs, trace=False)
    return assemble(res.results)


# revision 14
# speedup vs baseline: 1.4344x; 1.1788x over previous
"""Trainium2 Bass kernel for nn_EDTransformer (encoder-decoder transformer).

Sharding: 8 cores = 4 batch items x 2 sequence halves.
 - Each core owns (item b, half h): computes Q/scores/AV/Wo/MLP/LN for its
   256 local positions, K/V redundantly for the full 512 positions.
 - One 2-core AllGather of fp16 activations per layer (pairs share an item).
 - Unembedding: pair-split over vocab (16000 rows/core), transposed layout
   (tokens on partitions): softmax denominator via activation accum_out,
   one tiny pair AllReduce, per-partition normalize on scalar/vector.
 - PE executes in scheduler order, so independent matmuls (K/V projections,
   decoder fill work) are emitted inside the gaps left by LN/softmax chains.
Dtypes: fp16 residual stream + matmul operands, fp32 PSUM/stats, fp16 output
 scaled by 2^8 (host divides it back out).
"""
import os
import sys

sys.path.insert(0, '/opt/trn_rl_repo')
import numpy as np

import concourse.bacc as bacc
import concourse.tile as tile
import concourse.mybir as mybir
from concourse.bass_utils import run_bass_kernel_spmd

DT = mybir.dt
F16 = DT.float16
F32 = DT.float32
ALU = mybir.AluOpType
AF = mybir.ActivationFunctionType

N_CORES = 8
P = 128
DE = 1024          # model dim        (8 ptiles)
KO = DE // P       # 8
DMLP = 4096        # mlp dim          (32 ptiles)
MO = DMLP // P     # 32
H = 16             # heads
DA = 64            # attn dim per head
L = 512            # sequence length
LL = 256           # local positions per core
KT = L // P        # 4 key tiles
NV = 32000
NVH = NV // 2      # 16000 vocab rows per core (pair-split)
UV = 500           # vocab chunk width
UVC = NVH // UV    # 32 chunks
QT = L // P        # 4 query tiles of 128
LENC = 2
LDEC = 2
EPS = 1e-5
OSCALE = 256.0     # output scale (keeps fp16 out of subnormals)

PAIR_GROUPS = [[0, 1], [2, 3], [4, 5], [6, 7]]

_CACHE = {}


# ----------------------------------------------------------------------------
# device program
# ----------------------------------------------------------------------------

def _attn_kv(nc, pools, kvin16, wk_d, wv_d, tg):
    """K/V projections from the full-sequence stream. tg = tile tag prefix."""
    sb = pools['att']
    p512 = pools['p512']
    k16 = sb.tile([P, KO, L], F16, tag=tg + 'k16')
    for prp in range(0, KO, 2):
        wt = pools['wkp'].tile([P, KO, 2 * P], F16, tag='wkt')
        nc.sync.dma_start(wt[:], wk_d[:, :, prp * P:(prp + 2) * P])
        for p2 in range(2):
            ps = p512.tile([P, L], F32, tag='p512')
            for k in range(KO):
                nc.tensor.matmul(ps[:], wt[:, k, p2 * P:(p2 + 1) * P],
                                 kvin16[:, k, :], start=(k == 0),
                                 stop=(k == KO - 1))
            nc.vector.tensor_copy(k16[:, prp + p2, :], ps[:])
    vt16 = sb.tile([P, KT, H * DA], F16, tag=tg + 'vt16')
    for nch in range(2):
        wt = pools['wvp'].tile([P, KO, 512], F16, tag='wvt')
        nc.sync.dma_start(wt[:], wv_d[:, :, nch * 512:(nch + 1) * 512])
        for kt in range(KT):
            ps = p512.tile([P, 512], F32, tag='p512')
            for k in range(KO):
                nc.tensor.matmul(ps[:], kvin16[:, k, kt * P:(kt + 1) * P],
                                 wt[:, k, :],
                                 start=(k == 0), stop=(k == KO - 1))
            nc.vector.tensor_copy(vt16[:, kt, nch * 512:(nch + 1) * 512],
                                  ps[:])
    return dict(k16=k16, vt16=vt16)


def _attn_q(nc, pools, st, qin16, wq_d, tg):
    """Q projection from the local stream (adds to state st)."""
    sb = pools['att']
    p256 = pools['p256']
    q16 = sb.tile([P, KO, LL], F16, tag=tg + 'q16')
    for prp in range(0, KO, 2):
        wt = pools['wqp'].tile([P, KO, 2 * P], F16, tag='wqt')
        nc.sync.dma_start(wt[:], wq_d[:, :, prp * P:(prp + 2) * P])
        for p2 in range(2):
            ps = p256.tile([P, LL], F32, tag='p256')
            for k in range(KO):
                nc.tensor.matmul(ps[:], wt[:, k, p2 * P:(p2 + 1) * P],
                                 qin16[:, k, :], start=(k == 0),
                                 stop=(k == KO - 1))
            nc.vector.tensor_copy(q16[:, prp + p2, :], ps[:])
    st['q16'] = q16


def _attn_core(nc, pools, st, Eres16, wo_d, mask, tg):
    """scores -> exp -> (mask) -> deno/recip -> AV -> y -> Wo += residual.

    Software-pipelined: group pr's scores are emitted before group pr-1's
    deno/AV so the PE never waits on the exp activation latency.
    """
    sb = pools['att']
    p256 = pools['p256']
    p512 = pools['p512']
    ones = pools['ones']
    q16, k16, vt16 = st['q16'], st['k16'], st['vt16']

    y16 = sb.tile([P, KO, LL], F16, tag=tg + 'y16')
    expgs = {}

    def scores(pr):
        expg = pools['attg'].tile([P, 2, KT, LL], F16, tag='expg')
        expgs[pr] = expg
        for h2 in range(2):
            hp = h2 * DA
            for ktp in range(2):
                ps = p512.tile([P, 2, LL], F32, tag='p512')
                for j in range(2):
                    kt = 2 * ktp + j
                    nc.tensor.matmul(ps[:, j, :],
                                     k16[hp:hp + DA, pr, kt * P:(kt + 1) * P],
                                     q16[hp:hp + DA, pr, :],
                                     start=True, stop=True)
                nc.scalar.activation(expg[:, h2, 2 * ktp:2 * ktp + 2, :],
                                     ps[:], AF.Exp)
        if mask is not None:
            for kt in range(KT):
                nc.vector.tensor_tensor(
                    expg[:, :, kt, :], expg[:, :, kt, :],
                    mask[:, kt, None, :].to_broadcast((P, 2, LL)),
                    ALU.mult)

    def av(pr):
        hA = 2 * pr
        expg = expgs.pop(pr)
        pd = p512.tile([P, 2, LL], F32, tag='p512')
        for kt in range(KT):
            nc.tensor.matmul(pd[:], ones[:, :], expg[:, :, kt, :],
                             start=(kt == 0), stop=(kt == KT - 1))
        ysc = pools['yscp'].tile([P, 2, LL], F32, tag='ysc')
        nc.vector.reciprocal_approx_fast(ysc[:], pd[:])
        ps = p256.tile([P, LL], F32, tag='p256')
        for kt in range(KT):
            nc.tensor.matmul(ps[:DA, :], vt16[:, kt, hA * DA:(hA + 1) * DA],
                             expg[:, 0, kt, :], start=(kt == 0),
                             stop=(kt == KT - 1), tile_position=(0, 0))
            nc.tensor.matmul(ps[DA:, :], vt16[:, kt, (hA + 1) * DA:(hA + 2) * DA],
                             expg[:, 1, kt, :], start=(kt == 0),
                             stop=(kt == KT - 1), tile_position=(0, DA))
        nc.vector.tensor_tensor(y16[:DA, pr, :], ps[:DA, :], ysc[:DA, 0, :],
                                ALU.mult)
        nc.vector.tensor_tensor(y16[DA:, pr, :], ps[DA:, :], ysc[DA:, 1, :],
                                ALU.mult)

    scores(0)
    for pr in range(1, KO):
        scores(pr)
        av(pr - 1)
    av(KO - 1)

    # Wo -> accumulate into residual
    for dt in range(0, KO, 2):
        wt = pools['wop'].tile([P, KO, 2 * P], F16, tag='wot')
        nc.sync.dma_start(wt[:], wo_d[:, :, dt * P:(dt + 2) * P])
        for d2 in range(2):
            ps = p256.tile([P, LL], F32, tag='p256')
            for k in range(KO):
                nc.tensor.matmul(ps[:], wt[:, k, d2 * P:(d2 + 1) * P],
                                 y16[:, k, :], start=(k == 0),
                                 stop=(k == KO - 1))
            nc.vector.tensor_tensor(Eres16[:, dt + d2, :],
                                    Eres16[:, dt + d2, :], ps[:],
                                    ALU.add)


def _mlp(nc, pools, Eres16, w1_dram, w2_dram):
    p256 = pools['p256']
    h16 = pools['mlp'].tile([P, MO, LL], F16, tag='h16')
    for mtp in range(0, MO, 2):
        w1t = pools['w1p'].tile([P, KO, 2 * P], F16, tag='w1t')
        nc.sync.dma_start(w1t[:], w1_dram[:, :, mtp * P:(mtp + 2) * P])
        for m2 in range(2):
            ps = p256.tile([P, LL], F32, tag='p256')
            for k in range(KO):
                nc.tensor.matmul(ps[:], w1t[:, k, m2 * P:(m2 + 1) * P],
                                 Eres16[:, k, :],
                                 start=(k == 0), stop=(k == KO - 1))
            nc.scalar.activation(h16[:, mtp + m2, :], ps[:], AF.Relu)
    for dt in range(KO):
        w2t = pools['w2p'].tile([P, MO, P], F16, tag='w2t')
        nc.sync.dma_start(w2t[:], w2_dram[:, :, dt * P:(dt + 1) * P])
        ps = p256.tile([P, LL], F32, tag='p256')
        for k in range(MO):
            nc.tensor.matmul(ps[:], w2t[:, k, :], h16[:, k, :],
                             start=(k == 0), stop=(k == MO - 1))
        nc.vector.tensor_tensor(Eres16[:, dt, :], Eres16[:, dt, :], ps[:],
                                ALU.add)


def _ln(nc, pools, Eres16, name):
    """In-place layernorm over features on the fp16 stream."""
    p256 = pools['p256']
    ones = pools['ones']
    stat = pools['stat']

    sq16 = pools['lnp'].tile([P, KO, LL], F16, tag='sq16')
    nc.gpsimd.tensor_tensor(sq16[:], Eres16[:], Eres16[:], ALU.mult)
    pss = p256.tile([P, LL], F32, tag='p256')
    psq = p256.tile([P, LL], F32, tag='p256')
    for k in range(KO):
        nc.tensor.matmul(pss[:], ones[:, :], Eres16[:, k, :],
                         start=(k == 0), stop=(k == KO - 1))
    for k in range(KO):
        nc.tensor.matmul(psq[:], ones[:, :], sq16[:, k, :],
                         start=(k == 0), stop=(k == KO - 1))
    mean = stat.tile([P, LL], F32, tag='mean')
    nc.vector.tensor_scalar_mul(mean[:], pss[:], 1.0 / DE)
    msq = stat.tile([P, LL], F32, tag='msq')
    nc.vector.scalar_tensor_tensor(msq[:], mean[:], float(DE) / (DE - 1),
                                   mean[:], ALU.mult, ALU.mult)
    var = stat.tile([P, LL], F32, tag='var')
    nc.vector.scalar_tensor_tensor(var[:], psq[:], 1.0 / (DE - 1),
                                   msq[:], ALU.mult, ALU.subtract)
    std = stat.tile([P, LL], F32, tag='std')
    nc.scalar.activation(std[:], var[:], AF.Sqrt, bias=pools['eps128'])
    inv = stat.tile([P, LL], F32, tag='inv')
    nc.vector.reciprocal_approx_fast(inv[:], std[:])
    inv16 = stat.tile([P, LL], F16, tag='inv16')
    nc.vector.tensor_copy(inv16[:], inv[:])
    negms = stat.tile([P, LL], F16, tag='negms')
    nc.vector.scalar_tensor_tensor(negms[:], mean[:], -1.0, inv[:],
                                   ALU.mult, ALU.mult)
    nc.vector.tensor_tensor(
        Eres16[:], Eres16[:],
        inv16[:, None, :].to_broadcast((P, KO, LL)), ALU.mult)
    nc.vector.tensor_tensor(
        Eres16[:], Eres16[:],
        negms[:, None, :].to_broadcast((P, KO, LL)), ALU.add)
    tp = pools.get('tapfn')
    if tp:
        tp(f'{name}_out', Eres16)


def _allgather_pair(nc, e16loc, full16, agin, agout):
    """e16loc [128, KO, LL] -> pair AllGather -> full16 [128, KO, L]."""
    nc.gpsimd.dma_start(agin[:], e16loc[:])
    nc.gpsimd.collective_compute(
        "AllGather", ALU.bypass,
        ins=[agin[:]], outs=[agout[:]],
        replica_groups=PAIR_GROUPS)
    nc.gpsimd.dma_start(
        full16[:].rearrange('ki ko (r p) -> ki ko r p', r=2),
        agout[:].rearrange('r ki ko p -> ki ko r p'))


def build_program(taps=()):
    taps = set(taps)
    nc = bacc.Bacc("TRN2", target_bir_lowering=False, debug=False,
                   num_devices=N_CORES)

    # ---- dram inputs ----
    din = {}
    def dram_in(nm, shape, dt=F16):
        din[nm] = nc.dram_tensor(nm, list(shape), dt, kind="ExternalInput")
        return din[nm]

    z0f = dram_in('z0_full16', [P, KO, L])
    x0f = dram_in('x0_full16', [P, KO, L])
    z0l16 = dram_in('z0_loc16', [P, KO, LL])
    x0l16 = dram_in('x0_loc16', [P, KO, LL])
    mask_self = dram_in('mask_self', [P, 4, LL])
    for pfx, nl in (('enc', LENC), ('dec', LDEC)):
        for w in ('wqT', 'wkT', 'wvT', 'woT'):
            dram_in(f'{pfx}_{w}', [nl, P, KO, DE])
        dram_in(f'{pfx}_w1T', [nl, P, KO, DMLP])
        dram_in(f'{pfx}_w2T', [nl, P, MO, DE])
    wuC = dram_in('wuC', [UVC, P, KO, UV])

    outp = nc.dram_tensor('outp', [UVC, QT, P, UV], F16, kind="ExternalOutput")

    # internal dram for collectives (one pair per AllGather: a core that
    # runs ahead would otherwise clobber its partner's previous AG output)
    agbufs = [(nc.dram_tensor(f'agin{i}', [P, KO, LL], F16),
               nc.dram_tensor(f'agout{i}', [2, P, KO, LL], F16))
              for i in range(LENC + LDEC)]
    arin = nc.dram_tensor('arin', [P, QT], F32)
    arout = nc.dram_tensor('arout', [P, QT], F32)
    wuin = nc.dram_tensor('wuin', [1, 64], F16)
    wuout = nc.dram_tensor('wuout', [2, 1, 64], F16)

    import contextlib
    with tile.TileContext(nc) as tc, contextlib.ExitStack() as octx:
        const = octx.enter_context(tc.tile_pool(name='const', bufs=1))
        ones = const.tile([P, P], F16)
        nc.vector.memset(ones[:], 1.0)
        eps128 = const.tile([P, 1], F32)
        nc.vector.memset(eps128[:], EPS)
        msk = const.tile([P, 4, LL], F16)
        nc.sync.dma_start(msk[:], mask_self[:])
        XFu = const.tile([P, KO, L], F16)
        # warm up the collective path with a tiny AllGather (overlaps compute)
        wu16 = const.tile([1, 64], F16)
        nc.vector.memset(wu16[:], 0.0)
        nc.gpsimd.dma_start(wuin[:], wu16[:])
        nc.gpsimd.collective_compute(
            "AllGather", ALU.bypass, ins=[wuin[:]], outs=[wuout[:]],
            replica_groups=PAIR_GROUPS)

        # ================= layer phase =================
        with contextlib.ExitStack() as ctx:
            stream = ctx.enter_context(tc.tile_pool(name='stream', bufs=1))
            att = ctx.enter_context(tc.tile_pool(name='att', bufs=1))
            attg = ctx.enter_context(tc.tile_pool(name='attg', bufs=3))
            mlpp = ctx.enter_context(tc.tile_pool(name='mlpp', bufs=1))
            lnp = ctx.enter_context(tc.tile_pool(name='lnp', bufs=1))
            stat = ctx.enter_context(tc.tile_pool(name='stat', bufs=1))
            yscp = ctx.enter_context(tc.tile_pool(name='yscp', bufs=2))
            wqp = ctx.enter_context(tc.tile_pool(name='wqp', bufs=2))
            wkp = ctx.enter_context(tc.tile_pool(name='wkp', bufs=2))
            wvp = ctx.enter_context(tc.tile_pool(name='wvp', bufs=2))
            wop = ctx.enter_context(tc.tile_pool(name='wop', bufs=2))
            w1p = ctx.enter_context(tc.tile_pool(name='w1p', bufs=4))
            w2p = ctx.enter_context(tc.tile_pool(name='w2p', bufs=2))
            p256 = ctx.enter_context(tc.tile_pool(name='p256', bufs=3,
                                                  space='PSUM'))
            p512 = ctx.enter_context(tc.tile_pool(name='p512', bufs=4,
                                                  space='PSUM'))

            pools = dict(att=att, attg=attg, mlp=mlpp, lnp=lnp, p256=p256,
                         p512=p512, stat=stat, yscp=yscp, ones=ones,
                         eps128=eps128[:], wqp=wqp, wkp=wkp,
                         wvp=wvp, wop=wop, w1p=w1p, w2p=w2p)

            def tapfn(nm, t):
                if nm not in taps:
                    return
                d = nc.dram_tensor('tap_' + nm, list(t.shape),
                                   t.dtype, kind="ExternalOutput")
                nc.sync.dma_start(d[:], t[:])
            pools['tapfn'] = tapfn

            # ======== streams ========
            Zres = stream.tile([P, KO, LL], F16, tag='zres')
            nc.sync.dma_start(Zres[:], z0l16[:])
            Zfull = stream.tile([P, KO, L], F16, tag='Zfull')
            nc.sync.dma_start(Zfull[:], z0f[:])
            Xres = stream.tile([P, KO, LL], F16, tag='xres')
            nc.sync.dma_start(Xres[:], x0l16[:])
            Xfull = stream.tile([P, KO, L], F16, tag='Xfull')
            nc.sync.dma_start(Xfull[:], x0f[:])

            W = lambda p, w, l: din[f'{p}_{w}'][l]

            # ======== encoder, dec-l0 self-attn interleaved as PE filler ====
            # Fill atoms are placed where the PE would otherwise idle: the
            # in-order PE queue runs them during LN/softmax/AllGather waits.
            e0 = _attn_kv(nc, pools, Zfull, W('enc', 'wkT', 0),
                          W('enc', 'wvT', 0), 'e')
            _attn_q(nc, pools, e0, Zres, W('enc', 'wqT', 0), 'e')
            _attn_core(nc, pools, e0, Zres, W('enc', 'woT', 0), None, 'e')
            _ln(nc, pools, Zres, 'e0ln1')
            d0 = _attn_kv(nc, pools, Xfull, W('dec', 'wkT', 0),
                          W('dec', 'wvT', 0), 'd')          # fills e0ln1
            _mlp(nc, pools, Zres, W('enc', 'w1T', 0), W('enc', 'w2T', 0))
            _ln(nc, pools, Zres, 'e0ln2')
            _attn_q(nc, pools, d0, Xres, W('dec', 'wqT', 0), 'd')  # fills ln2
            Zfull = stream.tile([P, KO, L], F16, tag='Zfull')
            _allgather_pair(nc, Zres, Zfull, *agbufs[0])
            _attn_core(nc, pools, d0, Xres, W('dec', 'woT', 0), msk,
                       'd')                                 # fills AG0
            _ln(nc, pools, Xres, 'd0ln1')

            e1 = _attn_kv(nc, pools, Zfull, W('enc', 'wkT', 1),
                          W('enc', 'wvT', 1), 'e')
            _attn_q(nc, pools, e1, Zres, W('enc', 'wqT', 1), 'e')
            _attn_core(nc, pools, e1, Zres, W('enc', 'woT', 1), None, 'e')
            _ln(nc, pools, Zres, 'e1ln1')
            d0c = {}
            _attn_q(nc, pools, d0c, Xres, W('dec', 'wqT', 0), 'd')  # fills ln1
            _mlp(nc, pools, Zres, W('enc', 'w1T', 1), W('enc', 'w2T', 1))
            _ln(nc, pools, Zres, 'e1ln2')
            Zfull = stream.tile([P, KO, L], F16, tag='Zfull')
            _allgather_pair(nc, Zres, Zfull, *agbufs[1])
            if 'zfinal' in taps:
                tapfn('zfinal', Zfull)

            # ======== decoder ========
            # d0 cross-attn (kv needs the final Z -> after AG1)
            d0c.update(_attn_kv(nc, pools, Zfull, W('dec', 'wkT', 0),
                                W('dec', 'wvT', 0), 'd'))
            _attn_core(nc, pools, d0c, Xres, W('dec', 'woT', 0), None, 'd')
            _ln(nc, pools, Xres, 'd0ln2')
            # d1 cross k/v only needs Z: fills d0ln2 + mlp region ('e' tags
            # are free now -- encoder is done)
            d1c = _attn_kv(nc, pools, Zfull, W('dec', 'wkT', 1),
                           W('dec', 'wvT', 1), 'e')
            _mlp(nc, pools, Xres, W('dec', 'w1T', 0), W('dec', 'w2T', 0))
            _ln(nc, pools, Xres, 'd0ln3')
            # d1 self-attn: q only needs the local stream -> before the AG
            d1s = {}
            _attn_q(nc, pools, d1s, Xres, W('dec', 'wqT', 1), 'd')
            Xfull = stream.tile([P, KO, L], F16, tag='Xfull')
            _allgather_pair(nc, Xres, Xfull, *agbufs[2])
            d1s.update(_attn_kv(nc, pools, Xfull, W('dec', 'wkT', 1),
                                W('dec', 'wvT', 1), 'd'))
            _attn_core(nc, pools, d1s, Xres, W('dec', 'woT', 1), msk, 'd')
            _ln(nc, pools, Xres, 'd1ln1')
            _attn_q(nc, pools, d1c, Xres, W('dec', 'wqT', 1), 'e')
            _attn_core(nc, pools, d1c, Xres, W('dec', 'woT', 1), None, 'e')
            _ln(nc, pools, Xres, 'd1ln2')
            _mlp(nc, pools, Xres, W('dec', 'w1T', 1), W('dec', 'w2T', 1))
            _ln(nc, pools, Xres, 'd1ln3')
            _allgather_pair(nc, Xres, XFu, *agbufs[3])

        # ================= unembed phase =================
        # XFu: [128ki, KO, 512q]; per chunk: logits^T [128q, 500v]
        with contextlib.ExitStack() as ctx:
            usb = ctx.enter_context(tc.tile_pool(name='usb', bufs=1))
            uout = ctx.enter_context(tc.tile_pool(name='uout', bufs=6))
            wup = ctx.enter_context(tc.tile_pool(name='wup', bufs=6))
            upsum = ctx.enter_context(tc.tile_pool(name='upsum', bufs=8,
                                                   space='PSUM'))

            expu = usb.tile([P, QT, UVC, UV], F16, tag='expu')
            dparts = usb.tile([P, QT, UVC], F32, tag='dparts')
            for vc in range(UVC):
                wt = wup.tile([P, KO, UV], F16, tag='wut')
                nc.sync.dma_start(wt[:], wuC[vc])
                for qt in range(QT):
                    ps = upsum.tile([P, UV], F32, tag='ups')
                    for k in range(KO):
                        nc.tensor.matmul(ps[:], XFu[:, k, qt * P:(qt + 1) * P],
                                         wt[:, k, :],
                                         start=(k == 0), stop=(k == KO - 1))
                    nc.scalar.activation(expu[:, qt, vc, :], ps[:], AF.Exp,
                                         accum_out=dparts[:, qt, vc:vc + 1])
            # deno = sum over vocab chunks (free dim), pair AllReduce
            dloc = usb.tile([P, QT], F32, tag='dloc')
            nc.vector.tensor_reduce(dloc[:], dparts[:],
                                    mybir.AxisListType.X, ALU.add)
            nc.gpsimd.dma_start(arin[:], dloc[:])
            nc.gpsimd.collective_compute(
                "AllReduce", ALU.add,
                ins=[arin[:]], outs=[arout[:]], replica_groups=PAIR_GROUPS)
            dtot = usb.tile([P, QT], F32, tag='dtot')
            nc.gpsimd.dma_start(dtot[:], arout[:])
            rinv = usb.tile([P, QT], F32, tag='rinv')
            nc.vector.reciprocal_approx_fast(rinv[:], dtot[:])
            rsc = usb.tile([P, QT], F32, tag='rsc')
            nc.vector.tensor_scalar_mul(rsc[:], rinv[:], OSCALE)
            r16 = usb.tile([P, QT], F16, tag='r16')
            nc.vector.tensor_copy(r16[:], rsc[:])
            if 'deno' in taps:
                d = nc.dram_tensor('tap_deno', [P, QT], F32,
                                   kind="ExternalOutput")
                nc.sync.dma_start(d[:], dtot[:])
            # normalize + output: wide ops, materialized per-qt r rows for
            # DVE 2x mode; rotate DVE / ACT; out-DMA spread over queues
            rfull = [usb.tile([P, 2 * UV], F16, tag=f'rf{qt}',
                               name=f'rf{qt}')
                     for qt in range(QT)]
            for qt in range(QT):
                nc.vector.tensor_copy(
                    rfull[qt][:], rsc[:, qt:qt + 1].to_broadcast((P, 2 * UV)))
            dmae = [nc.sync, nc.gpsimd, nc.scalar, nc.sync]
            i = 0
            for vc in range(0, UVC, 2):
                for qt in range(QT):
                    ot = uout.tile([P, 2, UV], F16, tag='ot')
                    if i % 2 == 0:
                        nc.vector.tensor_tensor(
                            ot[:], expu[:, qt, vc:vc + 2, :],
                            rfull[qt][:, None, :].rearrange(
                                'p o (a v) -> p (o a) v', a=2),
                            ALU.mult)
                    else:
                        nc.scalar.activation(ot[:], expu[:, qt, vc:vc + 2, :],
                                             AF.Copy, scale=rsc[:, qt:qt + 1])
                    dmae[i % 4].dma_start(
                        outp[vc:vc + 2, qt].rearrange('a p v -> p a v'), ot[:])
                    i += 1

    nc.compile()
    return nc


# ----------------------------------------------------------------------------
# host-side prep
# ----------------------------------------------------------------------------

def _to_kimaj(a):
    """[K, M] -> [128, K//128, M] with K = ko*128 + ki."""
    K, M = a.shape
    return np.ascontiguousarray(
        a.reshape(K // P, P, M).transpose(1, 0, 2))


def prep_inputs(inputs):
    f = lambda k: np.asarray(inputs[k], dtype=np.float32)
    We, Wp, Wu = f('We'), f('Wp'), f('Wu')
    x = np.asarray(inputs['x']).astype(np.int64)
    z = np.asarray(inputs['z']).astype(np.int64)

    shared = {}
    for pfx, nl in (('enc', LENC), ('dec', LDEC)):
        Wq, Wk, Wv = f(pfx + '_Wq'), f(pfx + '_Wk'), f(pfx + '_Wv')
        Wo, W1, W2 = f(pfx + '_Wo'), f(pfx + '_W1'), f(pfx + '_W2')
        wq, wk, wv, wo, w1, w2 = [], [], [], [], [], []
        for l in range(nl):
            qa = Wq[l].transpose(2, 0, 1).reshape(DE, H * DA) * (DA ** -0.5)
            ka = Wk[l].transpose(2, 0, 1).reshape(DE, H * DA)
            va = Wv[l].transpose(2, 0, 1).reshape(DE, H * DA)
            wq.append(_to_kimaj(qa)); wk.append(_to_kimaj(ka))
            wv.append(_to_kimaj(va))
            wo.append(_to_kimaj(Wo[l].T))
            w1.append(_to_kimaj(W1[l].T))
            w2.append(_to_kimaj(W2[l].T))
        shared[f'{pfx}_wqT'] = np.stack(wq).astype(np.float16)
        shared[f'{pfx}_wkT'] = np.stack(wk).astype(np.float16)
        shared[f'{pfx}_wvT'] = np.stack(wv).astype(np.float16)
        shared[f'{pfx}_woT'] = np.stack(wo).astype(np.float16)
        shared[f'{pfx}_w1T'] = np.stack(w1).astype(np.float16)
        shared[f'{pfx}_w2T'] = np.stack(w2).astype(np.float16)

    # unembedding, pair-split over vocab, chunked contiguous:
    # wuC[vc, ki, ko, v] = Wu[h*NVH + vc*UV + v, ko*128 + ki]
    wuC_h = []
    for h in range(2):
        wh = Wu[h * NVH:(h + 1) * NVH]                 # [16000, 1024]
        a = wh.reshape(UVC, UV, KO, P).transpose(0, 3, 2, 1)
        wuC_h.append(np.ascontiguousarray(a).astype(np.float16))

    pos = Wp[:L]  # [512, 1024]
    in_maps = []
    for c in range(N_CORES):
        b, h = c // 2, c % 2
        m = dict(shared)
        for nm, tok in (('z0', z[b]), ('x0', x[b])):
            E0 = (We[tok] + pos).T.astype(np.float32)      # [1024, 512]
            E0k = E0.reshape(KO, P, L)                     # [ko, ki, p]
            m[nm + '_full16'] = np.ascontiguousarray(
                E0k.transpose(1, 0, 2)).astype(np.float16)
            loc = E0k[:, :, h * LL:(h + 1) * LL].transpose(1, 0, 2)
            m[nm + '_loc16'] = np.ascontiguousarray(loc).astype(np.float16)
        kglob = np.arange(L)[:, None]
        qglob = (h * LL + np.arange(LL))[None, :]
        msk = (kglob <= qglob).astype(np.float16)          # [512, 256]
        m['mask_self'] = np.ascontiguousarray(
            msk.reshape(4, P, LL).transpose(1, 0, 2))
        m['wuC'] = wuC_h[h]
        in_maps.append(m)
    return in_maps


def assemble(results):
    """results: per-core dicts with 'outp' [UVC, QT, P, UV] fp16 (scaled)."""
    out = np.empty((4, NV, L), dtype=np.float32)
    for c, r in enumerate(results):
        b, h = c // 2, c % 2
        o = np.asarray(r['outp'], dtype=np.float32) / OSCALE
        # outp[vc, qt, qi, v] -> [vc, v, qt, qi] -> [16000, 512]
        o = o.transpose(0, 3, 1, 2).reshape(NVH, L)
        out[b, h * NVH:(h + 1) * NVH, :] = o
    return out


def run(inputs, trace=False, taps=(), trace_kwargs=None):
    key = ('prog', tuple(sorted(taps)))
    if key not in _CACHE:
        _CACHE[key] = build_program(taps=taps)
    nc = _CACHE[key]
    in_maps = prep_inputs(inputs)
    res = run_bass_kernel_spmd(nc, in_maps, list(range(N_CORES)),
                               trace=trace, **(trace_kwargs or {}))
    return res


def kernel(**inputs):
    res = run(inputs, trace=False)
    return assemble(res.results)


# revision 19
# speedup vs baseline: 1.6067x; 1.1201x over previous
"""Trainium2 Bass kernel for nn_EDTransformer (encoder-decoder transformer).

Sharding: 8 cores = 4 batch items x 2 sequence halves.
 - Each core owns (item b, half h): computes Q/scores/AV/Wo/MLP/LN for its
   256 local positions, K/V redundantly for the full 512 positions.
 - One 2-core AllGather of fp16 activations per layer (pairs share an item).
 - Unembedding: pair-split over vocab (16000 rows/core), transposed layout
   (tokens on partitions): softmax denominator via activation accum_out,
   one tiny pair AllReduce, per-partition normalize on scalar/vector.
 - PE executes in scheduler order, so independent matmuls (K/V projections,
   decoder fill work) are emitted inside the gaps left by LN/softmax chains.
Dtypes: fp16 residual stream + matmul operands, fp32 PSUM/stats, fp16 output
 scaled by 2^8 (host divides it back out).
"""
import os
import sys

sys.path.insert(0, '/opt/trn_rl_repo')
import numpy as np

import concourse.bacc as bacc
import concourse.tile as tile
import concourse.mybir as mybir
from concourse.bass_utils import run_bass_kernel_spmd

DT = mybir.dt
F16 = DT.float16
F32 = DT.float32
ALU = mybir.AluOpType
AF = mybir.ActivationFunctionType

N_CORES = 8
P = 128
DE = 1024          # model dim        (8 ptiles)
KO = DE // P       # 8
DMLP = 4096        # mlp dim          (32 ptiles)
MO = DMLP // P     # 32
H = 16             # heads
DA = 64            # attn dim per head
L = 512            # sequence length
LL = 256           # local positions per core
KT = L // P        # 4 key tiles
NV = 32000
NVH = NV // 2      # 16000 vocab rows per core (pair-split)
UV = 500           # vocab chunk width
UVC = NVH // UV    # 32 chunks
QT = L // P        # 4 query tiles of 128
LENC = 2
LDEC = 2
EPS = 1e-5
OSCALE = 256.0     # output scale (keeps fp16 out of subnormals)

PAIR_GROUPS = [[0, 1], [2, 3], [4, 5], [6, 7]]

_CACHE = {}


# ----------------------------------------------------------------------------
# device program
# ----------------------------------------------------------------------------

def _attn_kv(nc, pools, loc16, rem16, wk_d, wv_d, tg):
    """K/V projections in [local | remote] column order.

    loc16/rem16: [128, KO, LL] fp16 halves of the kv-input stream. The
    local-half matmuls only need loc16, so they execute during the
    AllGather that produces rem16. Key order is core-dependent; the host
    permutes the causal mask to match (attention is key-permutation
    invariant otherwise).
    """
    sb = pools['att']
    p256 = pools['p256']
    p512 = pools['p512']
    k16 = sb.tile([P, KO, 2, LL], F16, tag=tg + 'k16')
    vt16 = sb.tile([P, KT, H * DA], F16, tag=tg + 'vt16')
    wts = []
    for prp in range(0, KO, 2):
        wt = pools['wkp'].tile([P, KO, 2 * P], F16, tag='wkt')
        nc.sync.dma_start(wt[:], wk_d[:, :, prp * P:(prp + 2) * P])
        wts.append(wt)
        for p2 in range(2):
            ps = p256.tile([P, LL], F32, tag='p256', name='psk')
            for k in range(KO):
                nc.tensor.matmul(ps[:], wt[:, k, p2 * P:(p2 + 1) * P],
                                 loc16[:, k, :], start=(k == 0),
                                 stop=(k == KO - 1))
            nc.vector.tensor_copy(k16[:, prp + p2, 0, :], ps[:])
    wvts = []
    for nch in range(2):
        wt = pools['wvp'].tile([P, KO, 512], F16, tag='wvt')
        nc.sync.dma_start(wt[:], wv_d[:, :, nch * 512:(nch + 1) * 512])
        wvts.append(wt)
        for kt in range(2):
            ps = p512.tile([P, 512], F32, tag='p512')
            for k in range(KO):
                nc.tensor.matmul(ps[:], loc16[:, k, kt * P:(kt + 1) * P],
                                 wt[:, k, :],
                                 start=(k == 0), stop=(k == KO - 1))
            nc.vector.tensor_copy(vt16[:, kt, nch * 512:(nch + 1) * 512],
                                  ps[:])
    # remote half (blocks until rem16 is reconstructed from the AllGather)
    for prp in range(0, KO, 2):
        wt = wts[prp // 2]
        for p2 in range(2):
            ps = p256.tile([P, LL], F32, tag='p256', name='psk')
            for k in range(KO):
                nc.tensor.matmul(ps[:], wt[:, k, p2 * P:(p2 + 1) * P],
                                 rem16[:, k, :], start=(k == 0),
                                 stop=(k == KO - 1))
            nc.vector.tensor_copy(k16[:, prp + p2, 1, :], ps[:])
    for nch in range(2):
        wt = wvts[nch]
        for kt in range(2):
            ps = p512.tile([P, 512], F32, tag='p512')
            for k in range(KO):
                nc.tensor.matmul(ps[:], rem16[:, k, kt * P:(kt + 1) * P],
                                 wt[:, k, :],
                                 start=(k == 0), stop=(k == KO - 1))
            nc.vector.tensor_copy(vt16[:, 2 + kt, nch * 512:(nch + 1) * 512],
                                  ps[:])
    return dict(k16=k16, vt16=vt16)


def _attn_q(nc, pools, st, qin16, wq_d, tg):
    """Q projection from the local stream (adds to state st)."""
    sb = pools['att']
    p256 = pools['p256']
    q16 = sb.tile([P, KO, LL], F16, tag='q16')
    for prp in range(0, KO, 2):
        wt = pools['wqp'].tile([P, KO, 2 * P], F16, tag='wqt')
        nc.sync.dma_start(wt[:], wq_d[:, :, prp * P:(prp + 2) * P])
        for p2 in range(2):
            ps = p256.tile([P, LL], F32, tag='p256')
            for k in range(KO):
                nc.tensor.matmul(ps[:], wt[:, k, p2 * P:(p2 + 1) * P],
                                 qin16[:, k, :], start=(k == 0),
                                 stop=(k == KO - 1))
            nc.vector.tensor_copy(q16[:, prp + p2, :], ps[:])
    st['q16'] = q16


def _attn_core(nc, pools, st, Eres16, wo_d, mask, tg):
    """scores -> exp -> (mask) -> deno/recip -> AV -> y -> Wo += residual.

    Software-pipelined: group pr's scores are emitted before group pr-1's
    deno/AV so the PE never waits on the exp activation latency.
    """
    sb = pools['att']
    p256 = pools['p256']
    p512 = pools['p512']
    ones = pools['ones']
    q16, k16, vt16 = st['q16'], st['k16'], st['vt16']

    y16 = sb.tile([P, KO, LL], F16, tag='y16')
    expgs = {}

    def scores(pr):
        expg = pools['attg'].tile([P, 2, KT, LL], F16, tag='expg')
        expgs[pr] = expg
        for h2 in range(2):
            hp = h2 * DA
            for ktp in range(2):
                ps = p512.tile([P, 2, LL], F32, tag='p512')
                for j in range(2):
                    kt = 2 * ktp + j
                    nc.tensor.matmul(ps[:, j, :],
                                     k16[hp:hp + DA, pr, kt // 2,
                                         (kt % 2) * P:(kt % 2 + 1) * P],
                                     q16[hp:hp + DA, pr, :],
                                     start=True, stop=True)
                nc.scalar.activation(expg[:, h2, 2 * ktp:2 * ktp + 2, :],
                                     ps[:], AF.Exp)
        if mask is not None:
            for kt in range(KT):
                nc.vector.tensor_tensor(
                    expg[:, :, kt, :], expg[:, :, kt, :],
                    mask[:, kt, None, :].to_broadcast((P, 2, LL)),
                    ALU.mult)

    def av(pr):
        hA = 2 * pr
        expg = expgs.pop(pr)
        pd = p512.tile([P, 2, LL], F32, tag='p512')
        for kt in range(KT):
            nc.tensor.matmul(pd[:], ones[:, :], expg[:, :, kt, :],
                             start=(kt == 0), stop=(kt == KT - 1))
        ysc = pools['yscp'].tile([P, 2, LL], F32, tag='ysc')
        nc.vector.reciprocal_approx_fast(ysc[:], pd[:])
        ps = p256.tile([P, LL], F32, tag='p256')
        for kt in range(KT):
            nc.tensor.matmul(ps[:DA, :], vt16[:, kt, hA * DA:(hA + 1) * DA],
                             expg[:, 0, kt, :], start=(kt == 0),
                             stop=(kt == KT - 1), tile_position=(0, 0))
            nc.tensor.matmul(ps[DA:, :], vt16[:, kt, (hA + 1) * DA:(hA + 2) * DA],
                             expg[:, 1, kt, :], start=(kt == 0),
                             stop=(kt == KT - 1), tile_position=(0, DA))
        nc.vector.tensor_tensor(y16[:DA, pr, :], ps[:DA, :], ysc[:DA, 0, :],
                                ALU.mult)
        nc.vector.tensor_tensor(y16[DA:, pr, :], ps[DA:, :], ysc[DA:, 1, :],
                                ALU.mult)

    scores(0)
    for pr in range(1, KO):
        scores(pr)
        av(pr - 1)
    av(KO - 1)

    # Wo -> accumulate into residual
    for dt in range(0, KO, 2):
        wt = pools['wop'].tile([P, KO, 2 * P], F16, tag='wot')
        nc.sync.dma_start(wt[:], wo_d[:, :, dt * P:(dt + 2) * P])
        for d2 in range(2):
            ps = p256.tile([P, LL], F32, tag='p256')
            for k in range(KO):
                nc.tensor.matmul(ps[:], wt[:, k, d2 * P:(d2 + 1) * P],
                                 y16[:, k, :], start=(k == 0),
                                 stop=(k == KO - 1))
            nc.vector.tensor_tensor(Eres16[:, dt + d2, :],
                                    Eres16[:, dt + d2, :], ps[:],
                                    ALU.add)


def _mlp(nc, pools, Eres16, w1_dram, w2_dram):
    p256 = pools['p256']
    h16 = pools['mlp'].tile([P, MO, LL], F16, tag='h16')
    for mtp in range(0, MO, 2):
        w1t = pools['w1p'].tile([P, KO, 2 * P], F16, tag='w1t')
        nc.sync.dma_start(w1t[:], w1_dram[:, :, mtp * P:(mtp + 2) * P])
        for m2 in range(2):
            ps = p256.tile([P, LL], F32, tag='p256')
            for k in range(KO):
                nc.tensor.matmul(ps[:], w1t[:, k, m2 * P:(m2 + 1) * P],
                                 Eres16[:, k, :],
                                 start=(k == 0), stop=(k == KO - 1))
            nc.scalar.activation(h16[:, mtp + m2, :], ps[:], AF.Relu)
    for dt in range(KO):
        w2t = pools['w2p'].tile([P, MO, P], F16, tag='w2t')
        nc.sync.dma_start(w2t[:], w2_dram[:, :, dt * P:(dt + 1) * P])
        ps = p256.tile([P, LL], F32, tag='p256')
        for k in range(MO):
            nc.tensor.matmul(ps[:], w2t[:, k, :], h16[:, k, :],
                             start=(k == 0), stop=(k == MO - 1))
        nc.vector.tensor_tensor(Eres16[:, dt, :], Eres16[:, dt, :], ps[:],
                                ALU.add)


def _ln(nc, pools, Eres16, name):
    """In-place layernorm over features on the fp16 stream."""
    p256 = pools['p256']
    ones = pools['ones']
    stat = pools['stat']

    sq16 = pools['lnp'].tile([P, KO, LL], F16, tag='sq16')
    nc.gpsimd.tensor_tensor(sq16[:], Eres16[:], Eres16[:], ALU.mult)
    pss = p256.tile([P, LL], F32, tag='p256')
    psq = p256.tile([P, LL], F32, tag='p256')
    for k in range(KO):
        nc.tensor.matmul(pss[:], ones[:, :], Eres16[:, k, :],
                         start=(k == 0), stop=(k == KO - 1))
    for k in range(KO):
        nc.tensor.matmul(psq[:], ones[:, :], sq16[:, k, :],
                         start=(k == 0), stop=(k == KO - 1))
    mean = stat.tile([P, LL], F32, tag='mean')
    nc.vector.tensor_scalar_mul(mean[:], pss[:], 1.0 / DE)
    msq = stat.tile([P, LL], F32, tag='msq')
    nc.vector.scalar_tensor_tensor(msq[:], mean[:], float(DE) / (DE - 1),
                                   mean[:], ALU.mult, ALU.mult)
    var = stat.tile([P, LL], F32, tag='var')
    nc.vector.scalar_tensor_tensor(var[:], psq[:], 1.0 / (DE - 1),
                                   msq[:], ALU.mult, ALU.subtract)
    std = stat.tile([P, LL], F32, tag='std')
    nc.scalar.activation(std[:], var[:], AF.Sqrt, bias=pools['eps128'])
    inv = stat.tile([P, LL], F32, tag='inv')
    nc.vector.reciprocal_approx_fast(inv[:], std[:])
    inv16 = stat.tile([P, LL], F16, tag='inv16')
    nc.vector.tensor_copy(inv16[:], inv[:])
    negms = stat.tile([P, LL], F16, tag='negms')
    nc.vector.scalar_tensor_tensor(negms[:], mean[:], -1.0, inv[:],
                                   ALU.mult, ALU.mult)
    nc.vector.tensor_tensor(
        Eres16[:], Eres16[:],
        inv16[:, None, :].to_broadcast((P, KO, LL)), ALU.mult)
    nc.vector.tensor_tensor(
        Eres16[:], Eres16[:],
        negms[:, None, :].to_broadcast((P, KO, LL)), ALU.add)
    tp = pools.get('tapfn')
    if tp:
        tp(f'{name}_out', Eres16)


def _remote_half(nc, pools, agout, loc16, tag):
    """rem = (agout[0] + agout[1]) - loc, exact in fp32 (fp16 inputs)."""
    sp = pools['stream']
    xa = sp.tile([P, KO, LL], F16, tag='gxa', name='xa')
    xb = sp.tile([P, KO, LL], F16, tag='gxb', name='xb')
    nc.gpsimd.dma_start(xa[:], agout[0])
    nc.gpsimd.dma_start(xb[:], agout[1])
    s32 = sp.tile([P, KO, LL], F32, tag='gs32', name='s32')
    nc.vector.tensor_tensor(s32[:], xa[:], xb[:], ALU.add)
    rem = sp.tile([P, KO, LL], F16, tag=tag + 'rem', name='rem')
    nc.vector.tensor_tensor(rem[:], s32[:], loc16[:], ALU.subtract)
    return rem


def _allgather_pair(nc, e16loc, agin, agout):
    """e16loc [128, KO, LL] -> pair AllGather into agout [2, 128, KO, LL]."""
    nc.gpsimd.dma_start(agin[:], e16loc[:])
    nc.gpsimd.collective_compute(
        "AllGather", ALU.bypass,
        ins=[agin[:]], outs=[agout[:]],
        replica_groups=PAIR_GROUPS)


def build_program(taps=()):
    taps = set(taps)
    nc = bacc.Bacc("TRN2", target_bir_lowering=False, debug=False,
                   num_devices=N_CORES)

    # ---- dram inputs ----
    din = {}
    def dram_in(nm, shape, dt=F16):
        din[nm] = nc.dram_tensor(nm, list(shape), dt, kind="ExternalInput")
        return din[nm]

    z0l16 = dram_in('z0_loc16', [P, KO, LL])
    x0l16 = dram_in('x0_loc16', [P, KO, LL])
    z0r16 = dram_in('z0_rem16', [P, KO, LL])
    x0r16 = dram_in('x0_rem16', [P, KO, LL])
    mask_self = dram_in('mask_self', [P, 4, LL])
    for pfx, nl in (('enc', LENC), ('dec', LDEC)):
        for w in ('wqT', 'wkT', 'wvT', 'woT'):
            dram_in(f'{pfx}_{w}', [nl, P, KO, DE])
        dram_in(f'{pfx}_w1T', [nl, P, KO, DMLP])
        dram_in(f'{pfx}_w2T', [nl, P, MO, DE])
    wuC = dram_in('wuC', [UVC, P, KO, UV])

    outp = nc.dram_tensor('outp', [QT, P, UVC, UV], F16, kind="ExternalOutput")

    # internal dram for collectives (one pair per AllGather: a core that
    # runs ahead would otherwise clobber its partner's previous AG output)
    agbufs = [(nc.dram_tensor(f'agin{i}', [P, KO, LL], F16),
               nc.dram_tensor(f'agout{i}', [2, P, KO, LL], F16))
              for i in range(LENC + LDEC)]
    arin = nc.dram_tensor('arin', [P, QT], F32)
    arout = nc.dram_tensor('arout', [P, QT], F32)
    wuin = nc.dram_tensor('wuin', [1, 64], F16)
    wuout = nc.dram_tensor('wuout', [2, 1, 64], F16)

    import contextlib
    with tile.TileContext(nc) as tc, contextlib.ExitStack() as octx:
        const = octx.enter_context(tc.tile_pool(name='const', bufs=1))
        ones = const.tile([P, P], F16)
        nc.vector.memset(ones[:], 1.0)
        eps128 = const.tile([P, 1], F32)
        nc.vector.memset(eps128[:], EPS)
        msk = const.tile([P, 4, LL], F16)
        nc.sync.dma_start(msk[:], mask_self[:])
        XFu = const.tile([P, KO, L], F16)
        # warm up the collective path with a tiny AllGather (overlaps compute)
        wu16 = const.tile([1, 64], F16)
        nc.vector.memset(wu16[:], 0.0)
        nc.gpsimd.dma_start(wuin[:], wu16[:])
        nc.gpsimd.collective_compute(
            "AllGather", ALU.bypass, ins=[wuin[:]], outs=[wuout[:]],
            replica_groups=PAIR_GROUPS)

        # ================= layer phase =================
        with contextlib.ExitStack() as ctx:
            stream = ctx.enter_context(tc.tile_pool(name='stream', bufs=1))
            stat = ctx.enter_context(tc.tile_pool(name='stat', bufs=1))
            lctx = ctx.enter_context(contextlib.ExitStack())
            att = lctx.enter_context(tc.tile_pool(name='att', bufs=1))
            attg = lctx.enter_context(tc.tile_pool(name='attg', bufs=2))
            mlpp = lctx.enter_context(tc.tile_pool(name='mlpp', bufs=1))
            lnp = lctx.enter_context(tc.tile_pool(name='lnp', bufs=1))
            yscp = lctx.enter_context(tc.tile_pool(name='yscp', bufs=2))
            wqp = lctx.enter_context(tc.tile_pool(name='wqp', bufs=2))
            wkp = lctx.enter_context(tc.tile_pool(name='wkp', bufs=4))
            wvp = lctx.enter_context(tc.tile_pool(name='wvp', bufs=2))
            wop = lctx.enter_context(tc.tile_pool(name='wop', bufs=2))
            w1p = lctx.enter_context(tc.tile_pool(name='w1p', bufs=4))
            w2p = lctx.enter_context(tc.tile_pool(name='w2p', bufs=2))
            p256 = lctx.enter_context(tc.tile_pool(name='p256', bufs=3,
                                                   space='PSUM'))
            p512 = lctx.enter_context(tc.tile_pool(name='p512', bufs=4,
                                                   space='PSUM'))

            pools = dict(att=att, attg=attg, mlp=mlpp, lnp=lnp, p256=p256,
                         p512=p512, stat=stat, yscp=yscp, ones=ones,
                         eps128=eps128[:], wqp=wqp, wkp=wkp,
                         wvp=wvp, wop=wop, w1p=w1p, w2p=w2p, stream=stream)

            def tapfn(nm, t):
                if nm not in taps:
                    return
                d = nc.dram_tensor('tap_' + nm, list(t.shape),
                                   t.dtype, kind="ExternalOutput")
                nc.sync.dma_start(d[:], t[:])
            pools['tapfn'] = tapfn

            # ======== streams ========
            Zres = stream.tile([P, KO, LL], F16, tag='zres')
            nc.sync.dma_start(Zres[:], z0l16[:])
            ZR = stream.tile([P, KO, LL], F16, tag='zrem')
            nc.sync.dma_start(ZR[:], z0r16[:])
            Xres = stream.tile([P, KO, LL], F16, tag='xres')
            nc.sync.dma_start(Xres[:], x0l16[:])
            XR = stream.tile([P, KO, LL], F16, tag='xrem')
            nc.sync.dma_start(XR[:], x0r16[:])

            W = lambda p, w, l: din[f'{p}_{w}'][l]

            # ======== encoder, dec-l0 self-attn interleaved as PE filler ====
            e0 = _attn_kv(nc, pools, Zres, ZR, W('enc', 'wkT', 0),
                          W('enc', 'wvT', 0), 'e')
            _attn_q(nc, pools, e0, Zres, W('enc', 'wqT', 0), 'e')
            _attn_core(nc, pools, e0, Zres, W('enc', 'woT', 0), None, 'e')
            _ln(nc, pools, Zres, 'e0ln1')
            d0 = _attn_kv(nc, pools, Xres, XR, W('dec', 'wkT', 0),
                          W('dec', 'wvT', 0), 'd')          # fills e0ln1
            _mlp(nc, pools, Zres, W('enc', 'w1T', 0), W('enc', 'w2T', 0))
            _ln(nc, pools, Zres, 'e0ln2')
            _attn_q(nc, pools, d0, Xres, W('dec', 'wqT', 0), 'd')  # fills ln2
            _allgather_pair(nc, Zres, *agbufs[0])
            _attn_core(nc, pools, d0, Xres, W('dec', 'woT', 0), msk,
                       'd')                                 # fills AG0
            _ln(nc, pools, Xres, 'd0ln1')
            ZR = _remote_half(nc, pools, agbufs[0][1], Zres, 'z')

            e1 = _attn_kv(nc, pools, Zres, ZR, W('enc', 'wkT', 1),
                          W('enc', 'wvT', 1), 'e')
            _attn_q(nc, pools, e1, Zres, W('enc', 'wqT', 1), 'e')
            _attn_core(nc, pools, e1, Zres, W('enc', 'woT', 1), None, 'e')
            _ln(nc, pools, Zres, 'e1ln1')
            d0c = {}
            _attn_q(nc, pools, d0c, Xres, W('dec', 'wqT', 0), 'd')  # fills ln1
            _mlp(nc, pools, Zres, W('enc', 'w1T', 1), W('enc', 'w2T', 1))
            _ln(nc, pools, Zres, 'e1ln2')
            _allgather_pair(nc, Zres, *agbufs[1])
            ZR = _remote_half(nc, pools, agbufs[1][1], Zres, 'z')
            if 'zfinal' in taps:
                tapfn('zfinal', Zres)

            # ======== decoder ========
            # d0 cross-attn: local k/v from Zres fills the AG1 wait
            d0c.update(_attn_kv(nc, pools, Zres, ZR, W('dec', 'wkT', 0),
                                W('dec', 'wvT', 0), 'd'))
            _attn_core(nc, pools, d0c, Xres, W('dec', 'woT', 0), None, 'd')
            _ln(nc, pools, Xres, 'd0ln2')
            # d1 cross k/v needs only Z: fills d0ln2 + mlp region ('e' tags
            # are free now -- encoder is done)
            d1c = _attn_kv(nc, pools, Zres, ZR, W('dec', 'wkT', 1),
                           W('dec', 'wvT', 1), 'e')
            _mlp(nc, pools, Xres, W('dec', 'w1T', 0), W('dec', 'w2T', 0))
            _ln(nc, pools, Xres, 'd0ln3')
            # d1 self-attn: q + local k/v only need the local stream
            d1s = {}
            _attn_q(nc, pools, d1s, Xres, W('dec', 'wqT', 1), 'd')
            _allgather_pair(nc, Xres, *agbufs[2])
            XR1 = _remote_half(nc, pools, agbufs[2][1], Xres, 'x')
            d1s.update(_attn_kv(nc, pools, Xres, XR1, W('dec', 'wkT', 1),
                                W('dec', 'wvT', 1), 'd'))
            _attn_core(nc, pools, d1s, Xres, W('dec', 'woT', 1), msk, 'd')
            _ln(nc, pools, Xres, 'd1ln1')
            _attn_q(nc, pools, d1c, Xres, W('dec', 'wqT', 1), 'e')
            _attn_core(nc, pools, d1c, Xres, W('dec', 'woT', 1), None, 'e')
            _ln(nc, pools, Xres, 'd1ln2')
            _mlp(nc, pools, Xres, W('dec', 'w1T', 1), W('dec', 'w2T', 1))
            _ln(nc, pools, Xres, 'd1ln3')
            _allgather_pair(nc, Xres, *agbufs[3])
            lctx.close()

            # ============= unembed phase (slot order: local|remote) =========
            # logits^T [128q, 500v] per (slot, chunk); slots 0,1 read the
            # local stream and run during the final AllGather.
            usb = ctx.enter_context(tc.tile_pool(name='usb', bufs=1))
            wup = ctx.enter_context(tc.tile_pool(name='wup', bufs=2))
            upsum = ctx.enter_context(tc.tile_pool(name='upsum', bufs=8,
                                                   space='PSUM'))
            NPRE = 8

            expu = usb.tile([P, QT, UVC, UV], F16, tag='expu')
            dparts = usb.tile([P, QT, UVC], F32, tag='dparts')

            def uchunk(vc, slots, xl, xr):
                wt = wup.tile([P, KO, UV], F16, tag='wut')
                nc.sync.dma_start(wt[:], wuC[vc])
                for sl in slots:
                    src = xl if sl < 2 else xr
                    so = (sl % 2) * P
                    ps = upsum.tile([P, UV], F32, tag='ups')
                    for k in range(KO):
                        nc.tensor.matmul(ps[:], src[:, k, so:so + P],
                                         wt[:, k, :],
                                         start=(k == 0), stop=(k == KO - 1))
                    nc.scalar.activation(expu[:, sl, vc, :], ps[:], AF.Exp,
                                         accum_out=dparts[:, sl, vc:vc + 1])

            for vc in range(NPRE):
                uchunk(vc, (0, 1), Xres, None)      # pre-AG local slots
            XR2 = _remote_half(nc, pools, agbufs[3][1], Xres, 'x')
            for vc in range(NPRE):
                uchunk(vc, (2, 3), Xres, XR2)       # re-reads wuC[vc]
            for vc in range(NPRE, UVC):
                uchunk(vc, (0, 1, 2, 3), Xres, XR2)

            # deno = sum over vocab chunks (free dim), pair AllReduce
            dloc = usb.tile([P, QT], F32, tag='dloc')
            nc.vector.tensor_reduce(dloc[:], dparts[:],
                                    mybir.AxisListType.X, ALU.add)
            nc.gpsimd.dma_start(arin[:], dloc[:])
            nc.gpsimd.collective_compute(
                "AllReduce", ALU.add,
                ins=[arin[:]], outs=[arout[:]], replica_groups=PAIR_GROUPS)
            dtot = usb.tile([P, QT], F32, tag='dtot')
            nc.gpsimd.dma_start(dtot[:], arout[:])
            rinv = usb.tile([P, QT], F32, tag='rinv')
            nc.vector.reciprocal_approx_fast(rinv[:], dtot[:])
            rsc = usb.tile([P, QT], F32, tag='rsc')
            nc.vector.tensor_scalar_mul(rsc[:], rinv[:], OSCALE)
            r16 = usb.tile([P, QT], F16, tag='r16')
            nc.vector.tensor_copy(r16[:], rsc[:])
            if 'deno' in taps:
                d = nc.dram_tensor('tap_deno', [P, QT], F32,
                                   kind="ExternalOutput")
                nc.sync.dma_start(d[:], dtot[:])
            # normalize expu in place (wide sub-ops on ACT + DVE), then one
            # big contiguous DMA per slot (32KB per partition line)
            dmae = [nc.sync, nc.gpsimd, nc.scalar, nc.sync]
            NSUB = 4
            W8 = UVC // NSUB
            for sl in range(QT):
                for j in range(NSUB):
                    seg = expu[:, sl, j * W8:(j + 1) * W8, :]
                    if j % 2 == 0:
                        nc.scalar.activation(seg, seg, AF.Copy,
                                             scale=rsc[:, sl:sl + 1])
                    else:
                        nc.vector.tensor_tensor(
                            seg, seg,
                            r16[:, sl:sl + 1, None].to_broadcast(
                                (P, W8, UV)), ALU.mult)
                dmae[sl].dma_start(outp[sl], expu[:, sl, :, :])

    nc.compile()
    return nc


# ----------------------------------------------------------------------------
# host-side prep
# ----------------------------------------------------------------------------

def _to_kimaj(a):
    """[K, M] -> [128, K//128, M] with K = ko*128 + ki."""
    K, M = a.shape
    return np.ascontiguousarray(
        a.reshape(K // P, P, M).transpose(1, 0, 2))


def prep_inputs(inputs):
    f = lambda k: np.asarray(inputs[k], dtype=np.float32)
    We, Wp, Wu = f('We'), f('Wp'), f('Wu')
    x = np.asarray(inputs['x']).astype(np.int64)
    z = np.asarray(inputs['z']).astype(np.int64)

    shared = {}
    for pfx, nl in (('enc', LENC), ('dec', LDEC)):
        Wq, Wk, Wv = f(pfx + '_Wq'), f(pfx + '_Wk'), f(pfx + '_Wv')
        Wo, W1, W2 = f(pfx + '_Wo'), f(pfx + '_W1'), f(pfx + '_W2')
        wq, wk, wv, wo, w1, w2 = [], [], [], [], [], []
        for l in range(nl):
            qa = Wq[l].transpose(2, 0, 1).reshape(DE, H * DA) * (DA ** -0.5)
            ka = Wk[l].transpose(2, 0, 1).reshape(DE, H * DA)
            va = Wv[l].transpose(2, 0, 1).reshape(DE, H * DA)
            wq.append(_to_kimaj(qa)); wk.append(_to_kimaj(ka))
            wv.append(_to_kimaj(va))
            wo.append(_to_kimaj(Wo[l].T))
            w1.append(_to_kimaj(W1[l].T))
            w2.append(_to_kimaj(W2[l].T))
        shared[f'{pfx}_wqT'] = np.stack(wq).astype(np.float16)
        shared[f'{pfx}_wkT'] = np.stack(wk).astype(np.float16)
        shared[f'{pfx}_wvT'] = np.stack(wv).astype(np.float16)
        shared[f'{pfx}_woT'] = np.stack(wo).astype(np.float16)
        shared[f'{pfx}_w1T'] = np.stack(w1).astype(np.float16)
        shared[f'{pfx}_w2T'] = np.stack(w2).astype(np.float16)

    # unembedding, pair-split over vocab, chunked contiguous:
    # wuC[vc, ki, ko, v] = Wu[h*NVH + vc*UV + v, ko*128 + ki]
    wuC_h = []
    for h in range(2):
        wh = Wu[h * NVH:(h + 1) * NVH]                 # [16000, 1024]
        a = wh.reshape(UVC, UV, KO, P).transpose(0, 3, 2, 1)
        wuC_h.append(np.ascontiguousarray(a).astype(np.float16))

    pos = Wp[:L]  # [512, 1024]
    in_maps = []
    for c in range(N_CORES):
        b, h = c // 2, c % 2
        m = dict(shared)
        for nm, tok in (('z0', z[b]), ('x0', x[b])):
            E0 = (We[tok] + pos).T.astype(np.float32)      # [1024, 512]
            E0k = E0.reshape(KO, P, L)                     # [ko, ki, p]
            loc = E0k[:, :, h * LL:(h + 1) * LL].transpose(1, 0, 2)
            m[nm + '_loc16'] = np.ascontiguousarray(loc).astype(np.float16)
            rr = (1 - h)
            rem = E0k[:, :, rr * LL:(rr + 1) * LL].transpose(1, 0, 2)
            m[nm + '_rem16'] = np.ascontiguousarray(rem).astype(np.float16)
        kglob = np.arange(L)[:, None]
        qglob = (h * LL + np.arange(LL))[None, :]
        msk = (kglob <= qglob).astype(np.float16)          # [512, 256]
        mm4 = msk.reshape(4, P, LL)
        # key tiles in this core's [local | remote] order
        perm = [2 * h, 2 * h + 1, 2 * (1 - h), 2 * (1 - h) + 1]
        m['mask_self'] = np.ascontiguousarray(
            mm4[perm].transpose(1, 0, 2))
        m['wuC'] = wuC_h[h]
        in_maps.append(m)
    return in_maps


def assemble(results):
    """results: per-core dicts with 'outp' [QT(slot), P, UVC, UV] fp16."""
    out = np.empty((4, NV, L), dtype=np.float32)
    for c, r in enumerate(results):
        b, h = c // 2, c % 2
        o = np.asarray(r['outp'], dtype=np.float32) / OSCALE
        # slot s covers global query tile g: local slots first
        for sl in range(QT):
            g = 2 * h + sl if sl < 2 else 2 * (1 - h) + (sl - 2)
            # o[sl] : [P(qi), UVC, UV] -> [16000, 128]
            blk = o[sl].transpose(1, 2, 0).reshape(NVH, P)
            out[b, h * NVH:(h + 1) * NVH, g * P:(g + 1) * P] = blk
    return out


def run(inputs, trace=False, taps=(), trace_kwargs=None):
    key = ('prog', tuple(sorted(taps)))
    if key not in _CACHE:
        _CACHE[key] = build_program(taps=taps)
    nc = _CACHE[key]
    in_maps = prep_inputs(inputs)
    res = run_bass_kernel_spmd(nc, in_maps, list(range(N_CORES)),
                               trace=trace, **(trace_kwargs or {}))
    return res


def kernel(**inputs):
    res = run(inputs, trace=False)
    return assemble(res.results)


# revision 20
# speedup vs baseline: 1.6157x; 1.0056x over previous
"""Trainium2 Bass kernel for nn_EDTransformer (encoder-decoder transformer).

Sharding: 8 cores = 4 batch items x 2 sequence halves.
 - Each core owns (item b, half h): computes Q/scores/AV/Wo/MLP/LN for its
   256 local positions, K/V redundantly for the full 512 positions.
 - One 2-core AllGather of fp16 activations per layer (pairs share an item).
 - Unembedding: pair-split over vocab (16000 rows/core), transposed layout
   (tokens on partitions): softmax denominator via activation accum_out,
   one tiny pair AllReduce, per-partition normalize on scalar/vector.
 - PE executes in scheduler order, so independent matmuls (K/V projections,
   decoder fill work) are emitted inside the gaps left by LN/softmax chains.
Dtypes: fp16 residual stream + matmul operands, fp32 PSUM/stats, fp16 output
 scaled by 2^8 (host divides it back out).
"""
import os
import sys

sys.path.insert(0, '/opt/trn_rl_repo')
import numpy as np

import concourse.bacc as bacc
import concourse.tile as tile
import concourse.mybir as mybir
from concourse.bass_utils import run_bass_kernel_spmd

DT = mybir.dt
F16 = DT.float16
F32 = DT.float32
ALU = mybir.AluOpType
AF = mybir.ActivationFunctionType

N_CORES = 8
P = 128
DE = 1024          # model dim        (8 ptiles)
KO = DE // P       # 8
DMLP = 4096        # mlp dim          (32 ptiles)
MO = DMLP // P     # 32
H = 16             # heads
DA = 64            # attn dim per head
L = 512            # sequence length
LL = 256           # local positions per core
KT = L // P        # 4 key tiles
NV = 32000
NVH = NV // 2      # 16000 vocab rows per core (pair-split)
UV = 500           # vocab chunk width
UVC = NVH // UV    # 32 chunks
QT = L // P        # 4 query tiles of 128
LENC = 2
LDEC = 2
EPS = 1e-5
OSCALE = 256.0     # output scale (keeps fp16 out of subnormals)

PAIR_GROUPS = [[0, 1], [2, 3], [4, 5], [6, 7]]

_CACHE = {}


# ----------------------------------------------------------------------------
# device program
# ----------------------------------------------------------------------------

def _attn_kv(nc, pools, loc16, rem16, wk_d, wv_d, tg):
    """K/V projections in [local | remote] column order.

    loc16/rem16: [128, KO, LL] fp16 halves of the kv-input stream. The
    local-half matmuls only need loc16, so they execute during the
    AllGather that produces rem16. Key order is core-dependent; the host
    permutes the causal mask to match (attention is key-permutation
    invariant otherwise).
    """
    sb = pools['att']
    p256 = pools['p256']
    p512 = pools['p512']
    k16 = sb.tile([P, KO, 2, LL], F16, tag=tg + 'k16')
    vt16 = sb.tile([P, KT, H * DA], F16, tag=tg + 'vt16')
    wts = []
    for prp in range(0, KO, 2):
        wt = pools['wkp'].tile([P, KO, 2 * P], F16, tag='wkt')
        nc.sync.dma_start(wt[:], wk_d[:, :, prp * P:(prp + 2) * P])
        wts.append(wt)
        for p2 in range(2):
            ps = p256.tile([P, LL], F32, tag='p256', name='psk')
            for k in range(KO):
                nc.tensor.matmul(ps[:], wt[:, k, p2 * P:(p2 + 1) * P],
                                 loc16[:, k, :], start=(k == 0),
                                 stop=(k == KO - 1))
            nc.vector.tensor_copy(k16[:, prp + p2, 0, :], ps[:])
    wvts = []
    for nch in range(2):
        wt = pools['wvp'].tile([P, KO, 512], F16, tag='wvt')
        nc.sync.dma_start(wt[:], wv_d[:, :, nch * 512:(nch + 1) * 512])
        wvts.append(wt)
        for kt in range(2):
            ps = p512.tile([P, 512], F32, tag='p512')
            for k in range(KO):
                nc.tensor.matmul(ps[:], loc16[:, k, kt * P:(kt + 1) * P],
                                 wt[:, k, :],
                                 start=(k == 0), stop=(k == KO - 1))
            nc.vector.tensor_copy(vt16[:, kt, nch * 512:(nch + 1) * 512],
                                  ps[:])
    # remote half (blocks until rem16 is reconstructed from the AllGather)
    for prp in range(0, KO, 2):
        wt = wts[prp // 2]
        for p2 in range(2):
            ps = p256.tile([P, LL], F32, tag='p256', name='psk')
            for k in range(KO):
                nc.tensor.matmul(ps[:], wt[:, k, p2 * P:(p2 + 1) * P],
                                 rem16[:, k, :], start=(k == 0),
                                 stop=(k == KO - 1))
            nc.vector.tensor_copy(k16[:, prp + p2, 1, :], ps[:])
    for nch in range(2):
        wt = wvts[nch]
        for kt in range(2):
            ps = p512.tile([P, 512], F32, tag='p512')
            for k in range(KO):
                nc.tensor.matmul(ps[:], rem16[:, k, kt * P:(kt + 1) * P],
                                 wt[:, k, :],
                                 start=(k == 0), stop=(k == KO - 1))
            nc.vector.tensor_copy(vt16[:, 2 + kt, nch * 512:(nch + 1) * 512],
                                  ps[:])
    return dict(k16=k16, vt16=vt16)


def _attn_q(nc, pools, st, qin16, wq_d, tg):
    """Q projection from the local stream (adds to state st)."""
    sb = pools['att']
    p256 = pools['p256']
    q16 = sb.tile([P, KO, LL], F16, tag='q16')
    for prp in range(0, KO, 2):
        wt = pools['wqp'].tile([P, KO, 2 * P], F16, tag='wqt')
        nc.sync.dma_start(wt[:], wq_d[:, :, prp * P:(prp + 2) * P])
        for p2 in range(2):
            ps = p256.tile([P, LL], F32, tag='p256')
            for k in range(KO):
                nc.tensor.matmul(ps[:], wt[:, k, p2 * P:(p2 + 1) * P],
                                 qin16[:, k, :], start=(k == 0),
                                 stop=(k == KO - 1))
            nc.vector.tensor_copy(q16[:, prp + p2, :], ps[:])
    st['q16'] = q16


def _attn_core(nc, pools, st, Eres16, wo_d, mask, tg):
    """scores -> exp -> (mask) -> deno/recip -> AV -> y -> Wo += residual.

    Software-pipelined: group pr's scores are emitted before group pr-1's
    deno/AV so the PE never waits on the exp activation latency.
    """
    sb = pools['att']
    p256 = pools['p256']
    p512 = pools['p512']
    ones = pools['ones']
    q16, k16, vt16 = st['q16'], st['k16'], st['vt16']

    y16 = sb.tile([P, KO, LL], F16, tag='y16')
    expgs = {}

    def scores(pr):
        expg = pools['attg'].tile([P, 2, KT, LL], F16, tag='expg')
        expgs[pr] = expg
        for h2 in range(2):
            hp = h2 * DA
            for ktp in range(2):
                ps = p512.tile([P, 2, LL], F32, tag='p512')
                for j in range(2):
                    kt = 2 * ktp + j
                    nc.tensor.matmul(ps[:, j, :],
                                     k16[hp:hp + DA, pr, kt // 2,
                                         (kt % 2) * P:(kt % 2 + 1) * P],
                                     q16[hp:hp + DA, pr, :],
                                     start=True, stop=True)
                nc.scalar.activation(expg[:, h2, 2 * ktp:2 * ktp + 2, :],
                                     ps[:], AF.Exp)
        if mask is not None:
            for kt in range(KT):
                nc.vector.tensor_tensor(
                    expg[:, :, kt, :], expg[:, :, kt, :],
                    mask[:, kt, None, :].to_broadcast((P, 2, LL)),
                    ALU.mult)

    def av(pr):
        hA = 2 * pr
        expg = expgs.pop(pr)
        pd = p512.tile([P, 2, LL], F32, tag='p512')
        for kt in range(KT):
            nc.tensor.matmul(pd[:], ones[:, :], expg[:, :, kt, :],
                             start=(kt == 0), stop=(kt == KT - 1))
        ysc = pools['yscp'].tile([P, 2, LL], F32, tag='ysc')
        nc.vector.reciprocal_approx_fast(ysc[:], pd[:])
        ps = p256.tile([P, LL], F32, tag='p256')
        for kt in range(KT):
            nc.tensor.matmul(ps[:DA, :], vt16[:, kt, hA * DA:(hA + 1) * DA],
                             expg[:, 0, kt, :], start=(kt == 0),
                             stop=(kt == KT - 1), tile_position=(0, 0))
            nc.tensor.matmul(ps[DA:, :], vt16[:, kt, (hA + 1) * DA:(hA + 2) * DA],
                             expg[:, 1, kt, :], start=(kt == 0),
                             stop=(kt == KT - 1), tile_position=(0, DA))
        nc.vector.tensor_tensor(y16[:DA, pr, :], ps[:DA, :], ysc[:DA, 0, :],
                                ALU.mult)
        nc.vector.tensor_tensor(y16[DA:, pr, :], ps[DA:, :], ysc[DA:, 1, :],
                                ALU.mult)

    scores(0)
    for pr in range(1, KO):
        scores(pr)
        av(pr - 1)
    av(KO - 1)

    # Wo -> accumulate into residual
    for dt in range(0, KO, 2):
        wt = pools['wop'].tile([P, KO, 2 * P], F16, tag='wot')
        nc.sync.dma_start(wt[:], wo_d[:, :, dt * P:(dt + 2) * P])
        for d2 in range(2):
            ps = p256.tile([P, LL], F32, tag='p256')
            for k in range(KO):
                nc.tensor.matmul(ps[:], wt[:, k, d2 * P:(d2 + 1) * P],
                                 y16[:, k, :], start=(k == 0),
                                 stop=(k == KO - 1))
            nc.vector.tensor_tensor(Eres16[:, dt + d2, :],
                                    Eres16[:, dt + d2, :], ps[:],
                                    ALU.add)


def _mlp(nc, pools, Eres16, w1_dram, w2_dram):
    p256 = pools['p256']
    h16 = pools['mlp'].tile([P, MO, LL], F16, tag='h16')
    for mtp in range(0, MO, 2):
        w1t = pools['w1p'].tile([P, KO, 2 * P], F16, tag='w1t')
        nc.sync.dma_start(w1t[:], w1_dram[:, :, mtp * P:(mtp + 2) * P])
        for m2 in range(2):
            ps = p256.tile([P, LL], F32, tag='p256')
            for k in range(KO):
                nc.tensor.matmul(ps[:], w1t[:, k, m2 * P:(m2 + 1) * P],
                                 Eres16[:, k, :],
                                 start=(k == 0), stop=(k == KO - 1))
            nc.scalar.activation(h16[:, mtp + m2, :], ps[:], AF.Relu)
    for dt in range(KO):
        w2t = pools['w2p'].tile([P, MO, P], F16, tag='w2t')
        nc.sync.dma_start(w2t[:], w2_dram[:, :, dt * P:(dt + 1) * P])
        ps = p256.tile([P, LL], F32, tag='p256')
        for k in range(MO):
            nc.tensor.matmul(ps[:], w2t[:, k, :], h16[:, k, :],
                             start=(k == 0), stop=(k == MO - 1))
        nc.vector.tensor_tensor(Eres16[:, dt, :], Eres16[:, dt, :], ps[:],
                                ALU.add)


def _ln(nc, pools, Eres16, name):
    """In-place layernorm over features on the fp16 stream."""
    p256 = pools['p256']
    ones = pools['ones']
    stat = pools['stat']

    sq16 = pools['lnp'].tile([P, KO, LL], F16, tag='sq16')
    nc.gpsimd.tensor_tensor(sq16[:], Eres16[:], Eres16[:], ALU.mult)
    pss = p256.tile([P, LL], F32, tag='p256')
    psq = p256.tile([P, LL], F32, tag='p256')
    for k in range(KO):
        nc.tensor.matmul(pss[:], ones[:, :], Eres16[:, k, :],
                         start=(k == 0), stop=(k == KO - 1))
    for k in range(KO):
        nc.tensor.matmul(psq[:], ones[:, :], sq16[:, k, :],
                         start=(k == 0), stop=(k == KO - 1))
    mean = stat.tile([P, LL], F32, tag='mean')
    nc.vector.tensor_scalar_mul(mean[:], pss[:], 1.0 / DE)
    msq = stat.tile([P, LL], F32, tag='msq')
    nc.vector.scalar_tensor_tensor(msq[:], mean[:], float(DE) / (DE - 1),
                                   mean[:], ALU.mult, ALU.mult)
    var = stat.tile([P, LL], F32, tag='var')
    nc.vector.scalar_tensor_tensor(var[:], psq[:], 1.0 / (DE - 1),
                                   msq[:], ALU.mult, ALU.subtract)
    std = stat.tile([P, LL], F32, tag='std')
    nc.scalar.activation(std[:], var[:], AF.Sqrt, bias=pools['eps128'])
    inv = stat.tile([P, LL], F32, tag='inv')
    nc.vector.reciprocal_approx_fast(inv[:], std[:])
    inv16 = stat.tile([P, LL], F16, tag='inv16')
    nc.vector.tensor_copy(inv16[:], inv[:])
    negms = stat.tile([P, LL], F16, tag='negms')
    nc.vector.scalar_tensor_tensor(negms[:], mean[:], -1.0, inv[:],
                                   ALU.mult, ALU.mult)
    nc.vector.tensor_tensor(
        Eres16[:], Eres16[:],
        inv16[:, None, :].to_broadcast((P, KO, LL)), ALU.mult)
    nc.vector.tensor_tensor(
        Eres16[:], Eres16[:],
        negms[:, None, :].to_broadcast((P, KO, LL)), ALU.add)
    tp = pools.get('tapfn')
    if tp:
        tp(f'{name}_out', Eres16)


def _remote_half(nc, pools, agout, loc16, tag):
    """rem = (agout[0] + agout[1]) - loc, exact in fp32 (fp16 inputs)."""
    sp = pools['stream']
    xa = sp.tile([P, KO, LL], F16, tag='gxa', name='xa')
    xb = sp.tile([P, KO, LL], F16, tag='gxb', name='xb')
    nc.gpsimd.dma_start(xa[:], agout[0])
    nc.gpsimd.dma_start(xb[:], agout[1])
    s32 = sp.tile([P, KO, LL], F32, tag='gs32', name='s32')
    nc.vector.tensor_tensor(s32[:], xa[:], xb[:], ALU.add)
    rem = sp.tile([P, KO, LL], F16, tag=tag + 'rem', name='rem')
    nc.vector.tensor_tensor(rem[:], s32[:], loc16[:], ALU.subtract)
    return rem


def _allgather_pair(nc, e16loc, agin, agout):
    """e16loc [128, KO, LL] -> pair AllGather into agout [2, 128, KO, LL]."""
    nc.gpsimd.dma_start(agin[:], e16loc[:])
    nc.gpsimd.collective_compute(
        "AllGather", ALU.bypass,
        ins=[agin[:]], outs=[agout[:]],
        replica_groups=PAIR_GROUPS)


def build_program(taps=()):
    taps = set(taps)
    nc = bacc.Bacc("TRN2", target_bir_lowering=False, debug=False,
                   num_devices=N_CORES)

    # ---- dram inputs ----
    din = {}
    def dram_in(nm, shape, dt=F16):
        din[nm] = nc.dram_tensor(nm, list(shape), dt, kind="ExternalInput")
        return din[nm]

    z0l16 = dram_in('z0_loc16', [P, KO, LL])
    x0l16 = dram_in('x0_loc16', [P, KO, LL])
    z0r16 = dram_in('z0_rem16', [P, KO, LL])
    x0r16 = dram_in('x0_rem16', [P, KO, LL])
    mask_self = dram_in('mask_self', [P, 4, LL])
    for pfx, nl in (('enc', LENC), ('dec', LDEC)):
        for w in ('wqT', 'wkT', 'wvT', 'woT'):
            dram_in(f'{pfx}_{w}', [nl, P, KO, DE])
        dram_in(f'{pfx}_w1T', [nl, P, KO, DMLP])
        dram_in(f'{pfx}_w2T', [nl, P, MO, DE])
    wuC = dram_in('wuC', [UVC, P, KO, UV])

    outp = nc.dram_tensor('outp', [QT, P, UVC, UV], F16, kind="ExternalOutput")

    # internal dram for collectives (one pair per AllGather: a core that
    # runs ahead would otherwise clobber its partner's previous AG output)
    agbufs = [(nc.dram_tensor(f'agin{i}', [P, KO, LL], F16),
               nc.dram_tensor(f'agout{i}', [2, P, KO, LL], F16))
              for i in range(LENC + LDEC)]
    arin = nc.dram_tensor('arin', [P, QT], F32)
    arout = nc.dram_tensor('arout', [2, P, QT], F32)
    wuin = nc.dram_tensor('wuin', [1, 64], F16)
    wuout = nc.dram_tensor('wuout', [2, 1, 64], F16)

    import contextlib
    with tile.TileContext(nc) as tc, contextlib.ExitStack() as octx:
        const = octx.enter_context(tc.tile_pool(name='const', bufs=1))
        ones = const.tile([P, P], F16)
        nc.vector.memset(ones[:], 1.0)
        eps128 = const.tile([P, 1], F32)
        nc.vector.memset(eps128[:], EPS)
        msk = const.tile([P, 4, LL], F16)
        nc.sync.dma_start(msk[:], mask_self[:])
        XFu = const.tile([P, KO, L], F16)
        # warm up the collective path with a tiny AllGather (overlaps compute)
        wu16 = const.tile([1, 64], F16)
        nc.vector.memset(wu16[:], 0.0)
        nc.gpsimd.dma_start(wuin[:], wu16[:])
        nc.gpsimd.collective_compute(
            "AllGather", ALU.bypass, ins=[wuin[:]], outs=[wuout[:]],
            replica_groups=PAIR_GROUPS)

        # ================= layer phase =================
        with contextlib.ExitStack() as ctx:
            stream = ctx.enter_context(tc.tile_pool(name='stream', bufs=1))
            stat = ctx.enter_context(tc.tile_pool(name='stat', bufs=1))
            lctx = ctx.enter_context(contextlib.ExitStack())
            att = lctx.enter_context(tc.tile_pool(name='att', bufs=1))
            attg = lctx.enter_context(tc.tile_pool(name='attg', bufs=2))
            mlpp = lctx.enter_context(tc.tile_pool(name='mlpp', bufs=1))
            lnp = lctx.enter_context(tc.tile_pool(name='lnp', bufs=1))
            yscp = lctx.enter_context(tc.tile_pool(name='yscp', bufs=2))
            wqp = lctx.enter_context(tc.tile_pool(name='wqp', bufs=2))
            wkp = lctx.enter_context(tc.tile_pool(name='wkp', bufs=4))
            wvp = lctx.enter_context(tc.tile_pool(name='wvp', bufs=2))
            wop = lctx.enter_context(tc.tile_pool(name='wop', bufs=2))
            w1p = lctx.enter_context(tc.tile_pool(name='w1p', bufs=4))
            w2p = lctx.enter_context(tc.tile_pool(name='w2p', bufs=2))
            p256 = lctx.enter_context(tc.tile_pool(name='p256', bufs=3,
                                                   space='PSUM'))
            p512 = lctx.enter_context(tc.tile_pool(name='p512', bufs=4,
                                                   space='PSUM'))

            pools = dict(att=att, attg=attg, mlp=mlpp, lnp=lnp, p256=p256,
                         p512=p512, stat=stat, yscp=yscp, ones=ones,
                         eps128=eps128[:], wqp=wqp, wkp=wkp,
                         wvp=wvp, wop=wop, w1p=w1p, w2p=w2p, stream=stream)

            def tapfn(nm, t):
                if nm not in taps:
                    return
                d = nc.dram_tensor('tap_' + nm, list(t.shape),
                                   t.dtype, kind="ExternalOutput")
                nc.sync.dma_start(d[:], t[:])
            pools['tapfn'] = tapfn

            # ======== streams ========
            Zres = stream.tile([P, KO, LL], F16, tag='zres')
            nc.sync.dma_start(Zres[:], z0l16[:])
            ZR = stream.tile([P, KO, LL], F16, tag='zrem')
            nc.sync.dma_start(ZR[:], z0r16[:])
            Xres = stream.tile([P, KO, LL], F16, tag='xres')
            nc.sync.dma_start(Xres[:], x0l16[:])
            XR = stream.tile([P, KO, LL], F16, tag='xrem')
            nc.sync.dma_start(XR[:], x0r16[:])

            W = lambda p, w, l: din[f'{p}_{w}'][l]

            # ======== encoder, dec-l0 self-attn interleaved as PE filler ====
            e0 = _attn_kv(nc, pools, Zres, ZR, W('enc', 'wkT', 0),
                          W('enc', 'wvT', 0), 'e')
            _attn_q(nc, pools, e0, Zres, W('enc', 'wqT', 0), 'e')
            _attn_core(nc, pools, e0, Zres, W('enc', 'woT', 0), None, 'e')
            _ln(nc, pools, Zres, 'e0ln1')
            d0 = _attn_kv(nc, pools, Xres, XR, W('dec', 'wkT', 0),
                          W('dec', 'wvT', 0), 'd')          # fills e0ln1
            _mlp(nc, pools, Zres, W('enc', 'w1T', 0), W('enc', 'w2T', 0))
            _ln(nc, pools, Zres, 'e0ln2')
            _attn_q(nc, pools, d0, Xres, W('dec', 'wqT', 0), 'd')  # fills ln2
            _allgather_pair(nc, Zres, *agbufs[0])
            _attn_core(nc, pools, d0, Xres, W('dec', 'woT', 0), msk,
                       'd')                                 # fills AG0
            _ln(nc, pools, Xres, 'd0ln1')
            ZR = _remote_half(nc, pools, agbufs[0][1], Zres, 'z')

            e1 = _attn_kv(nc, pools, Zres, ZR, W('enc', 'wkT', 1),
                          W('enc', 'wvT', 1), 'e')
            _attn_q(nc, pools, e1, Zres, W('enc', 'wqT', 1), 'e')
            _attn_core(nc, pools, e1, Zres, W('enc', 'woT', 1), None, 'e')
            _ln(nc, pools, Zres, 'e1ln1')
            d0c = {}
            _attn_q(nc, pools, d0c, Xres, W('dec', 'wqT', 0), 'd')  # fills ln1
            _mlp(nc, pools, Zres, W('enc', 'w1T', 1), W('enc', 'w2T', 1))
            _ln(nc, pools, Zres, 'e1ln2')
            _allgather_pair(nc, Zres, *agbufs[1])
            ZR = _remote_half(nc, pools, agbufs[1][1], Zres, 'z')
            if 'zfinal' in taps:
                tapfn('zfinal', Zres)

            # ======== decoder ========
            # d0 cross-attn: local k/v from Zres fills the AG1 wait
            d0c.update(_attn_kv(nc, pools, Zres, ZR, W('dec', 'wkT', 0),
                                W('dec', 'wvT', 0), 'd'))
            _attn_core(nc, pools, d0c, Xres, W('dec', 'woT', 0), None, 'd')
            _ln(nc, pools, Xres, 'd0ln2')
            # d1 cross k/v needs only Z: fills d0ln2 + mlp region ('e' tags
            # are free now -- encoder is done)
            d1c = _attn_kv(nc, pools, Zres, ZR, W('dec', 'wkT', 1),
                           W('dec', 'wvT', 1), 'e')
            _mlp(nc, pools, Xres, W('dec', 'w1T', 0), W('dec', 'w2T', 0))
            _ln(nc, pools, Xres, 'd0ln3')
            # d1 self-attn: q + local k/v only need the local stream
            d1s = {}
            _attn_q(nc, pools, d1s, Xres, W('dec', 'wqT', 1), 'd')
            _allgather_pair(nc, Xres, *agbufs[2])
            XR1 = _remote_half(nc, pools, agbufs[2][1], Xres, 'x')
            d1s.update(_attn_kv(nc, pools, Xres, XR1, W('dec', 'wkT', 1),
                                W('dec', 'wvT', 1), 'd'))
            _attn_core(nc, pools, d1s, Xres, W('dec', 'woT', 1), msk, 'd')
            _ln(nc, pools, Xres, 'd1ln1')
            _attn_q(nc, pools, d1c, Xres, W('dec', 'wqT', 1), 'e')
            _attn_core(nc, pools, d1c, Xres, W('dec', 'woT', 1), None, 'e')
            _ln(nc, pools, Xres, 'd1ln2')
            _mlp(nc, pools, Xres, W('dec', 'w1T', 1), W('dec', 'w2T', 1))
            _ln(nc, pools, Xres, 'd1ln3')
            _allgather_pair(nc, Xres, *agbufs[3])
            lctx.close()

            # ============= unembed phase (slot order: local|remote) =========
            # logits^T [128q, 500v] per (slot, chunk); slots 0,1 read the
            # local stream and run during the final AllGather.
            usb = ctx.enter_context(tc.tile_pool(name='usb', bufs=1))
            wup = ctx.enter_context(tc.tile_pool(name='wup', bufs=2))
            upsum = ctx.enter_context(tc.tile_pool(name='upsum', bufs=8,
                                                   space='PSUM'))
            NPRE = 8

            expu = usb.tile([P, QT, UVC, UV], F16, tag='expu')
            dparts = usb.tile([P, QT, UVC], F32, tag='dparts')

            def uchunk(vc, slots, xl, xr):
                wt = wup.tile([P, KO, UV], F16, tag='wut')
                nc.sync.dma_start(wt[:], wuC[vc])
                for sl in slots:
                    src = xl if sl < 2 else xr
                    so = (sl % 2) * P
                    ps = upsum.tile([P, UV], F32, tag='ups')
                    for k in range(KO):
                        nc.tensor.matmul(ps[:], src[:, k, so:so + P],
                                         wt[:, k, :],
                                         start=(k == 0), stop=(k == KO - 1))
                    nc.scalar.activation(expu[:, sl, vc, :], ps[:], AF.Exp,
                                         accum_out=dparts[:, sl, vc:vc + 1])

            for vc in range(NPRE):
                uchunk(vc, (0, 1), Xres, None)      # pre-AG local slots
            XR2 = _remote_half(nc, pools, agbufs[3][1], Xres, 'x')
            for vc in range(NPRE):
                uchunk(vc, (2, 3), Xres, XR2)       # re-reads wuC[vc]
            for vc in range(NPRE, UVC):
                uchunk(vc, (0, 1, 2, 3), Xres, XR2)

            # deno = sum over vocab chunks (free dim). The pair exchange is
            # slot-aware: my slot sl covers the same global columns as the
            # partner's slot sl^2, so AllGather + permuted add (AllReduce
            # would sum mismatched columns).
            dloc = usb.tile([P, QT], F32, tag='dloc')
            nc.vector.tensor_reduce(dloc[:], dparts[:],
                                    mybir.AxisListType.X, ALU.add)
            nc.gpsimd.dma_start(arin[:], dloc[:])
            nc.gpsimd.collective_compute(
                "AllGather", ALU.bypass,
                ins=[arin[:]], outs=[arout[:]], replica_groups=PAIR_GROUPS)
            dga = usb.tile([P, 2, QT], F32, tag='dga')
            nc.gpsimd.dma_start(dga[:], arout[:].rearrange('r p q -> p r q'))
            dsum = usb.tile([P, QT], F32, tag='dsum')
            nc.vector.tensor_tensor(dsum[:], dga[:, 0, :], dga[:, 1, :],
                                    ALU.add)
            doth = usb.tile([P, QT], F32, tag='doth')
            nc.vector.tensor_tensor(doth[:], dsum[:], dloc[:], ALU.subtract)
            dtot = usb.tile([P, QT], F32, tag='dtot')
            nc.vector.tensor_tensor(dtot[:, 0:2], dloc[:, 0:2], doth[:, 2:4],
                                    ALU.add)
            nc.vector.tensor_tensor(dtot[:, 2:4], dloc[:, 2:4], doth[:, 0:2],
                                    ALU.add)
            rinv = usb.tile([P, QT], F32, tag='rinv')
            nc.vector.reciprocal_approx_fast(rinv[:], dtot[:])
            rsc = usb.tile([P, QT], F32, tag='rsc')
            nc.vector.tensor_scalar_mul(rsc[:], rinv[:], OSCALE)
            r16 = usb.tile([P, QT], F16, tag='r16')
            nc.vector.tensor_copy(r16[:], rsc[:])
            if 'deno' in taps:
                d = nc.dram_tensor('tap_deno', [P, QT], F32,
                                   kind="ExternalOutput")
                nc.sync.dma_start(d[:], dtot[:])
            # normalize expu in place (wide sub-ops on ACT + DVE), then one
            # big contiguous DMA per slot (32KB per partition line)
            dmae = [nc.sync, nc.gpsimd, nc.scalar, nc.sync]
            NSUB = 4
            W8 = UVC // NSUB
            for sl in range(QT):
                for j in range(NSUB):
                    seg = expu[:, sl, j * W8:(j + 1) * W8, :]
                    if j % 2 == 0:
                        nc.scalar.activation(seg, seg, AF.Copy,
                                             scale=rsc[:, sl:sl + 1])
                    else:
                        nc.vector.tensor_tensor(
                            seg, seg,
                            r16[:, sl:sl + 1, None].to_broadcast(
                                (P, W8, UV)), ALU.mult)
                dmae[sl].dma_start(outp[sl], expu[:, sl, :, :])

    nc.compile()
    return nc


# ----------------------------------------------------------------------------
# host-side prep
# ----------------------------------------------------------------------------

def _to_kimaj(a):
    """[K, M] -> [128, K//128, M] with K = ko*128 + ki."""
    K, M = a.shape
    return np.ascontiguousarray(
        a.reshape(K // P, P, M).transpose(1, 0, 2))


def prep_inputs(inputs):
    f = lambda k: np.asarray(inputs[k], dtype=np.float32)
    We, Wp, Wu = f('We'), f('Wp'), f('Wu')
    x = np.asarray(inputs['x']).astype(np.int64)
    z = np.asarray(inputs['z']).astype(np.int64)

    shared = {}
    for pfx, nl in (('enc', LENC), ('dec', LDEC)):
        Wq, Wk, Wv = f(pfx + '_Wq'), f(pfx + '_Wk'), f(pfx + '_Wv')
        Wo, W1, W2 = f(pfx + '_Wo'), f(pfx + '_W1'), f(pfx + '_W2')
        wq, wk, wv, wo, w1, w2 = [], [], [], [], [], []
        for l in range(nl):
            qa = Wq[l].transpose(2, 0, 1).reshape(DE, H * DA) * (DA ** -0.5)
            ka = Wk[l].transpose(2, 0, 1).reshape(DE, H * DA)
            va = Wv[l].transpose(2, 0, 1).reshape(DE, H * DA)
            wq.append(_to_kimaj(qa)); wk.append(_to_kimaj(ka))
            wv.append(_to_kimaj(va))
            wo.append(_to_kimaj(Wo[l].T))
            w1.append(_to_kimaj(W1[l].T))
            w2.append(_to_kimaj(W2[l].T))
        shared[f'{pfx}_wqT'] = np.stack(wq).astype(np.float16)
        shared[f'{pfx}_wkT'] = np.stack(wk).astype(np.float16)
        shared[f'{pfx}_wvT'] = np.stack(wv).astype(np.float16)
        shared[f'{pfx}_woT'] = np.stack(wo).astype(np.float16)
        shared[f'{pfx}_w1T'] = np.stack(w1).astype(np.float16)
        shared[f'{pfx}_w2T'] = np.stack(w2).astype(np.float16)

    # unembedding, pair-split over vocab, chunked contiguous:
    # wuC[vc, ki, ko, v] = Wu[h*NVH + vc*UV + v, ko*128 + ki]
    wuC_h = []
    for h in range(2):
        wh = Wu[h * NVH:(h + 1) * NVH]                 # [16000, 1024]
        a = wh.reshape(UVC, UV, KO, P).transpose(0, 3, 2, 1)
        wuC_h.append(np.ascontiguousarray(a).astype(np.float16))

    pos = Wp[:L]  # [512, 1024]
    in_maps = []
    for c in range(N_CORES):
        b, h = c // 2, c % 2
        m = dict(shared)
        for nm, tok in (('z0', z[b]), ('x0', x[b])):
            E0 = (We[tok] + pos).T.astype(np.float32)      # [1024, 512]
            E0k = E0.reshape(KO, P, L)                     # [ko, ki, p]
            loc = E0k[:, :, h * LL:(h + 1) * LL].transpose(1, 0, 2)
            m[nm + '_loc16'] = np.ascontiguousarray(loc).astype(np.float16)
            rr = (1 - h)
            rem = E0k[:, :, rr * LL:(rr + 1) * LL].transpose(1, 0, 2)
            m[nm + '_rem16'] = np.ascontiguousarray(rem).astype(np.float16)
        kglob = np.arange(L)[:, None]
        qglob = (h * LL + np.arange(LL))[None, :]
        msk = (kglob <= qglob).astype(np.float16)          # [512, 256]
        mm4 = msk.reshape(4, P, LL)
        # key tiles in this core's [local | remote] order
        perm = [2 * h, 2 * h + 1, 2 * (1 - h), 2 * (1 - h) + 1]
        m['mask_self'] = np.ascontiguousarray(
            mm4[perm].transpose(1, 0, 2))
        m['wuC'] = wuC_h[h]
        in_maps.append(m)
    return in_maps


def assemble(results):
    """results: per-core dicts with 'outp' [QT(slot), P, UVC, UV] fp16."""
    out = np.empty((4, NV, L), dtype=np.float32)
    for c, r in enumerate(results):
        b, h = c // 2, c % 2
        o = np.asarray(r['outp'], dtype=np.float32) / OSCALE
        # slot s covers global query tile g: local slots first
        for sl in range(QT):
            g = 2 * h + sl if sl < 2 else 2 * (1 - h) + (sl - 2)
            # o[sl] : [P(qi), UVC, UV] -> [16000, 128]
            blk = o[sl].transpose(1, 2, 0).reshape(NVH, P)
            out[b, h * NVH:(h + 1) * NVH, g * P:(g + 1) * P] = blk
    return out


def run(inputs, trace=False, taps=(), trace_kwargs=None):
    key = ('prog', tuple(sorted(taps)))
    if key not in _CACHE:
        _CACHE[key] = build_program(taps=taps)
    nc = _CACHE[key]
    in_maps = prep_inputs(inputs)
    res = run_bass_kernel_spmd(nc, in_maps, list(range(N_CORES)),
                               trace=trace, **(trace_kwargs or {}))
    return res


def kernel(**inputs):
    res = run(inputs, trace=False)
    return assemble(res.results)


# revision 22
# speedup vs baseline: 1.6555x; 1.0246x over previous
"""Trainium2 Bass kernel for nn_EDTransformer (encoder-decoder transformer).

Sharding: 8 cores = 4 batch items x 2 sequence halves.
 - Each core owns (item b, half h): computes Q/scores/AV/Wo/MLP/LN for its
   256 local positions, K/V redundantly for the full 512 positions.
 - One 2-core AllGather of fp16 activations per layer (pairs share an item).
 - Unembedding: pair-split over vocab (16000 rows/core), transposed layout
   (tokens on partitions): softmax denominator via activation accum_out,
   one tiny pair AllReduce, per-partition normalize on scalar/vector.
 - PE executes in scheduler order, so independent matmuls (K/V projections,
   decoder fill work) are emitted inside the gaps left by LN/softmax chains.
Dtypes: fp16 residual stream + matmul operands, fp32 PSUM/stats, fp16 output
 scaled by 2^8 (host divides it back out).
"""
import os
import sys

sys.path.insert(0, '/opt/trn_rl_repo')
import numpy as np

import concourse.bacc as bacc
import concourse.tile as tile
import concourse.mybir as mybir
from concourse.bass_utils import run_bass_kernel_spmd

DT = mybir.dt
F16 = DT.float16
F32 = DT.float32
ALU = mybir.AluOpType
AF = mybir.ActivationFunctionType

N_CORES = 8
P = 128
DE = 1024          # model dim        (8 ptiles)
KO = DE // P       # 8
DMLP = 4096        # mlp dim          (32 ptiles)
MO = DMLP // P     # 32
H = 16             # heads
DA = 64            # attn dim per head
L = 512            # sequence length
LL = 256           # local positions per core
KT = L // P        # 4 key tiles
NV = 32000
NVH = NV // 2      # 16000 vocab rows per core (pair-split)
UV = 500           # vocab chunk width
UVC = NVH // UV    # 32 chunks
QT = L // P        # 4 query tiles of 128
LENC = 2
LDEC = 2
EPS = 1e-5
OSCALE = 256.0     # output scale (keeps fp16 out of subnormals)

PAIR_GROUPS = [[0, 1], [2, 3], [4, 5], [6, 7]]

_CACHE = {}


# ----------------------------------------------------------------------------
# device program
# ----------------------------------------------------------------------------

def _attn_kv(nc, pools, loc16, rem16, wk_d, wv_d, tg):
    """K/V projections in [local | remote] column order.

    loc16/rem16: [128, KO, LL] fp16 halves of the kv-input stream. The
    local-half matmuls only need loc16, so they execute during the
    AllGather that produces rem16. Key order is core-dependent; the host
    permutes the causal mask to match (attention is key-permutation
    invariant otherwise).
    """
    sb = pools['att']
    p256 = pools['p256']
    p512 = pools['p512']
    k16 = sb.tile([P, KO, 2, LL], F16, tag=tg + 'k16')
    vt16 = sb.tile([P, KT, H * DA], F16, tag=tg + 'vt16')
    wts = []
    for prp in range(0, KO, 2):
        wt = pools['wkp'].tile([P, KO, 2 * P], F16, tag='wkt')
        nc.sync.dma_start(wt[:], wk_d[:, :, prp * P:(prp + 2) * P])
        wts.append(wt)
        for p2 in range(2):
            ps = p256.tile([P, LL], F32, tag='p256', name='psk')
            for k in range(KO):
                nc.tensor.matmul(ps[:], wt[:, k, p2 * P:(p2 + 1) * P],
                                 loc16[:, k, :], start=(k == 0),
                                 stop=(k == KO - 1))
            nc.vector.tensor_copy(k16[:, prp + p2, 0, :], ps[:])
    wvts = []
    for nch in range(2):
        wt = pools['wvp'].tile([P, KO, 512], F16, tag='wvt')
        nc.sync.dma_start(wt[:], wv_d[:, :, nch * 512:(nch + 1) * 512])
        wvts.append(wt)
        for kt in range(2):
            ps = p512.tile([P, 512], F32, tag='p512')
            for k in range(KO):
                nc.tensor.matmul(ps[:], loc16[:, k, kt * P:(kt + 1) * P],
                                 wt[:, k, :],
                                 start=(k == 0), stop=(k == KO - 1))
            nc.vector.tensor_copy(vt16[:, kt, nch * 512:(nch + 1) * 512],
                                  ps[:])
    # remote half (blocks until rem16 is reconstructed from the AllGather)
    for prp in range(0, KO, 2):
        wt = wts[prp // 2]
        for p2 in range(2):
            ps = p256.tile([P, LL], F32, tag='p256', name='psk')
            for k in range(KO):
                nc.tensor.matmul(ps[:], wt[:, k, p2 * P:(p2 + 1) * P],
                                 rem16[:, k, :], start=(k == 0),
                                 stop=(k == KO - 1))
            nc.vector.tensor_copy(k16[:, prp + p2, 1, :], ps[:])
    for nch in range(2):
        wt = wvts[nch]
        for kt in range(2):
            ps = p512.tile([P, 512], F32, tag='p512')
            for k in range(KO):
                nc.tensor.matmul(ps[:], rem16[:, k, kt * P:(kt + 1) * P],
                                 wt[:, k, :],
                                 start=(k == 0), stop=(k == KO - 1))
            nc.vector.tensor_copy(vt16[:, 2 + kt, nch * 512:(nch + 1) * 512],
                                  ps[:])
    return dict(k16=k16, vt16=vt16)


def _attn_q(nc, pools, st, qin16, wq_d, tg):
    """Q projection from the local stream (adds to state st)."""
    sb = pools['att']
    p256 = pools['p256']
    q16 = sb.tile([P, KO, LL], F16, tag='q16')
    for prp in range(0, KO, 2):
        wt = pools['wqp'].tile([P, KO, 2 * P], F16, tag='wqt')
        nc.sync.dma_start(wt[:], wq_d[:, :, prp * P:(prp + 2) * P])
        for p2 in range(2):
            ps = p256.tile([P, LL], F32, tag='p256')
            for k in range(KO):
                nc.tensor.matmul(ps[:], wt[:, k, p2 * P:(p2 + 1) * P],
                                 qin16[:, k, :], start=(k == 0),
                                 stop=(k == KO - 1))
            nc.vector.tensor_copy(q16[:, prp + p2, :], ps[:])
    st['q16'] = q16


def _attn_core(nc, pools, st, Eres16, wo_d, mask, tg):
    """scores -> exp -> (mask) -> deno/recip -> AV -> y -> Wo += residual.

    Software-pipelined: group pr's scores are emitted before group pr-1's
    deno/AV so the PE never waits on the exp activation latency.
    """
    sb = pools['att']
    p256 = pools['p256']
    p512 = pools['p512']
    ones = pools['ones']
    q16, k16, vt16 = st['q16'], st['k16'], st['vt16']

    y16 = sb.tile([P, KO, LL], F16, tag='y16')
    expgs = {}

    def scores(pr):
        expg = pools['attg'].tile([P, 2, KT, LL], F16, tag='expg')
        expgs[pr] = expg
        for h2 in range(2):
            hp = h2 * DA
            for ktp in range(2):
                ps = p512.tile([P, 2, LL], F32, tag='p512')
                for j in range(2):
                    kt = 2 * ktp + j
                    nc.tensor.matmul(ps[:, j, :],
                                     k16[hp:hp + DA, pr, kt // 2,
                                         (kt % 2) * P:(kt % 2 + 1) * P],
                                     q16[hp:hp + DA, pr, :],
                                     start=True, stop=True)
                nc.scalar.activation(expg[:, h2, 2 * ktp:2 * ktp + 2, :],
                                     ps[:], AF.Exp)
        if mask is not None:
            for kt in range(KT):
                nc.vector.tensor_tensor(
                    expg[:, :, kt, :], expg[:, :, kt, :],
                    mask[:, kt, None, :].to_broadcast((P, 2, LL)),
                    ALU.mult)

    def av(pr):
        hA = 2 * pr
        expg = expgs.pop(pr)
        pd = p512.tile([P, 2, LL], F32, tag='p512')
        for kt in range(KT):
            nc.tensor.matmul(pd[:], ones[:, :], expg[:, :, kt, :],
                             start=(kt == 0), stop=(kt == KT - 1))
        ysc = pools['yscp'].tile([P, 2, LL], F32, tag='ysc')
        nc.vector.reciprocal_approx_fast(ysc[:], pd[:])
        ps = p256.tile([P, LL], F32, tag='p256')
        for kt in range(KT):
            nc.tensor.matmul(ps[:DA, :], vt16[:, kt, hA * DA:(hA + 1) * DA],
                             expg[:, 0, kt, :], start=(kt == 0),
                             stop=(kt == KT - 1), tile_position=(0, 0))
            nc.tensor.matmul(ps[DA:, :], vt16[:, kt, (hA + 1) * DA:(hA + 2) * DA],
                             expg[:, 1, kt, :], start=(kt == 0),
                             stop=(kt == KT - 1), tile_position=(0, DA))
        nc.vector.tensor_tensor(y16[:DA, pr, :], ps[:DA, :], ysc[:DA, 0, :],
                                ALU.mult)
        nc.vector.tensor_tensor(y16[DA:, pr, :], ps[DA:, :], ysc[DA:, 1, :],
                                ALU.mult)

    scores(0)
    for pr in range(1, KO):
        scores(pr)
        av(pr - 1)
    av(KO - 1)

    # Wo -> accumulate into residual
    for dt in range(0, KO, 2):
        wt = pools['wop'].tile([P, KO, 2 * P], F16, tag='wot')
        nc.sync.dma_start(wt[:], wo_d[:, :, dt * P:(dt + 2) * P])
        for d2 in range(2):
            ps = p256.tile([P, LL], F32, tag='p256')
            for k in range(KO):
                nc.tensor.matmul(ps[:], wt[:, k, d2 * P:(d2 + 1) * P],
                                 y16[:, k, :], start=(k == 0),
                                 stop=(k == KO - 1))
            nc.vector.tensor_tensor(Eres16[:, dt + d2, :],
                                    Eres16[:, dt + d2, :], ps[:],
                                    ALU.add)


def _mlp(nc, pools, Eres16, w1_dram, w2_dram):
    p256 = pools['p256']
    h16 = pools['mlp'].tile([P, MO, LL], F16, tag='h16')
    for mtp in range(0, MO, 2):
        w1t = pools['w1p'].tile([P, KO, 2 * P], F16, tag='w1t')
        nc.sync.dma_start(w1t[:], w1_dram[:, :, mtp * P:(mtp + 2) * P])
        for m2 in range(2):
            ps = p256.tile([P, LL], F32, tag='p256')
            for k in range(KO):
                nc.tensor.matmul(ps[:], w1t[:, k, m2 * P:(m2 + 1) * P],
                                 Eres16[:, k, :],
                                 start=(k == 0), stop=(k == KO - 1))
            nc.scalar.activation(h16[:, mtp + m2, :], ps[:], AF.Relu)
    for dt in range(KO):
        w2t = pools['w2p'].tile([P, MO, P], F16, tag='w2t')
        nc.sync.dma_start(w2t[:], w2_dram[:, :, dt * P:(dt + 1) * P])
        ps = p256.tile([P, LL], F32, tag='p256')
        for k in range(MO):
            nc.tensor.matmul(ps[:], w2t[:, k, :], h16[:, k, :],
                             start=(k == 0), stop=(k == MO - 1))
        nc.vector.tensor_tensor(Eres16[:, dt, :], Eres16[:, dt, :], ps[:],
                                ALU.add)


def _ln(nc, pools, Eres16, name):
    """In-place layernorm over features on the fp16 stream."""
    p256 = pools['p256']
    ones = pools['ones']
    stat = pools['stat']

    sq16 = pools['lnp'].tile([P, KO, LL], F16, tag='sq16')
    nc.gpsimd.tensor_tensor(sq16[:], Eres16[:], Eres16[:], ALU.mult)
    pss = p256.tile([P, LL], F32, tag='p256')
    psq = p256.tile([P, LL], F32, tag='p256')
    for k in range(KO):
        nc.tensor.matmul(pss[:], ones[:, :], Eres16[:, k, :],
                         start=(k == 0), stop=(k == KO - 1))
    for k in range(KO):
        nc.tensor.matmul(psq[:], ones[:, :], sq16[:, k, :],
                         start=(k == 0), stop=(k == KO - 1))
    mean = stat.tile([P, LL], F32, tag='mean')
    nc.vector.tensor_scalar_mul(mean[:], pss[:], 1.0 / DE)
    msq = stat.tile([P, LL], F32, tag='msq')
    nc.vector.scalar_tensor_tensor(msq[:], mean[:], float(DE) / (DE - 1),
                                   mean[:], ALU.mult, ALU.mult)
    var = stat.tile([P, LL], F32, tag='var')
    nc.vector.scalar_tensor_tensor(var[:], psq[:], 1.0 / (DE - 1),
                                   msq[:], ALU.mult, ALU.subtract)
    std = stat.tile([P, LL], F32, tag='std')
    nc.scalar.activation(std[:], var[:], AF.Sqrt, bias=pools['eps128'])
    inv = stat.tile([P, LL], F32, tag='inv')
    nc.vector.reciprocal_approx_fast(inv[:], std[:])
    inv16 = stat.tile([P, LL], F16, tag='inv16')
    nc.vector.tensor_copy(inv16[:], inv[:])
    negms = stat.tile([P, LL], F16, tag='negms')
    nc.vector.scalar_tensor_tensor(negms[:], mean[:], -1.0, inv[:],
                                   ALU.mult, ALU.mult)
    nc.vector.tensor_tensor(
        Eres16[:], Eres16[:],
        inv16[:, None, :].to_broadcast((P, KO, LL)), ALU.mult)
    nc.vector.tensor_tensor(
        Eres16[:], Eres16[:],
        negms[:, None, :].to_broadcast((P, KO, LL)), ALU.add)
    tp = pools.get('tapfn')
    if tp:
        tp(f'{name}_out', Eres16)


def _remote_half(nc, pools, agout, loc16, tag):
    """rem = (agout[0] + agout[1]) - loc, exact in fp32 (fp16 inputs)."""
    sp = pools['stream']
    xa = sp.tile([P, KO, LL], F16, tag='gxa', name='xa')
    xb = sp.tile([P, KO, LL], F16, tag='gxb', name='xb')
    nc.gpsimd.dma_start(xa[:], agout[0])
    nc.gpsimd.dma_start(xb[:], agout[1])
    s32 = sp.tile([P, KO, LL], F32, tag='gs32', name='s32')
    nc.vector.tensor_tensor(s32[:], xa[:], xb[:], ALU.add)
    rem = sp.tile([P, KO, LL], F16, tag=tag + 'rem', name='rem')
    nc.vector.tensor_tensor(rem[:], s32[:], loc16[:], ALU.subtract)
    return rem


def _allgather_pair(nc, e16loc, agin, agout):
    """e16loc [128, KO, LL] -> pair AllGather into agout [2, 128, KO, LL]."""
    nc.gpsimd.dma_start(agin[:], e16loc[:])
    nc.gpsimd.collective_compute(
        "AllGather", ALU.bypass,
        ins=[agin[:]], outs=[agout[:]],
        replica_groups=PAIR_GROUPS)


def build_program(taps=()):
    taps = set(taps)
    nc = bacc.Bacc("TRN2", target_bir_lowering=False, debug=False,
                   num_devices=N_CORES)

    # ---- dram inputs ----
    din = {}
    def dram_in(nm, shape, dt=F16):
        din[nm] = nc.dram_tensor(nm, list(shape), dt, kind="ExternalInput")
        return din[nm]

    z0l16 = dram_in('z0_loc16', [P, KO, LL])
    x0l16 = dram_in('x0_loc16', [P, KO, LL])
    z0r16 = dram_in('z0_rem16', [P, KO, LL])
    x0r16 = dram_in('x0_rem16', [P, KO, LL])
    mask_self = dram_in('mask_self', [P, 4, LL])
    for pfx, nl in (('enc', LENC), ('dec', LDEC)):
        for w in ('wqT', 'wkT', 'wvT', 'woT'):
            dram_in(f'{pfx}_{w}', [nl, P, KO, DE])
        dram_in(f'{pfx}_w1T', [nl, P, KO, DMLP])
        dram_in(f'{pfx}_w2T', [nl, P, MO, DE])
    wuC = dram_in('wuC', [UVC, P, KO, UV])

    outp = nc.dram_tensor('outp', [QT, P, UVC, UV], F16, kind="ExternalOutput")

    # internal dram for collectives (one pair per AllGather: a core that
    # runs ahead would otherwise clobber its partner's previous AG output)
    agbufs = [(nc.dram_tensor(f'agin{i}', [P, KO, LL], F16),
               nc.dram_tensor(f'agout{i}', [2, P, KO, LL], F16))
              for i in range(LENC + LDEC)]
    arin = nc.dram_tensor('arin', [P, QT], F32)
    arout = nc.dram_tensor('arout', [2, P, QT], F32)
    wuin = nc.dram_tensor('wuin', [1, 64], F16)
    wuout = nc.dram_tensor('wuout', [2, 1, 64], F16)

    import contextlib
    with tile.TileContext(nc) as tc, contextlib.ExitStack() as octx:
        const = octx.enter_context(tc.tile_pool(name='const', bufs=1))
        ones = const.tile([P, P], F16)
        nc.vector.memset(ones[:], 1.0)
        eps128 = const.tile([P, 1], F32)
        nc.vector.memset(eps128[:], EPS)
        msk = const.tile([P, 4, LL], F16)
        nc.sync.dma_start(msk[:], mask_self[:])
        XFu = const.tile([P, KO, L], F16)
        # warm up the collective path with a tiny AllGather (overlaps compute)
        wu16 = const.tile([1, 64], F16)
        nc.vector.memset(wu16[:], 0.0)
        nc.gpsimd.dma_start(wuin[:], wu16[:])
        nc.gpsimd.collective_compute(
            "AllGather", ALU.bypass, ins=[wuin[:]], outs=[wuout[:]],
            replica_groups=PAIR_GROUPS)

        # ================= layer phase =================
        with contextlib.ExitStack() as ctx:
            stream = ctx.enter_context(tc.tile_pool(name='stream', bufs=1))
            stat = ctx.enter_context(tc.tile_pool(name='stat', bufs=1))
            lctx = ctx.enter_context(contextlib.ExitStack())
            att = lctx.enter_context(tc.tile_pool(name='att', bufs=1))
            attg = lctx.enter_context(tc.tile_pool(name='attg', bufs=2))
            mlpp = lctx.enter_context(tc.tile_pool(name='mlpp', bufs=1))
            lnp = lctx.enter_context(tc.tile_pool(name='lnp', bufs=1))
            yscp = lctx.enter_context(tc.tile_pool(name='yscp', bufs=2))
            wqp = lctx.enter_context(tc.tile_pool(name='wqp', bufs=2))
            wkp = lctx.enter_context(tc.tile_pool(name='wkp', bufs=4))
            wvp = lctx.enter_context(tc.tile_pool(name='wvp', bufs=2))
            wop = lctx.enter_context(tc.tile_pool(name='wop', bufs=2))
            w1p = lctx.enter_context(tc.tile_pool(name='w1p', bufs=5))
            w2p = lctx.enter_context(tc.tile_pool(name='w2p', bufs=2))
            p256 = lctx.enter_context(tc.tile_pool(name='p256', bufs=3,
                                                   space='PSUM'))
            p512 = lctx.enter_context(tc.tile_pool(name='p512', bufs=4,
                                                   space='PSUM'))

            pools = dict(att=att, attg=attg, mlp=mlpp, lnp=lnp, p256=p256,
                         p512=p512, stat=stat, yscp=yscp, ones=ones,
                         eps128=eps128[:], wqp=wqp, wkp=wkp,
                         wvp=wvp, wop=wop, w1p=w1p, w2p=w2p, stream=stream)

            def tapfn(nm, t):
                if nm not in taps:
                    return
                d = nc.dram_tensor('tap_' + nm, list(t.shape),
                                   t.dtype, kind="ExternalOutput")
                nc.sync.dma_start(d[:], t[:])
            pools['tapfn'] = tapfn

            # ======== streams ========
            Zres = stream.tile([P, KO, LL], F16, tag='zres')
            nc.sync.dma_start(Zres[:], z0l16[:])
            ZR = stream.tile([P, KO, LL], F16, tag='zrem')
            nc.sync.dma_start(ZR[:], z0r16[:])
            Xres = stream.tile([P, KO, LL], F16, tag='xres')
            nc.sync.dma_start(Xres[:], x0l16[:])
            XR = stream.tile([P, KO, LL], F16, tag='xrem')
            nc.sync.dma_start(XR[:], x0r16[:])

            W = lambda p, w, l: din[f'{p}_{w}'][l]

            # ======== encoder, dec-l0 self-attn interleaved as PE filler ====
            e0 = _attn_kv(nc, pools, Zres, ZR, W('enc', 'wkT', 0),
                          W('enc', 'wvT', 0), 'e')
            _attn_q(nc, pools, e0, Zres, W('enc', 'wqT', 0), 'e')
            _attn_core(nc, pools, e0, Zres, W('enc', 'woT', 0), None, 'e')
            _ln(nc, pools, Zres, 'e0ln1')
            d0 = _attn_kv(nc, pools, Xres, XR, W('dec', 'wkT', 0),
                          W('dec', 'wvT', 0), 'd')          # fills e0ln1
            _mlp(nc, pools, Zres, W('enc', 'w1T', 0), W('enc', 'w2T', 0))
            _ln(nc, pools, Zres, 'e0ln2')
            _attn_q(nc, pools, d0, Xres, W('dec', 'wqT', 0), 'd')  # fills ln2
            _allgather_pair(nc, Zres, *agbufs[0])
            _attn_core(nc, pools, d0, Xres, W('dec', 'woT', 0), msk,
                       'd')                                 # fills AG0
            _ln(nc, pools, Xres, 'd0ln1')
            ZR = _remote_half(nc, pools, agbufs[0][1], Zres, 'z')

            e1 = _attn_kv(nc, pools, Zres, ZR, W('enc', 'wkT', 1),
                          W('enc', 'wvT', 1), 'e')
            _attn_q(nc, pools, e1, Zres, W('enc', 'wqT', 1), 'e')
            _attn_core(nc, pools, e1, Zres, W('enc', 'woT', 1), None, 'e')
            _ln(nc, pools, Zres, 'e1ln1')
            d0c = {}
            _attn_q(nc, pools, d0c, Xres, W('dec', 'wqT', 0), 'd')  # fills ln1
            _mlp(nc, pools, Zres, W('enc', 'w1T', 1), W('enc', 'w2T', 1))
            _ln(nc, pools, Zres, 'e1ln2')
            _allgather_pair(nc, Zres, *agbufs[1])
            ZR = _remote_half(nc, pools, agbufs[1][1], Zres, 'z')
            if 'zfinal' in taps:
                tapfn('zfinal', Zres)

            # ======== decoder ========
            # d0 cross-attn: local k/v from Zres fills the AG1 wait
            d0c.update(_attn_kv(nc, pools, Zres, ZR, W('dec', 'wkT', 0),
                                W('dec', 'wvT', 0), 'd'))
            _attn_core(nc, pools, d0c, Xres, W('dec', 'woT', 0), None, 'd')
            _ln(nc, pools, Xres, 'd0ln2')
            # d1 cross k/v needs only Z: fills d0ln2 + mlp region ('e' tags
            # are free now -- encoder is done)
            d1c = _attn_kv(nc, pools, Zres, ZR, W('dec', 'wkT', 1),
                           W('dec', 'wvT', 1), 'e')
            _mlp(nc, pools, Xres, W('dec', 'w1T', 0), W('dec', 'w2T', 0))
            _ln(nc, pools, Xres, 'd0ln3')
            # d1 self-attn: q + local k/v only need the local stream
            d1s = {}
            _attn_q(nc, pools, d1s, Xres, W('dec', 'wqT', 1), 'd')
            _allgather_pair(nc, Xres, *agbufs[2])
            XR1 = _remote_half(nc, pools, agbufs[2][1], Xres, 'x')
            d1s.update(_attn_kv(nc, pools, Xres, XR1, W('dec', 'wkT', 1),
                                W('dec', 'wvT', 1), 'd'))
            _attn_core(nc, pools, d1s, Xres, W('dec', 'woT', 1), msk, 'd')
            _ln(nc, pools, Xres, 'd1ln1')
            _attn_q(nc, pools, d1c, Xres, W('dec', 'wqT', 1), 'e')
            _attn_core(nc, pools, d1c, Xres, W('dec', 'woT', 1), None, 'e')
            _ln(nc, pools, Xres, 'd1ln2')
            _mlp(nc, pools, Xres, W('dec', 'w1T', 1), W('dec', 'w2T', 1))
            _ln(nc, pools, Xres, 'd1ln3')
            _allgather_pair(nc, Xres, *agbufs[3])
            lctx.close()

            # ============= unembed phase (slot order: local|remote) =========
            # logits^T [128q, 500v] per (slot, chunk); slots 0,1 read the
            # local stream and run during the final AllGather.
            usb = ctx.enter_context(tc.tile_pool(name='usb', bufs=1))
            wup = ctx.enter_context(tc.tile_pool(name='wup', bufs=3))
            upsum = ctx.enter_context(tc.tile_pool(name='upsum', bufs=8,
                                                   space='PSUM'))
            NPRE = 8

            expu = usb.tile([P, QT, UVC, UV], F16, tag='expu')
            dparts = usb.tile([P, QT, UVC], F32, tag='dparts')

            def uchunk(vc, slots, xl, xr):
                wt = wup.tile([P, KO, UV], F16, tag='wut')
                nc.sync.dma_start(wt[:], wuC[vc])
                for sl in slots:
                    src = xl if sl < 2 else xr
                    so = (sl % 2) * P
                    ps = upsum.tile([P, UV], F32, tag='ups')
                    for k in range(KO):
                        nc.tensor.matmul(ps[:], src[:, k, so:so + P],
                                         wt[:, k, :],
                                         start=(k == 0), stop=(k == KO - 1))
                    nc.scalar.activation(expu[:, sl, vc, :], ps[:], AF.Exp,
                                         accum_out=dparts[:, sl, vc:vc + 1])

            for vc in range(NPRE):
                uchunk(vc, (0, 1), Xres, None)      # pre-AG local slots
            XR2 = _remote_half(nc, pools, agbufs[3][1], Xres, 'x')
            for vc in range(NPRE):
                uchunk(vc, (2, 3), Xres, XR2)       # re-reads wuC[vc]
            for vc in range(NPRE, UVC):
                uchunk(vc, (0, 1, 2, 3), Xres, XR2)

            # deno = sum over vocab chunks (free dim). The pair exchange is
            # slot-aware: my slot sl covers the same global columns as the
            # partner's slot sl^2, so AllGather + permuted add (AllReduce
            # would sum mismatched columns).
            dloc = usb.tile([P, QT], F32, tag='dloc')
            nc.vector.tensor_reduce(dloc[:], dparts[:],
                                    mybir.AxisListType.X, ALU.add)
            nc.gpsimd.dma_start(arin[:], dloc[:])
            nc.gpsimd.collective_compute(
                "AllGather", ALU.bypass,
                ins=[arin[:]], outs=[arout[:]], replica_groups=PAIR_GROUPS)
            dga = usb.tile([P, 2, QT], F32, tag='dga')
            nc.gpsimd.dma_start(dga[:], arout[:].rearrange('r p q -> p r q'))
            dsum = usb.tile([P, QT], F32, tag='dsum')
            nc.vector.tensor_tensor(dsum[:], dga[:, 0, :], dga[:, 1, :],
                                    ALU.add)
            doth = usb.tile([P, QT], F32, tag='doth')
            nc.vector.tensor_tensor(doth[:], dsum[:], dloc[:], ALU.subtract)
            dtot = usb.tile([P, QT], F32, tag='dtot')
            nc.vector.tensor_tensor(dtot[:, 0:2], dloc[:, 0:2], doth[:, 2:4],
                                    ALU.add)
            nc.vector.tensor_tensor(dtot[:, 2:4], dloc[:, 2:4], doth[:, 0:2],
                                    ALU.add)
            rinv = usb.tile([P, QT], F32, tag='rinv')
            nc.vector.reciprocal_approx_fast(rinv[:], dtot[:])
            rsc = usb.tile([P, QT], F32, tag='rsc')
            nc.vector.tensor_scalar_mul(rsc[:], rinv[:], OSCALE)
            r16 = usb.tile([P, QT], F16, tag='r16')
            nc.vector.tensor_copy(r16[:], rsc[:])
            if 'deno' in taps:
                d = nc.dram_tensor('tap_deno', [P, QT], F32,
                                   kind="ExternalOutput")
                nc.sync.dma_start(d[:], dtot[:])
            # normalize expu in place (wide sub-ops on ACT + DVE), then one
            # big contiguous DMA per slot (32KB per partition line)
            dmae = [nc.sync, nc.gpsimd, nc.scalar, nc.sync]
            NSUB = 4
            W8 = UVC // NSUB
            for sl in range(QT):
                for j in range(NSUB):
                    seg = expu[:, sl, j * W8:(j + 1) * W8, :]
                    if j % 2 == 0:
                        nc.scalar.activation(seg, seg, AF.Copy,
                                             scale=rsc[:, sl:sl + 1])
                    else:
                        nc.vector.tensor_tensor(
                            seg, seg,
                            r16[:, sl:sl + 1, None].to_broadcast(
                                (P, W8, UV)), ALU.mult)
                dmae[sl].dma_start(outp[sl], expu[:, sl, :, :])

    nc.compile()
    return nc


# ----------------------------------------------------------------------------
# host-side prep
# ----------------------------------------------------------------------------

def _to_kimaj(a):
    """[K, M] -> [128, K//128, M] with K = ko*128 + ki."""
    K, M = a.shape
    return np.ascontiguousarray(
        a.reshape(K // P, P, M).transpose(1, 0, 2))


def prep_inputs(inputs):
    f = lambda k: np.asarray(inputs[k], dtype=np.float32)
    We, Wp, Wu = f('We'), f('Wp'), f('Wu')
    x = np.asarray(inputs['x']).astype(np.int64)
    z = np.asarray(inputs['z']).astype(np.int64)

    shared = {}
    for pfx, nl in (('enc', LENC), ('dec', LDEC)):
        Wq, Wk, Wv = f(pfx + '_Wq'), f(pfx + '_Wk'), f(pfx + '_Wv')
        Wo, W1, W2 = f(pfx + '_Wo'), f(pfx + '_W1'), f(pfx + '_W2')
        wq, wk, wv, wo, w1, w2 = [], [], [], [], [], []
        for l in range(nl):
            qa = Wq[l].transpose(2, 0, 1).reshape(DE, H * DA) * (DA ** -0.5)
            ka = Wk[l].transpose(2, 0, 1).reshape(DE, H * DA)
            va = Wv[l].transpose(2, 0, 1).reshape(DE, H * DA)
            wq.append(_to_kimaj(qa)); wk.append(_to_kimaj(ka))
            wv.append(_to_kimaj(va))
            wo.append(_to_kimaj(Wo[l].T))
            w1.append(_to_kimaj(W1[l].T))
            w2.append(_to_kimaj(W2[l].T))
        shared[f'{pfx}_wqT'] = np.stack(wq).astype(np.float16)
        shared[f'{pfx}_wkT'] = np.stack(wk).astype(np.float16)
        shared[f'{pfx}_wvT'] = np.stack(wv).astype(np.float16)
        shared[f'{pfx}_woT'] = np.stack(wo).astype(np.float16)
        shared[f'{pfx}_w1T'] = np.stack(w1).astype(np.float16)
        shared[f'{pfx}_w2T'] = np.stack(w2).astype(np.float16)

    # unembedding, pair-split over vocab, chunked contiguous:
    # wuC[vc, ki, ko, v] = Wu[h*NVH + vc*UV + v, ko*128 + ki]
    wuC_h = []
    for h in range(2):
        wh = Wu[h * NVH:(h + 1) * NVH]                 # [16000, 1024]
        a = wh.reshape(UVC, UV, KO, P).transpose(0, 3, 2, 1)
        wuC_h.append(np.ascontiguousarray(a).astype(np.float16))

    pos = Wp[:L]  # [512, 1024]
    in_maps = []
    for c in range(N_CORES):
        b, h = c // 2, c % 2
        m = dict(shared)
        for nm, tok in (('z0', z[b]), ('x0', x[b])):
            E0 = (We[tok] + pos).T.astype(np.float32)      # [1024, 512]
            E0k = E0.reshape(KO, P, L)                     # [ko, ki, p]
            loc = E0k[:, :, h * LL:(h + 1) * LL].transpose(1, 0, 2)
            m[nm + '_loc16'] = np.ascontiguousarray(loc).astype(np.float16)
            rr = (1 - h)
            rem = E0k[:, :, rr * LL:(rr + 1) * LL].transpose(1, 0, 2)
            m[nm + '_rem16'] = np.ascontiguousarray(rem).astype(np.float16)
        kglob = np.arange(L)[:, None]
        qglob = (h * LL + np.arange(LL))[None, :]
        msk = (kglob <= qglob).astype(np.float16)          # [512, 256]
        mm4 = msk.reshape(4, P, LL)
        # key tiles in this core's [local | remote] order
        perm = [2 * h, 2 * h + 1, 2 * (1 - h), 2 * (1 - h) + 1]
        m['mask_self'] = np.ascontiguousarray(
            mm4[perm].transpose(1, 0, 2))
        m['wuC'] = wuC_h[h]
        in_maps.append(m)
    return in_maps


def assemble(results):
    """results: per-core dicts with 'outp' [QT(slot), P, UVC, UV] fp16."""
    out = np.empty((4, NV, L), dtype=np.float32)
    for c, r in enumerate(results):
        b, h = c // 2, c % 2
        o = np.asarray(r['outp'], dtype=np.float32) / OSCALE
        # slot s covers global query tile g: local slots first
        for sl in range(QT):
            g = 2 * h + sl if sl < 2 else 2 * (1 - h) + (sl - 2)
            # o[sl] : [P(qi), UVC, UV] -> [16000, 128]
            blk = o[sl].transpose(1, 2, 0).reshape(NVH, P)
            out[b, h * NVH:(h + 1) * NVH, g * P:(g + 1) * P] = blk
    return out


def run(inputs, trace=False, taps=(), trace_kwargs=None):
    key = ('prog', tuple(sorted(taps)))
    if key not in _CACHE:
        _CACHE[key] = build_program(taps=taps)
    nc = _CACHE[key]
    in_maps = prep_inputs(inputs)
    res = run_bass_kernel_spmd(nc, in_maps, list(range(N_CORES)),
                               trace=trace, **(trace_kwargs or {}))
    return res


def kernel(**inputs):
    res = run(inputs, trace=False)
    return assemble(res.results)
